# revision 1
# baseline (speedup 1.0000x reference)
"""AGNNet (2-layer AGNN conv + linear head) distributed over 8 trn2 NeuronCores.

Strategy (graph/data parallel, per sharding hint):
  - nodes sharded by dst range: core c owns nodes [c*6250, (c+1)*6250)
  - host groups edges by dst, degree-sorts each core's local nodes (undone on
    output), pads each 128-node tile's in-edge lists to a per-group slot count
  - device: L1 matmul from host-transposed x shard; build a bf16 row table
    [h(16), inv_norm, bias, pad] per node; AllGather the table; per node-tile
    indirect-DMA gather of neighbor rows + DVE/ACT softmax; PE accumulates the
    weighted sum over slots; second conv identical; head matmul + log_softmax.
  - a dedicated all-zero table row (bias column = -1e30) backs padding slots so
    they vanish in the softmax and contribute 0 to the weighted sum.
  - the kernel is split into sequential TileContexts: walrus tracks SWDGE
    (indirect DMA) ring occupancy in a cumulative 16-bit semaphore value, so
    each context must stay under ~60k gather descriptors; the context-exit
    drain+sem-clear resets the counter.
"""

import math
from contextlib import ExitStack
from dataclasses import dataclass

import numpy as np


@dataclass
class Cfg:
    n_cores: int = 8
    n_nodes: int = 50000
    f_in: int = 2000
    nh: int = 16
    nc_out: int = 20
    group: int = 4          # node tiles per conv gather group
    row: int = 20           # table row elems: 16 h, [16]=invn, [17]=bias, 18-19 pad
    P: int = 128
    max_ctx_descs: int = 55000   # SWDGE descriptor budget per TileContext

    @property
    def npc_raw(self) -> int:       # real nodes per core
        return self.n_nodes // self.n_cores

    @property
    def npc(self) -> int:           # padded nodes per core (mult of 128)
        return ((self.npc_raw + self.P - 1) // self.P) * self.P

    @property
    def ntiles(self) -> int:
        return self.npc // self.P

    @property
    def kchunks(self) -> int:       # 128-row chunks of the padded f_in
        return (self.f_in + self.P - 1) // self.P

    @property
    def f_pad(self) -> int:
        return self.kchunks * self.P

    @property
    def pad_gid(self) -> int:       # index of the all-zero table row
        return self.n_cores * self.npc

    def groups(self, kg: list[int]):
        """[(tile0, gsz, K)] for the conv gather groups."""
        out = []
        t = 0
        for K in kg:
            gsz = min(self.group, self.ntiles - t)
            out.append((t, gsz, K))
            t += gsz
        assert t == self.ntiles
        return out

    def chunks(self, kg: list[int]):
        """Split groups into runs whose gather descriptors fit one context."""
        runs, cur, cum = [], [], 0
        for item in self.groups(kg):
            _, gsz, K = item
            d = self.P * gsz * K
            assert d <= self.max_ctx_descs
            if cum + d > self.max_ctx_descs and cur:
                runs.append(cur)
                cur, cum = [], 0
            cur.append(item)
            cum += d
        if cur:
            runs.append(cur)
        return runs

    @property
    def n_groups(self) -> int:
        return (self.ntiles + self.group - 1) // self.group


FULL = Cfg()
NEG_BIG = -1.0e30


# --------------------------------------------------------------------------
# host-side preprocessing
# --------------------------------------------------------------------------

def preprocess(cfg: Cfg, x: np.ndarray, edge_index: np.ndarray):
    """Shard + layout transform. Returns (per_core list of dicts, kg, order_c)."""
    P, NPC, NPCR = cfg.P, cfg.npc, cfg.npc_raw
    n, C = cfg.n_nodes, cfg.n_cores

    src = edge_index[0].astype(np.int64)
    dst = edge_index[1].astype(np.int64)
    loop = np.arange(n, dtype=np.int64)
    src = np.concatenate([src, loop])
    dst = np.concatenate([dst, loop])

    core_of_dst = dst // NPCR

    # pass 1: per-core degree sort -> rank of each node within its core
    order_c, rank_c, deg_c = [], [], []
    for c in range(C):
        m = core_of_dst == c
        ld = dst[m] - c * NPCR
        deg = np.bincount(ld, minlength=NPCR)
        order = np.argsort(deg, kind="stable")      # ascending degree
        rank = np.empty(NPCR, np.int64)
        rank[order] = np.arange(NPCR)
        order_c.append(order)
        rank_c.append(rank)
        deg_c.append(deg)

    # new global id after per-core permutation + padding
    new_gid = np.empty(n, np.int64)
    for c in range(C):
        new_gid[c * NPCR:(c + 1) * NPCR] = c * NPC + rank_c[c]

    # per-group K (max in-degree in the group, shared across cores for SPMD)
    ngrp = cfg.n_groups
    kg = np.zeros(ngrp, np.int64)
    for c in range(C):
        degp = np.zeros(NPC, np.int64)
        degp[rank_c[c]] = deg_c[c]
        for g in range(ngrp):
            t0 = g * cfg.group
            gsz = min(cfg.group, cfg.ntiles - t0)
            kmax = degp[t0 * P:(t0 + gsz) * P].max()
            kg[g] = max(kg[g], kmax)
    kg = [int(max(k, 1)) for k in kg]
    kmax_all = max(kg)

    per_core = []
    for c in range(C):
        m = core_of_dst == c
        ld = dst[m] - c * NPCR
        gs = new_gid[src[m]]
        er = rank_c[c][ld]                       # dst rank of each edge
        eo = np.argsort(er, kind="stable")
        er_s = er[eo]
        gs_s = gs[eo]
        starts = np.zeros(NPC + 1, np.int64)
        np.cumsum(np.bincount(er_s, minlength=NPC), out=starts[1:])
        k_e = np.arange(er_s.size) - starts[er_s]
        M = np.full((NPC, kmax_all), cfg.pad_gid, np.int32)
        M[er_s, k_e] = gs_s.astype(np.int32)

        # idx layout: per group a [128, gsz*K] block, col = t_in_g*K + k
        blocks = []
        for (t0, gsz, K) in cfg.groups(kg):
            blk = M[t0 * P:(t0 + gsz) * P, :K]           # [gsz*128, K]
            blk = blk.reshape(gsz, P, K).transpose(1, 0, 2).reshape(P, gsz * K)
            blocks.append(blk)
        idx = np.ascontiguousarray(np.concatenate(blocks, axis=1))

        # x shard: permuted, padded, transposed, f-padded
        xs = x[c * NPCR:(c + 1) * NPCR][order_c[c]]      # [NPCR, f_in]
        xt = np.zeros((cfg.f_pad, NPC), np.float32)
        xt[:cfg.f_in, :NPCR] = xs.T
        per_core.append({"xt": np.ascontiguousarray(xt), "idx": idx})

    return per_core, kg, order_c


# --------------------------------------------------------------------------
# device kernel builder
# --------------------------------------------------------------------------

def build_kernel(cfg: Cfg, kg: list[int], phases: str = "ABCDE"):
    import concourse.bacc as bacc
    import concourse.tile as tile
    from concourse import bass, mybir
    from concourse.masks import make_identity

    P = cfg.P
    NH, NCO, ROW = cfg.nh, cfg.nc_out, cfg.row
    NPC, NT, KC = cfg.npc, cfg.ntiles, cfg.kchunks
    NFULL = cfg.n_cores * NPC
    f32 = mybir.dt.float32
    bf16 = mybir.dt.bfloat16
    i32 = mybir.dt.int32
    AX = mybir.AxisListType.X
    OP = mybir.AluOpType
    AF = mybir.ActivationFunctionType
    slot_cols = sum(gsz * K for (_, gsz, K) in cfg.groups(kg))

    nc = bacc.Bacc("TRN2", target_bir_lowering=False, debug=False,
                   num_devices=cfg.n_cores)

    xt_d = nc.dram_tensor("xt", [cfg.f_pad, NPC], bf16, kind="ExternalInput")
    idx_d = nc.dram_tensor("idx", [P, slot_cols], i32, kind="ExternalInput")
    w1_d = nc.dram_tensor("w1p", [P, KC * NH], bf16, kind="ExternalInput")
    b1_d = nc.dram_tensor("b1r", [P, NH], f32, kind="ExternalInput")
    w4_d = nc.dram_tensor("w4r", [NH, NCO], f32, kind="ExternalInput")
    b4_d = nc.dram_tensor("b4r", [P, NCO], f32, kind="ExternalInput")
    be_d = nc.dram_tensor("beta3r", [P, 1], f32, kind="ExternalInput")
    out_d = nc.dram_tensor("out", [NPC, NCO], f32, kind="ExternalOutput")

    tabA_l = nc.dram_tensor("tabA_l", [NPC, ROW], bf16)
    tabA_f = nc.dram_tensor("tabA_f", [NFULL + 2, ROW], bf16, addr_space="Shared")
    tabB_l = nc.dram_tensor("tabB_l", [NPC, ROW], bf16)
    tabB_f = nc.dram_tensor("tabB_f", [NFULL + 2, ROW], bf16, addr_space="Shared")

    # persistent SBUF (survives across TileContexts)
    def sb(name, shape, dtype):
        return nc.alloc_sbuf_tensor(name, list(shape), dtype)

    ident = sb("ident", [P, P], bf16)
    ident_f = sb("identf", [P, P], f32)
    zeros = sb("zeros", [P, P], f32)
    w1_sb = sb("w1sb", [P, KC * NH], bf16)
    b1_sb = sb("b1sb", [P, NH], f32)
    w4_sb = sb("w4sb", [NH, NCO], f32)
    b4_sb = sb("b4sb", [P, NCO], f32)
    be_sb = sb("besb", [P, 1], f32)
    h_sb = sb("hsb", [P, NT * NH], f32)
    rows_sb = sb("rowssb", [P, NT * ROW], bf16)
    sq_sb = sb("sqsb", [P, NT * NH], f32)
    ss_sb = sb("sssb", [P, NT], f32)
    inv_sb = sb("invsb", [P, NT], f32)

    def epilogue_rows(tab_local):
        """h_sb -> inv norm -> rows_sb -> DMA to tab_local."""
        h3v = h_sb.ap().rearrange("p (t j) -> p t j", t=NT)
        nc.vector.tensor_mul(sq_sb.ap(), h_sb.ap(), h_sb.ap())
        nc.vector.reduce_sum(
            ss_sb.ap(), sq_sb.ap().rearrange("p (t j) -> p t j", t=NT), axis=AX)
        nc.scalar.sqrt(ss_sb.ap(), ss_sb.ap())
        nc.vector.tensor_scalar_add(ss_sb.ap(), ss_sb.ap(), 1.0e-12)
        nc.vector.reciprocal(inv_sb.ap(), ss_sb.ap())
        rv = rows_sb.ap().rearrange("p (t j) -> p t j", t=NT)
        nc.vector.tensor_copy(rv[:, :, 0:16], h3v)
        nc.vector.tensor_copy(rv[:, :, 16], inv_sb.ap())
        nc.vector.tensor_copy(rv[:, :, 17], zeros.ap()[:, 0:NT])
        nc.sync.dma_start(
            out=tab_local[:, :].rearrange("(t p) j -> p t j", p=P),
            in_=rv)

    def allgather(tab_local, tab_full):
        nc.gpsimd.collective_compute(
            "AllGather", OP.bypass,
            replica_groups=[list(range(cfg.n_cores))],
            ins=[tab_local.ap().opt()],
            outs=[tab_full.ap()[0:NFULL, :].opt()])

    # ---------------- phase A: consts, L1, table A, AG1 --------------------
    with tile.TileContext(nc) as tc:
        make_identity(nc, ident.ap())
        make_identity(nc, ident_f.ap())
        nc.gpsimd.memset(zeros.ap(), 0.0)
        nc.gpsimd.memset(rows_sb.ap(), 0.0)
        nc.sync.dma_start(out=w1_sb.ap(), in_=w1_d[:, :])
        nc.sync.dma_start(out=b1_sb.ap(), in_=b1_d[:, :])
        nc.sync.dma_start(out=w4_sb.ap(), in_=w4_d[:, :])
        nc.sync.dma_start(out=b4_sb.ap(), in_=b4_d[:, :])
        nc.sync.dma_start(out=be_sb.ap(), in_=be_d[:, :])
        with ExitStack() as ctx:
            const = ctx.enter_context(tc.tile_pool(name="pad", bufs=1))
            padrow = const.tile([1, ROW], bf16, tag="padrow")
            nc.gpsimd.memset(padrow[:], 0.0)
            nc.gpsimd.memset(padrow[:1, 17:18], NEG_BIG)
            nc.sync.dma_start(
                out=tabA_f[NFULL:NFULL + 2, :][None, :, :],
                in_=padrow[:1, None, :].to_broadcast([1, 2, ROW]))
            nc.sync.dma_start(
                out=tabB_f[NFULL:NFULL + 2, :][None, :, :],
                in_=padrow[:1, None, :].to_broadcast([1, 2, ROW]))

        with tc.tile_pool(name="l1x", bufs=3) as xp, \
             tc.tile_pool(name="l1p", bufs=4, space="PSUM") as pp:
            for t in range(NT):
                xw = xp.tile([P, KC * P], bf16, tag="xw")
                src = xt_d[:, :].rearrange("(c p) m -> p c m", p=P)[:, :, t * P:(t + 1) * P]
                nc.sync.dma_start(
                    out=xw[:].rearrange("p (c j) -> p c j", c=KC), in_=src)
                ps = pp.tile([P, NH], f32, tag="l1ps")
                for c in range(KC):
                    nc.tensor.matmul(
                        out=ps[:], lhsT=xw[:, c * P:(c + 1) * P],
                        rhs=w1_sb.ap()[:, c * NH:(c + 1) * NH],
                        start=(c == 0), stop=(c == KC - 1))
                hsl = h_sb.ap()[:, t * NH:(t + 1) * NH]
                nc.vector.tensor_add(hsl, ps[:], b1_sb.ap())
                nc.vector.tensor_scalar_max(hsl, hsl, 0.0)
        epilogue_rows(tabA_l)
        allgather(tabA_l, tabA_f)

    # ---------------- conv layer (one TileContext per chunk) ---------------
    def conv(tab_local, tab_full, beta_ap_fn):
        off = 0
        t_seen = 0
        for run in cfg.chunks(kg):
            with tile.TileContext(nc) as tc:
                with tc.tile_pool(name="cv", bufs=3) as cv, \
                     tc.tile_pool(name="cvp", bufs=2, space="PSUM") as cvp:
                    for (t0, gsz, K) in run:
                        gk = gsz * K
                        idx_sb = cv.tile([P, gk], i32, tag="idx")
                        nc.sync.dma_start(
                            out=idx_sb[:], in_=idx_d[:, off:off + gk])
                        hs = cv.tile([P, gk * ROW], bf16, tag="hs")
                        # HW indirect DMA = ONE index per partition reading
                        # contiguous elems; one gather per slot column. The
                        # completion sem fires at descriptor-generation, so a
                        # trailing plain SWDGE DMA on the same ring provides a
                        # data-landed fence for the whole group.
                        for j in range(gk):
                            nc.gpsimd.indirect_dma_start(
                                out=hs[:, j * ROW:(j + 1) * ROW],
                                out_offset=None,
                                in_=tab_full.ap(),
                                in_offset=bass.IndirectOffsetOnAxis(
                                    ap=idx_sb[:, j:j + 1], axis=0),
                            )
                        guard = cv.tile([P, 4], i32, tag="guard")
                        flush = nc.gpsimd.dma_start(
                            out=guard[:], in_=idx_d[:, 0:4])
                        hd = cv.tile([P, gsz * ROW], bf16, tag="hd")
                        nc.sync.dma_start(
                            out=hd[:].rearrange("p (g j) -> p g j", g=gsz),
                            in_=tab_local[t0 * P:(t0 + gsz) * P, :].rearrange(
                                "(g p) j -> p g j", p=P))

                        hs4 = hs[:].rearrange("p (g k j) -> p g k j", g=gsz, k=K)
                        hd3 = hd[:].rearrange("p (g j) -> p g j", g=gsz)
                        tmp = cv.tile([P, gk * NH], bf16, tag="tmp")
                        tm4 = tmp[:].rearrange("p (g k j) -> p g k j", g=gsz, k=K)
                        mul1 = nc.vector.tensor_mul(
                            tm4, hs4[:, :, :, 0:16],
                            hd3[:, :, None, 0:16].to_broadcast([P, gsz, K, 16]))
                        bass._add_dep_helper(
                            mul1.ins, flush.ins, sync=True,
                            reason="hs consumer waits for gather ring drain")
                        alpha = cv.tile([P, gk], f32, tag="alpha")
                        al3 = alpha[:].rearrange("p (g k) -> p g k", g=gsz)
                        nc.vector.reduce_sum(
                            alpha[:],
                            tmp[:].rearrange("p (gk j) -> p gk j", j=NH), axis=AX)
                        nc.vector.tensor_mul(al3, al3, hs4[:, :, :, 16])
                        invd = cv.tile([P, gsz], f32, tag="invd")
                        beta_ap = beta_ap_fn()
                        if beta_ap is None:
                            nc.vector.tensor_copy(invd[:], hd3[:, :, 16])
                        else:
                            nc.vector.tensor_scalar_mul(
                                invd[:], hd3[:, :, 16], beta_ap)
                        nc.vector.tensor_mul(
                            al3, al3,
                            invd[:][:, :, None].to_broadcast([P, gsz, K]))
                        nc.vector.tensor_add(al3, al3, hs4[:, :, :, 17])
                        nm = cv.tile([P, gsz], f32, tag="nm")
                        nc.vector.reduce_max(nm[:], al3, axis=AX, negate=True)
                        nc.vector.tensor_add(
                            al3, al3,
                            nm[:][:, :, None].to_broadcast([P, gsz, K]))
                        e_bf = cv.tile([P, gk], bf16, tag="e")
                        nc.scalar.activation(e_bf[:], alpha[:], AF.Exp)
                        s = cv.tile([P, gsz], f32, tag="s")
                        nc.vector.reduce_sum(
                            s[:], e_bf[:].rearrange("p (g k) -> p g k", g=gsz),
                            axis=AX)
                        nc.vector.tensor_scalar_add(s[:], s[:], 1.0e-16)
                        r = cv.tile([P, gsz], f32, tag="r")
                        nc.vector.reciprocal(r[:], s[:])
                        coef = cv.tile([P, gk], bf16, tag="coef")
                        nc.vector.tensor_mul(
                            coef[:].rearrange("p (g k) -> p g k", g=gsz),
                            e_bf[:].rearrange("p (g k) -> p g k", g=gsz),
                            r[:][:, :, None].to_broadcast([P, gsz, K]))
                        tmp2 = cv.tile([P, gk * NH], bf16, tag="tmp2")
                        t24 = tmp2[:].rearrange("p (g k j) -> p g k j", g=gsz, k=K)
                        nc.vector.tensor_mul(
                            t24, hs4[:, :, :, 0:16],
                            coef[:].rearrange("p (g k) -> p g k", g=gsz)
                            [:, :, :, None].to_broadcast([P, gsz, K, 16]))
                        h2v = h_sb.ap()[:, t0 * NH:(t0 + gsz) * NH]
                        nc.vector.reduce_sum(
                            h2v,
                            tmp2[:].rearrange(
                                "p (g k j) -> p g j k", g=gsz, k=K),
                            axis=AX)
                        off += gk
                        t_seen += gsz
        assert t_seen == NT

    if "B" in phases:
        conv(tabA_l, tabA_f, lambda: None)

    # ---------------- phase C: table B + AG2 -------------------------------
    if "C" in phases:
        with tile.TileContext(nc) as tc:
            epilogue_rows(tabB_l)
            allgather(tabB_l, tabB_f)

    if "D" in phases:
        conv(tabB_l, tabB_f, lambda: be_sb.ap()[:, 0:1])

    # ---------------- head + log_softmax -----------------------------------
    if "G" in phases:
        # debug: gather group 0 from tabA_f and dump raw rows (as f32)
        (t0g, gszg, Kg) = cfg.groups(kg)[0]
        gkg = gszg * Kg
        with tile.TileContext(nc) as tc:
            with tc.tile_pool(name="dbg", bufs=1) as dbg:
                idx_sb = dbg.tile([P, gkg], i32, tag="idx")
                nc.sync.dma_start(out=idx_sb[:], in_=idx_d[:, 0:gkg])
                hs = dbg.tile([P, gkg * ROW], bf16, tag="hs")
                for j in range(gkg):
                    nc.gpsimd.indirect_dma_start(
                        out=hs[:, j * ROW:(j + 1) * ROW], out_offset=None,
                        in_=tabA_f.ap(),
                        in_offset=bass.IndirectOffsetOnAxis(
                            ap=idx_sb[:, j:j + 1], axis=0))
                guard = dbg.tile([P, 4], i32, tag="guard")
                flush = nc.gpsimd.dma_start(out=guard[:], in_=idx_d[:, 0:4])
                ncols = min(gkg * ROW, (NPC // P) * NCO * (NPC // P and 1) * 980)
                ncols = min(gkg * ROW, 980)
                hf = dbg.tile([P, ncols], f32, tag="hf")
                cp = nc.vector.tensor_copy(hf[:], hs[:, 0:ncols])
                bass._add_dep_helper(
                    cp.ins, flush.ins, sync=True, reason="debug drain")
                ov = out_d.ap().rearrange("(p q) j -> p (q j)", p=P)
                nc.sync.dma_start(out=ov[:, 0:ncols], in_=hf[:])
        nc.compile()
        return nc

    if "E" not in phases:
        # debug: dump h_sb (and inv_sb) into out
        with tile.TileContext(nc) as tc:
            ov = out_d.ap().rearrange("(p q) j -> p (q j)", p=P)
            nc.sync.dma_start(out=ov[:, 0:NT * NH], in_=h_sb.ap())
            nc.sync.dma_start(out=ov[:, NT * NH:NT * NH + NT], in_=inv_sb.ap())
        nc.compile()
        return nc

    with tile.TileContext(nc) as tc:
        with tc.tile_pool(name="hd", bufs=1) as hp, \
             tc.tile_pool(name="hdp", bufs=4, space="PSUM") as hpp:
            h3t = hp.tile([NH, NT * P], f32, tag="h3t")
            for t in range(NT):
                pst = hpp.tile([NH, P], f32, tag="pst")
                nc.tensor.transpose(
                    out=pst[:], in_=h_sb.ap()[:, t * NH:(t + 1) * NH],
                    identity=ident_f.ap())
                nc.vector.tensor_copy(h3t[:, t * P:(t + 1) * P], pst[:])
            lg = hp.tile([P, NT * NCO], f32, tag="lg")
            for t in range(NT):
                psl = hpp.tile([P, NCO], f32, tag="psl")
                nc.tensor.matmul(
                    out=psl[:], lhsT=h3t[:, t * P:(t + 1) * P], rhs=w4_sb.ap(),
                    start=True, stop=True)
                nc.vector.tensor_add(
                    lg[:, t * NCO:(t + 1) * NCO], psl[:], b4_sb.ap())
            lg3 = lg[:].rearrange("p (t j) -> p t j", t=NT)
            nm = hp.tile([P, NT], f32, tag="hnm")
            nc.vector.reduce_max(nm[:], lg3, axis=AX, negate=True)
            nc.vector.tensor_add(
                lg3, lg3, nm[:][:, :, None].to_broadcast([P, NT, NCO]))
            ex = hp.tile([P, NT * NCO], f32, tag="ex")
            nc.scalar.activation(ex[:], lg[:], AF.Exp)
            s = hp.tile([P, NT], f32, tag="hs_sum")
            nc.vector.reduce_sum(
                s[:], ex[:].rearrange("p (t j) -> p t j", t=NT), axis=AX)
            ls = hp.tile([P, NT], f32, tag="ls")
            nc.scalar.activation(ls[:], s[:], AF.Ln)
            nc.vector.tensor_sub(
                lg3, lg3, ls[:][:, :, None].to_broadcast([P, NT, NCO]))
            nc.sync.dma_start(
                out=out_d[:, :].rearrange("(t p) j -> p t j", p=P),
                in_=lg3)

    nc.compile()
    return nc


# --------------------------------------------------------------------------
# entry point
# --------------------------------------------------------------------------

def run(cfg: Cfg, inputs: dict, trace: bool = False):
    from concourse import bass_utils

    x = np.asarray(inputs["x"], np.float32)
    edge_index = np.asarray(inputs["edge_index"])
    W1 = np.asarray(inputs["W1"], np.float32)
    b1 = np.asarray(inputs["b1"], np.float32)
    W4 = np.asarray(inputs["W4"], np.float32)
    b4 = np.asarray(inputs["b4"], np.float32)
    beta3 = np.asarray(inputs["beta3"], np.float32)

    import ml_dtypes

    per_core, kg, order_c = preprocess(cfg, x, edge_index)
    nc = build_kernel(cfg, kg)

    P, KC, NH = cfg.P, cfg.kchunks, cfg.nh
    w1p = np.zeros((cfg.f_pad, NH), np.float32)
    w1p[:cfg.f_in] = W1
    w1p = np.ascontiguousarray(
        w1p.reshape(KC, P, NH).transpose(1, 0, 2).reshape(P, KC * NH)
    ).astype(ml_dtypes.bfloat16)
    b1r = np.ascontiguousarray(np.broadcast_to(b1[None, :], (P, NH)))
    b4r = np.ascontiguousarray(np.broadcast_to(b4[None, :], (P, cfg.nc_out)))
    ber = np.ascontiguousarray(np.broadcast_to(beta3[None, :], (P, 1)))

    in_maps = []
    for c in range(cfg.n_cores):
        in_maps.append({
            "xt": per_core[c]["xt"].astype(ml_dtypes.bfloat16),
            "idx": per_core[c]["idx"],
            "w1p": w1p, "b1r": b1r, "w4r": np.ascontiguousarray(W4),
            "b4r": b4r, "beta3r": ber,
        })

    res = bass_utils.run_bass_kernel_spmd(
        nc, in_maps, core_ids=list(range(cfg.n_cores)), trace=trace)

    out = np.empty((cfg.n_nodes, cfg.nc_out), np.float32)
    for c in range(cfg.n_cores):
        oc = np.asarray(res.results[c]["out"])[:cfg.npc_raw]
        out[c * cfg.npc_raw + order_c[c]] = oc
    return out, res


def kernel(**inputs) -> np.ndarray:
    out, _ = run(FULL, inputs, trace=False)
    return out



# revision 18
# speedup vs baseline: 1.1020x; 1.1020x over previous
"""AGNNet (2-layer AGNN conv + linear head) distributed over 8 trn2 NeuronCores.

Strategy (graph/data parallel, per sharding hint):
  - nodes sharded by dst range: core c owns nodes [c*6250, (c+1)*6250)
  - host groups edges by dst, degree-sorts each core's local nodes (undone on
    output), drops self-loops (handled analytically in the softmax), and packs
    each 128-node tile's in-edge lists into per-tile slot columns
  - device: L1 matmul from host-transposed x shard; build a bf16 row table
    [h(16), inv_norm, bias, pad] per node; AllGather the table; expand it to a
    256B-strided copy in DRAM (SWDGE regular pattern, CounterMachine rate);
    per node-tile ONE dma_gather (ant extended Q7 kernel, ~7.9ns/idx) per
    index window fetches all neighbor rows; DVE softmax (no max-trick needed:
    |alpha| <= |beta|); the self-loop term exp(beta*cos(h,h)) is added
    analytically; second conv identical; head matmul + log_softmax.
  - dma_gather indices are int16, so the 50178-row table is addressed through
    TWO overlapping 32768-row windows (bases 0 and 17410); each node's slots
    are split into window-A columns then window-B columns (host balances the
    split per tile); a pad row at wide-row 17410/17411 (bias = -1e30) backs
    padding slots so they vanish in the softmax.
"""

import math
from contextlib import ExitStack
from dataclasses import dataclass

import numpy as np


@dataclass
class Cfg:
    n_cores: int = 8
    n_nodes: int = 50000
    f_in: int = 2000
    nh: int = 16
    nc_out: int = 20
    row: int = 20           # table row elems: 16 h, [16]=invn, [17]=bias, 18-19 pad
    wrow: int = 128         # wide-table row elems (256B stride for dma_gather)
    P: int = 128
    win: int = 32768        # dma_gather int16 index window (rows per base)

    @property
    def npc_raw(self) -> int:       # real nodes per core
        return self.n_nodes // self.n_cores

    @property
    def npc(self) -> int:           # padded nodes per core (mult of 128)
        return ((self.npc_raw + self.P - 1) // self.P) * self.P

    @property
    def ntiles(self) -> int:
        return self.npc // self.P

    @property
    def kchunks(self) -> int:       # 128-row chunks of the padded f_in
        return (self.f_in + self.P - 1) // self.P

    @property
    def f_pad(self) -> int:
        return self.kchunks * self.P

    @property
    def nfull(self) -> int:
        return self.n_cores * self.npc

    @property
    def nwide(self) -> int:         # wide-table rows (2 pad rows mid-table)
        return self.nfull + 2

    @property
    def wb(self) -> int:            # window-B base (overlap split point)
        return self.nwide - self.win

    @property
    def pad_wrow(self) -> int:      # wide-row of the all-zero pad row
        return self.wb


FULL = Cfg()
NEG_BIG = -1.0e30


def pack_idx16(idx_mat: np.ndarray) -> np.ndarray:
    """[128 partitions, nch chunks] window-relative indices -> dma_gather
    int16 index tile [128, nch*8]: list position c*128+j lands on partition j
    chunk c (the ucode lane swizzle applies to both the index read and the
    dst partition, so it cancels); the flat list is wrapped over 16
    partitions and replicated across the 8 16-partition groups."""
    P, nch = idx_mat.shape
    assert P == 128
    flat = np.ascontiguousarray(idx_mat.T).reshape(-1).astype(np.int16)
    tile = np.empty((128, nch * 8), np.int16)
    for p in range(16):
        tile[p] = flat[p::16]
    tile[16:] = np.tile(tile[:16], (7, 1))
    return np.ascontiguousarray(tile)


def dma_gather_raw(nc, out_ap, in_ap, idxs_ap, num_idxs: int,
                   elem_size: int, stride_elems: int):
    """nc.gpsimd.dma_gather minus the elem_size%256 assert (non-transpose
    path in the Q7 ucode has no such restriction; only the stride must be a
    multiple of 256B). single_packet=False: a single packet is capped at 64
    descriptors, large gathers hang with it."""
    from concourse import mybir
    g = nc.gpsimd
    stride_bytes = stride_elems * mybir.dt.size(in_ap.dtype)
    assert stride_bytes % 256 == 0 and stride_bytes // 256 < 256
    _in_ap = g.lower_ap_dma(in_ap, for_custom_bir_dma=True)
    _idxs_ap = g.lower_ap(idxs_ap)
    _out_ap = g.lower_ap(out_ap)
    return g.add_instruction(
        mybir.InstDMAGatherAnt(
            name=g.bass.get_next_instruction_name(),
            ins=[*_in_ap, _idxs_ap, g.lower_val_access(g.to_reg(num_idxs))],
            outs=[_out_ap],
            transpose=False,
            num_idxs=num_idxs,
            elem_size=elem_size,
            stride_bytes_256=stride_bytes // 256,
            gen_mode=0,
            single_packet=False,
            queue_num=0,
            sbuf_tokens_per_rank=0,
            sbuf_free_dim_per_rank=0,
            sbuf_free_dim_pad_per_rank=0,
            sbuf_byte_offset=0,
        ))


# --------------------------------------------------------------------------
# host-side preprocessing
# --------------------------------------------------------------------------

def preprocess(cfg: Cfg, x: np.ndarray, edge_index: np.ndarray):
    """Shard + layout transform.

    Returns (per_core list of dicts, kab, order_c) where kab is the shared
    [(KA_t, KB_t)] per tile (same across cores for SPMD)."""
    P, NPC, NPCR = cfg.P, cfg.npc, cfg.npc_raw
    n, C, NT = cfg.n_nodes, cfg.n_cores, cfg.ntiles
    WB = cfg.wb

    src = edge_index[0].astype(np.int64)
    dst = edge_index[1].astype(np.int64)
    keep = src != dst                     # self-loops handled analytically
    src, dst = src[keep], dst[keep]

    core_of_dst = dst // NPCR

    # pass 1: per-core degree sort -> rank of each node within its core
    order_c, rank_c, deg_c = [], [], []
    for c in range(C):
        m = core_of_dst == c
        ld = dst[m] - c * NPCR
        deg = np.bincount(ld, minlength=NPCR)
        order = np.argsort(deg, kind="stable")      # ascending degree
        rank = np.empty(NPCR, np.int64)
        rank[order] = np.arange(NPCR)
        order_c.append(order)
        rank_c.append(rank)
        deg_c.append(deg)

    # new global id after per-core permutation + padding, then wide-row shift
    new_gid = np.empty(n, np.int64)
    for c in range(C):
        new_gid[c * NPCR:(c + 1) * NPCR] = c * NPC + rank_c[c]

    def wrow_of(g):
        return np.where(g < WB, g, g + 2)

    # per-core, per-tile edge lists in (dst-rank, window-classified src) form
    per_core_lists = []
    for c in range(C):
        m = core_of_dst == c
        ld = dst[m] - c * NPCR
        gs = wrow_of(new_gid[src[m]])
        er = rank_c[c][ld]                       # dst rank of each edge
        eo = np.argsort(er, kind="stable")
        er_s = er[eo]
        gs_s = gs[eo]
        starts = np.zeros(NPC + 1, np.int64)
        np.cumsum(np.bincount(er_s, minlength=NPC), out=starts[1:])
        per_core_lists.append((starts, gs_s))

    # window split per (core, tile, node): nA in [lenA, lenA+lenF];
    # KA_t/KB_t shared across cores (SPMD): take max over cores.
    W = cfg.win
    kab = []
    nA_all = np.zeros((C, NPC), np.int64)
    degp_all = np.zeros((C, NPC), np.int64)
    lenA_all = np.zeros((C, NPC), np.int64)
    for c in range(C):
        starts, gs_s = per_core_lists[c]
        d_cnt = np.diff(starts)
        isA = gs_s < WB
        isF = (gs_s >= WB) & (gs_s < W)
        # segment sums by dst rank
        er_of_edge = np.repeat(np.arange(NPC), d_cnt)
        a_cnt = np.bincount(er_of_edge, weights=isA, minlength=NPC).astype(np.int64)
        f_cnt = np.bincount(er_of_edge, weights=isF, minlength=NPC).astype(np.int64)
        degp_all[c] = d_cnt
        lenA_all[c] = a_cnt
        nA_all[c] = a_cnt + f_cnt  # upper bound; refined per tile below

    for t in range(NT):
        s, e = t * P, (t + 1) * P
        bestKA, bestKB, bestT = None, None, None
        degs = degp_all[:, s:e]
        lA = lenA_all[:, s:e]
        lAF = nA_all[:, s:e]
        kmax = int(degs.max())
        best = None
        for T in range(kmax + 1):
            nA = np.clip(T, lA, lAF)
            KA = int(nA.max()) if nA.size else 0
            KB = int((degs - nA).max())
            if best is None or KA + KB < best[0]:
                best = (KA + KB, KA, KB)
        kab.append((max(best[1], 1), max(best[2], 1)))

    # build per-core idx16 inputs
    per_core = []
    for c in range(C):
        starts, gs_s = per_core_lists[c]
        packs = []
        for t in range(NT):
            KA, KB = kab[t]
            idxA = np.full((P, KA), cfg.pad_wrow, np.int64)
            idxB = np.full((P, KB), 0, np.int64)  # relative to WB: pad row
            degs = degp_all[c, t * P:(t + 1) * P]
            lA = lenA_all[c, t * P:(t + 1) * P]
            lAF = nA_all[c, t * P:(t + 1) * P]
            # recompute the tile's chosen T (re-derive nA with final KA/KB)
            # choose per-node nA: as many A-capable as fit in KA, rest to B
            nA = np.minimum(lAF, KA)
            nB = degs - nA
            assert (nB <= KB).all() and (nA <= KA).all()
            for p in range(P):
                node = t * P + p
                lst = gs_s[starts[node]:starts[node + 1]]
                la, laf = lA[p], lAF[p]
                na = nA[p]
                A_part = np.concatenate([
                    lst[lst < WB],
                    lst[(lst >= WB) & (lst < W)][:na - la]])
                B_part = np.concatenate([
                    lst[(lst >= WB) & (lst < W)][na - la:],
                    lst[lst >= W]])
                idxA[p, :A_part.size] = A_part
                idxB[p, :B_part.size] = B_part - WB
            assert idxA.max() < W and idxB.max() < W
            packs.append(pack_idx16(idxA))
            packs.append(pack_idx16(idxB))
        idx16 = np.concatenate(packs, axis=1)

        # x shard: permuted, padded, transposed, f-padded
        xs = x[c * NPCR:(c + 1) * NPCR][order_c[c]]      # [NPCR, f_in]
        xt = np.zeros((cfg.f_pad, NPC), np.float32)
        xt[:cfg.f_in, :NPCR] = xs.T
        per_core.append({"xt": np.ascontiguousarray(xt), "idx16": idx16})

    return per_core, kab, order_c


# --------------------------------------------------------------------------
# device kernel builder
# --------------------------------------------------------------------------

def build_kernel(cfg: Cfg, kab, phases: str = "ABCDE"):
    import concourse.bacc as bacc
    import concourse.tile as tile
    from concourse import bass, mybir
    from concourse.masks import make_identity

    P = cfg.P
    NH, NCO, ROW, WROW = cfg.nh, cfg.nc_out, cfg.row, cfg.wrow
    NPC, NT, KC = cfg.npc, cfg.ntiles, cfg.kchunks
    NFULL, NWIDE, WB, W = cfg.nfull, cfg.nwide, cfg.wb, cfg.win
    f32 = mybir.dt.float32
    bf16 = mybir.dt.bfloat16
    i16 = mybir.dt.int16
    AX = mybir.AxisListType.X
    OP = mybir.AluOpType
    AF = mybir.ActivationFunctionType
    idxw = sum(8 * (ka + kb) for (ka, kb) in kab)   # idx16 words per partition

    nc = bacc.Bacc("TRN2", target_bir_lowering=False, debug=False,
                   num_devices=cfg.n_cores)

    xt_d = nc.dram_tensor("xt", [cfg.f_pad, NPC], bf16, kind="ExternalInput")
    idx_d = nc.dram_tensor("idx16", [P, idxw], i16, kind="ExternalInput")
    w1_d = nc.dram_tensor("w1p", [P, KC * NH], bf16, kind="ExternalInput")
    b1_d = nc.dram_tensor("b1r", [P, NH], f32, kind="ExternalInput")
    w4_d = nc.dram_tensor("w4r", [NH, NCO], f32, kind="ExternalInput")
    b4_d = nc.dram_tensor("b4r", [P, NCO], f32, kind="ExternalInput")
    be_d = nc.dram_tensor("beta3r", [P, 1], f32, kind="ExternalInput")
    out_d = nc.dram_tensor("out", [NPC, NCO], f32, kind="ExternalOutput")

    tabA_l = nc.dram_tensor("tabA_l", [NPC, ROW], bf16)
    tabA_f = nc.dram_tensor("tabA_f", [NFULL + 2, ROW], bf16, addr_space="Shared")
    tabB_l = nc.dram_tensor("tabB_l", [NPC, ROW], bf16)
    tabB_f = nc.dram_tensor("tabB_f", [NFULL + 2, ROW], bf16, addr_space="Shared")
    tabA_w = nc.dram_tensor("tabA_w", [NWIDE, WROW], bf16)
    tabB_w = nc.dram_tensor("tabB_w", [NWIDE, WROW], bf16)

    # persistent SBUF (survives across TileContexts)
    def sb(name, shape, dtype):
        return nc.alloc_sbuf_tensor(name, list(shape), dtype)

    ident_f = sb("identf", [P, P], f32)
    zeros = sb("zeros", [P, P], f32)
    w1_sb = sb("w1sb", [P, KC * NH], bf16)
    b1_sb = sb("b1sb", [P, NH], f32)
    w4_sb = sb("w4sb", [NH, NCO], f32)
    b4_sb = sb("b4sb", [P, NCO], f32)
    be_sb = sb("besb", [P, 1], f32)
    bee_sb = sb("beesb", [P, 1], f32)      # exp-ready: beta3 value
    h_sb = sb("hsb", [P, NT * NH], f32)
    rows_sb = sb("rowssb", [P, NT * ROW], bf16)
    sq_sb = sb("sqsb", [P, NT * NH], f32)
    ss_sb = sb("sssb", [P, NT], f32)
    inv_sb = sb("invsb", [P, NT], f32)
    idx_sb = sb("idxsb", [P, idxw], i16)    # all tiles' gather indices

    def epilogue_rows(tab_local):
        """h_sb -> inv norm -> rows_sb -> DMA to tab_local."""
        h3v = h_sb.ap().rearrange("p (t j) -> p t j", t=NT)
        nc.vector.tensor_mul(sq_sb.ap(), h_sb.ap(), h_sb.ap())
        nc.vector.reduce_sum(
            ss_sb.ap(), sq_sb.ap().rearrange("p (t j) -> p t j", t=NT), axis=AX)
        nc.scalar.sqrt(ss_sb.ap(), ss_sb.ap())
        nc.vector.tensor_scalar_add(ss_sb.ap(), ss_sb.ap(), 1.0e-12)
        nc.vector.reciprocal(inv_sb.ap(), ss_sb.ap())
        rv = rows_sb.ap().rearrange("p (t j) -> p t j", t=NT)
        nc.vector.tensor_copy(rv[:, :, 0:16], h3v)
        nc.vector.tensor_copy(rv[:, :, 16], inv_sb.ap())
        nc.vector.tensor_copy(rv[:, :, 17], zeros.ap()[:, 0:NT])
        nc.sync.dma_start(
            out=tab_local[:, :].rearrange("(t p) j -> p t j", p=P),
            in_=rv)

    def allgather(tab_local, tab_full):
        nc.gpsimd.collective_compute(
            "AllGather", OP.bypass,
            replica_groups=[list(range(cfg.n_cores))],
            ins=[tab_local.ap().opt()],
            outs=[tab_full.ap()[0:NFULL, :].opt()])

    def expand(tab_full, tab_wide):
        """packed [NFULL+2, 20] -> 256B-strided [NWIDE, 128] (cols 0:20).

        Regular-pattern SWDGE DMAs (CounterMachine rate ~0.34ns/desc),
        chunked under the 16384-descriptor SWDGE carveout limit.
        wide rows [0, WB) <- packed [0, WB); [WB, WB+2) <- packed pads
        [NFULL, NFULL+2); [WB+2, NWIDE) <- packed [WB, NFULL)."""
        CH = 12288

        def copy(wlo, whi, plo):
            for o in range(0, whi - wlo, CH):
                n = min(CH, whi - wlo - o)
                nc.gpsimd.dma_start(
                    out=tab_wide[wlo + o:wlo + o + n, 0:ROW],
                    in_=tab_full[plo + o:plo + o + n, :])

        copy(0, WB, 0)
        copy(WB, WB + 2, NFULL)
        copy(WB + 2, NWIDE, WB)

    # ---------------- phase A: consts, idx preload, L1, table A, AG1 -------
    with tile.TileContext(nc) as tc:
        make_identity(nc, ident_f.ap())
        nc.gpsimd.memset(zeros.ap(), 0.0)
        nc.gpsimd.memset(rows_sb.ap(), 0.0)
        nc.sync.dma_start(out=idx_sb.ap(), in_=idx_d[:, :])
        nc.sync.dma_start(out=w1_sb.ap(), in_=w1_d[:, :])
        nc.sync.dma_start(out=b1_sb.ap(), in_=b1_d[:, :])
        nc.sync.dma_start(out=w4_sb.ap(), in_=w4_d[:, :])
        nc.sync.dma_start(out=b4_sb.ap(), in_=b4_d[:, :])
        nc.sync.dma_start(out=be_sb.ap(), in_=be_d[:, :])
        with ExitStack() as ctx:
            const = ctx.enter_context(tc.tile_pool(name="pad", bufs=1))
            padrow = const.tile([1, ROW], bf16, tag="padrow")
            nc.gpsimd.memset(padrow[:], 0.0)
            nc.gpsimd.memset(padrow[:1, 17:18], NEG_BIG)
            nc.sync.dma_start(
                out=tabA_f[NFULL:NFULL + 2, :][None, :, :],
                in_=padrow[:1, None, :].to_broadcast([1, 2, ROW]))
            nc.sync.dma_start(
                out=tabB_f[NFULL:NFULL + 2, :][None, :, :],
                in_=padrow[:1, None, :].to_broadcast([1, 2, ROW]))

        with tc.tile_pool(name="l1x", bufs=3) as xp, \
             tc.tile_pool(name="l1p", bufs=4, space="PSUM") as pp:
            for t in range(NT):
                xw = xp.tile([P, KC * P], bf16, tag="xw")
                src = xt_d[:, :].rearrange("(c p) m -> p c m", p=P)[:, :, t * P:(t + 1) * P]
                nc.sync.dma_start(
                    out=xw[:].rearrange("p (c j) -> p c j", c=KC), in_=src)
                ps = pp.tile([P, NH], f32, tag="l1ps")
                for c in range(KC):
                    nc.tensor.matmul(
                        out=ps[:], lhsT=xw[:, c * P:(c + 1) * P],
                        rhs=w1_sb.ap()[:, c * NH:(c + 1) * NH],
                        start=(c == 0), stop=(c == KC - 1))
                hsl = h_sb.ap()[:, t * NH:(t + 1) * NH]
                nc.vector.tensor_add(hsl, ps[:], b1_sb.ap())
                nc.vector.tensor_scalar_max(hsl, hsl, 0.0)
        epilogue_rows(tabA_l)
        allgather(tabA_l, tabA_f)
    with tile.TileContext(nc) as tc:
        expand(tabA_f, tabA_w)

    # ---------------- conv layer -------------------------------------------
    def conv(tab_local, tab_wide, beta_ap):
        """beta_ap: None for beta=1 (conv2), else [P,1] AP with beta value."""
        off = 0
        with tile.TileContext(nc) as tc:
            with tc.tile_pool(name="cv", bufs=3) as cv, \
                 tc.tile_pool(name="cvs", bufs=2) as cvs:
                for t in range(NT):
                    KA, KB = kab[t]
                    K = KA + KB
                    hs = cv.tile([P, K * ROW], bf16, tag="hs")
                    dma_gather_raw(
                        nc,
                        out_ap=hs[:, 0:KA * ROW].rearrange(
                            "p (c j) -> p c j", j=ROW),
                        in_ap=tab_wide[0:W, 0:ROW],
                        idxs_ap=idx_sb.ap()[:, off:off + 8 * KA],
                        num_idxs=P * KA, elem_size=ROW, stride_elems=WROW)
                    off += 8 * KA
                    dma_gather_raw(
                        nc,
                        out_ap=hs[:, KA * ROW:K * ROW].rearrange(
                            "p (c j) -> p c j", j=ROW),
                        in_ap=tab_wide[WB:WB + W, 0:ROW],
                        idxs_ap=idx_sb.ap()[:, off:off + 8 * KB],
                        num_idxs=P * KB, elem_size=ROW, stride_elems=WROW)
                    off += 8 * KB

                    hd = cvs.tile([P, ROW], bf16, tag="hd")
                    nc.sync.dma_start(
                        out=hd[:], in_=tab_local[t * P:(t + 1) * P, :]
                        .rearrange("(g p) j -> p (g j)", p=P))

                    hs3 = hs[:].rearrange("p (k j) -> p k j", k=K)
                    tmp = cv.tile([P, K * NH], bf16, tag="tmp")
                    tm3 = tmp[:].rearrange("p (k j) -> p k j", k=K)
                    nc.vector.tensor_mul(
                        tm3, hs3[:, :, 0:16],
                        hd[:, None, 0:16].to_broadcast([P, K, 16]))
                    alpha = cv.tile([P, K], f32, tag="alpha")
                    nc.vector.reduce_sum(
                        alpha[:],
                        tmp[:].rearrange("p (k j) -> p k j", j=NH), axis=AX)
                    nc.vector.tensor_mul(alpha[:], alpha[:], hs3[:, :, 16])
                    invd = cvs.tile([P, 1], f32, tag="invd")
                    if beta_ap is None:
                        nc.vector.tensor_copy(invd[:], hd[:, 16:17])
                    else:
                        nc.vector.tensor_mul(invd[:], hd[:, 16:17], beta_ap)
                    nc.vector.tensor_mul(
                        alpha[:], alpha[:],
                        invd[:].to_broadcast([P, K]))
                    # pad slots: hs row is zeros with bias=-1e30 -> alpha=-1e30
                    nc.vector.tensor_add(alpha[:], alpha[:], hs3[:, :, 17])
                    # |alpha| <= |beta| for real slots: exp without max-trick
                    e_bf = cv.tile([P, K], bf16, tag="e")
                    nc.scalar.activation(e_bf[:], alpha[:], AF.Exp)
                    s = cvs.tile([P, 1], f32, tag="s")
                    nc.vector.reduce_sum(
                        s[:], e_bf[:][:, None, :], axis=AX)
                    # analytic self-loop: cos(h,h) = (1 - 1e-12*invn)^2
                    selfa = cvs.tile([P, 1], f32, tag="selfa")
                    nc.vector.tensor_scalar(
                        selfa[:], hd[:, 16:17], -1.0e-12, 1.0,
                        op0=OP.mult, op1=OP.add)
                    nc.vector.tensor_mul(selfa[:], selfa[:], selfa[:])
                    if beta_ap is not None:
                        nc.vector.tensor_mul(selfa[:], selfa[:], beta_ap)
                    es = cvs.tile([P, 1], f32, tag="es")
                    nc.scalar.activation(es[:], selfa[:], AF.Exp)
                    nc.vector.tensor_add(s[:], s[:], es[:])
                    nc.vector.tensor_scalar_add(s[:], s[:], 1.0e-16)
                    r = cvs.tile([P, 1], f32, tag="r")
                    nc.vector.reciprocal(r[:], s[:])
                    coef = cv.tile([P, K], bf16, tag="coef")
                    nc.vector.tensor_mul(
                        coef[:], e_bf[:], r[:].to_broadcast([P, K]))
                    tmp2 = cv.tile([P, K * NH], bf16, tag="tmp2")
                    t23 = tmp2[:].rearrange("p (k j) -> p k j", k=K)
                    nc.vector.tensor_mul(
                        t23, hs3[:, :, 0:16],
                        coef[:][:, :, None].to_broadcast([P, K, 16]))
                    h2v = h_sb.ap()[:, t * NH:(t + 1) * NH]
                    nc.vector.reduce_sum(
                        h2v,
                        tmp2[:].rearrange("p (k j) -> p j k", k=K),
                        axis=AX)
                    # += (exp(self)/s) * h_d
                    rs = cvs.tile([P, 1], f32, tag="rs")
                    nc.vector.tensor_mul(rs[:], es[:], r[:])
                    sh = cvs.tile([P, NH], f32, tag="sh")
                    nc.vector.tensor_mul(
                        sh[:], hd[:, 0:16], rs[:].to_broadcast([P, NH]))
                    nc.vector.tensor_add(h2v, h2v, sh[:])
        assert off == idxw

    if "B" in phases:
        conv(tabA_l, tabA_w, None)

    # ---------------- phase C: table B + AG2 -------------------------------
    if "C" in phases:
        with tile.TileContext(nc) as tc:
            epilogue_rows(tabB_l)
            allgather(tabB_l, tabB_f)
        with tile.TileContext(nc) as tc:
            expand(tabB_f, tabB_w)

    if "D" in phases:
        conv(tabB_l, tabB_w, be_sb.ap()[:, 0:1])

    if "E" not in phases:
        # debug: dump h_sb (and inv_sb) into out
        with tile.TileContext(nc) as tc:
            ov = out_d.ap().rearrange("(p q) j -> p (q j)", p=P)
            nc.sync.dma_start(out=ov[:, 0:NT * NH], in_=h_sb.ap())
            nc.sync.dma_start(out=ov[:, NT * NH:NT * NH + NT], in_=inv_sb.ap())
        nc.compile()
        return nc

    with tile.TileContext(nc) as tc:
        with tc.tile_pool(name="hd", bufs=1) as hp, \
             tc.tile_pool(name="hdp", bufs=4, space="PSUM") as hpp:
            h3t = hp.tile([NH, NT * P], f32, tag="h3t")
            for t in range(NT):
                pst = hpp.tile([NH, P], f32, tag="pst")
                nc.tensor.transpose(
                    out=pst[:], in_=h_sb.ap()[:, t * NH:(t + 1) * NH],
                    identity=ident_f.ap())
                nc.vector.tensor_copy(h3t[:, t * P:(t + 1) * P], pst[:])
            lg = hp.tile([P, NT * NCO], f32, tag="lg")
            for t in range(NT):
                psl = hpp.tile([P, NCO], f32, tag="psl")
                nc.tensor.matmul(
                    out=psl[:], lhsT=h3t[:, t * P:(t + 1) * P], rhs=w4_sb.ap(),
                    start=True, stop=True)
                nc.vector.tensor_add(
                    lg[:, t * NCO:(t + 1) * NCO], psl[:], b4_sb.ap())
            lg3 = lg[:].rearrange("p (t j) -> p t j", t=NT)
            nm = hp.tile([P, NT], f32, tag="hnm")
            nc.vector.reduce_max(nm[:], lg3, axis=AX, negate=True)
            nc.vector.tensor_add(
                lg3, lg3, nm[:][:, :, None].to_broadcast([P, NT, NCO]))
            ex = hp.tile([P, NT * NCO], f32, tag="ex")
            nc.scalar.activation(ex[:], lg[:], AF.Exp)
            s = hp.tile([P, NT], f32, tag="hs_sum")
            nc.vector.reduce_sum(
                s[:], ex[:].rearrange("p (t j) -> p t j", t=NT), axis=AX)
            ls = hp.tile([P, NT], f32, tag="ls")
            nc.scalar.activation(ls[:], s[:], AF.Ln)
            nc.vector.tensor_sub(
                lg3, lg3, ls[:][:, :, None].to_broadcast([P, NT, NCO]))
            nc.sync.dma_start(
                out=out_d[:, :].rearrange("(t p) j -> p t j", p=P),
                in_=lg3)

    nc.compile()
    return nc


# --------------------------------------------------------------------------
# entry point
# --------------------------------------------------------------------------

def run(cfg: Cfg, inputs: dict, trace: bool = False):
    from concourse import bass_utils

    x = np.asarray(inputs["x"], np.float32)
    edge_index = np.asarray(inputs["edge_index"])
    W1 = np.asarray(inputs["W1"], np.float32)
    b1 = np.asarray(inputs["b1"], np.float32)
    W4 = np.asarray(inputs["W4"], np.float32)
    b4 = np.asarray(inputs["b4"], np.float32)
    beta3 = np.asarray(inputs["beta3"], np.float32)

    import ml_dtypes

    per_core, kab, order_c = preprocess(cfg, x, edge_index)
    nc = build_kernel(cfg, kab)

    P, KC, NH = cfg.P, cfg.kchunks, cfg.nh
    w1p = np.zeros((cfg.f_pad, NH), np.float32)
    w1p[:cfg.f_in] = W1
    w1p = np.ascontiguousarray(
        w1p.reshape(KC, P, NH).transpose(1, 0, 2).reshape(P, KC * NH)
    ).astype(ml_dtypes.bfloat16)
    b1r = np.ascontiguousarray(np.broadcast_to(b1[None, :], (P, NH)))
    b4r = np.ascontiguousarray(np.broadcast_to(b4[None, :], (P, cfg.nc_out)))
    ber = np.ascontiguousarray(np.broadcast_to(beta3[None, :], (P, 1)))

    in_maps = []
    for c in range(cfg.n_cores):
        in_maps.append({
            "xt": per_core[c]["xt"].astype(ml_dtypes.bfloat16),
            "idx16": per_core[c]["idx16"],
            "w1p": w1p, "b1r": b1r, "w4r": np.ascontiguousarray(W4),
            "b4r": b4r, "beta3r": ber,
        })

    res = bass_utils.run_bass_kernel_spmd(
        nc, in_maps, core_ids=list(range(cfg.n_cores)), trace=trace)

    out = np.empty((cfg.n_nodes, cfg.nc_out), np.float32)
    for c in range(cfg.n_cores):
        oc = np.asarray(res.results[c]["out"])[:cfg.npc_raw]
        out[c * cfg.npc_raw + order_c[c]] = oc
    return out, res


def kernel(**inputs) -> np.ndarray:
    out, _ = run(FULL, inputs, trace=False)
    return out


# revision 22
# speedup vs baseline: 1.2106x; 1.0985x over previous
"""AGNNet (2-layer AGNN conv + linear head) distributed over 8 trn2 NeuronCores.

Strategy (graph/data parallel, per sharding hint):
  - nodes sharded by dst range: core c owns nodes [c*6250, (c+1)*6250)
  - host groups edges by dst, degree-sorts each core's local nodes (undone on
    output), drops self-loops (handled analytically in the softmax), and packs
    each 128-node tile's in-edge lists into per-tile slot columns
  - device: L1 matmul from host-transposed x shard; build a bf16 row table
    [h(16), inv_norm, bias, pad] per node; AllGather the table; expand it to a
    256B-strided copy in DRAM (SWDGE regular pattern, CounterMachine rate);
    per node-tile ONE dma_gather (ant extended Q7 kernel, ~7.9ns/idx) per
    index window fetches all neighbor rows; DVE softmax (no max-trick needed:
    |alpha| <= |beta|); the self-loop term exp(beta*cos(h,h)) is added
    analytically; second conv identical; head matmul + log_softmax.
  - dma_gather indices are int16, so the 50178-row table is addressed through
    TWO overlapping 32768-row windows (bases 0 and 17410); each node's slots
    are split into window-A columns then window-B columns (host balances the
    split per tile); a pad row at wide-row 17410/17411 (bias = -1e30) backs
    padding slots so they vanish in the softmax.
"""

import math
from contextlib import ExitStack
from dataclasses import dataclass

import numpy as np


@dataclass
class Cfg:
    n_cores: int = 8
    n_nodes: int = 50000
    f_in: int = 2000
    nh: int = 16
    nc_out: int = 20
    row: int = 20           # table row elems: 16 h, [16]=invn, [17]=bias, 18-19 pad
    wrow: int = 128         # wide-table row elems (256B stride for dma_gather)
    P: int = 128
    win: int = 32768        # dma_gather int16 index window (rows per base)

    @property
    def npc_raw(self) -> int:       # real nodes per core
        return self.n_nodes // self.n_cores

    @property
    def npc(self) -> int:           # padded nodes per core (mult of 128)
        return ((self.npc_raw + self.P - 1) // self.P) * self.P

    @property
    def ntiles(self) -> int:
        return self.npc // self.P

    @property
    def kchunks(self) -> int:       # 128-row chunks of the padded f_in
        return (self.f_in + self.P - 1) // self.P

    @property
    def f_pad(self) -> int:
        return self.kchunks * self.P

    @property
    def nfull(self) -> int:
        return self.n_cores * self.npc

    @property
    def nwide(self) -> int:         # wide-table rows (2 pad rows mid-table)
        return self.nfull + 2

    @property
    def wb(self) -> int:            # last window base
        return self.nwide - self.win

    @property
    def wbases(self) -> tuple:      # 3 overlapping 32768-row window bases
        return (0, self.wb // 2, self.wb)

    @property
    def pad_wrow(self) -> int:      # wide-row of the all-zero pad row
        return self.wb


FULL = Cfg()
NEG_BIG = -1.0e30


def pack_idx16(idx_mat: np.ndarray) -> np.ndarray:
    """[128 partitions, nch chunks] window-relative indices -> dma_gather
    int16 index tile [128, nch*8]: list position c*128+j lands on partition j
    chunk c (the ucode lane swizzle applies to both the index read and the
    dst partition, so it cancels); the flat list is wrapped over 16
    partitions and replicated across the 8 16-partition groups."""
    P, nch = idx_mat.shape
    assert P == 128
    flat = np.ascontiguousarray(idx_mat.T).reshape(-1).astype(np.int16)
    tile = np.empty((128, nch * 8), np.int16)
    for p in range(16):
        tile[p] = flat[p::16]
    tile[16:] = np.tile(tile[:16], (7, 1))
    return np.ascontiguousarray(tile)


def dma_gather_raw(nc, out_ap, in_ap, idxs_ap, num_idxs: int,
                   elem_size: int, stride_elems: int):
    """nc.gpsimd.dma_gather minus the elem_size%256 assert (non-transpose
    path in the Q7 ucode has no such restriction; only the stride must be a
    multiple of 256B). single_packet=False: a single packet is capped at 64
    descriptors, large gathers hang with it."""
    from concourse import mybir
    g = nc.gpsimd
    stride_bytes = stride_elems * mybir.dt.size(in_ap.dtype)
    assert stride_bytes % 256 == 0 and stride_bytes // 256 < 256
    _in_ap = g.lower_ap_dma(in_ap, for_custom_bir_dma=True)
    _idxs_ap = g.lower_ap(idxs_ap)
    _out_ap = g.lower_ap(out_ap)
    return g.add_instruction(
        mybir.InstDMAGatherAnt(
            name=g.bass.get_next_instruction_name(),
            ins=[*_in_ap, _idxs_ap, g.lower_val_access(g.to_reg(num_idxs))],
            outs=[_out_ap],
            transpose=False,
            num_idxs=num_idxs,
            elem_size=elem_size,
            stride_bytes_256=stride_bytes // 256,
            gen_mode=0,
            single_packet=False,
            queue_num=0,
            sbuf_tokens_per_rank=0,
            sbuf_free_dim_per_rank=0,
            sbuf_free_dim_pad_per_rank=0,
            sbuf_byte_offset=0,
        ))


# --------------------------------------------------------------------------
# host-side preprocessing
# --------------------------------------------------------------------------

def preprocess(cfg: Cfg, x: np.ndarray, edge_index: np.ndarray):
    """Shard + layout transform.

    Returns (per_core list of dicts, kab, order_c) where kab is the shared
    [(K1_t, K2_t, K3_t)] per tile (same across cores for SPMD): slot columns
    per index window (3 overlapping 32768-row windows of the wide table)."""
    P, NPC, NPCR = cfg.P, cfg.npc, cfg.npc_raw
    n, C, NT = cfg.n_nodes, cfg.n_cores, cfg.ntiles
    W = cfg.win
    B1, B2, B3 = cfg.wbases

    src = edge_index[0].astype(np.int64)
    dst = edge_index[1].astype(np.int64)
    keep = src != dst                     # self-loops handled analytically
    src, dst = src[keep], dst[keep]

    core_of_dst = dst // NPCR

    # pass 1: per-core degree sort -> rank of each node within its core
    order_c, rank_c = [], []
    for c in range(C):
        m = core_of_dst == c
        ld = dst[m] - c * NPCR
        deg = np.bincount(ld, minlength=NPCR)
        order = np.argsort(deg, kind="stable")      # ascending degree
        rank = np.empty(NPCR, np.int64)
        rank[order] = np.arange(NPCR)
        order_c.append(order)
        rank_c.append(rank)

    # new global id after per-core permutation + padding, then wide-row shift
    # (pad rows occupy wide rows [B3, B3+2))
    new_gid = np.empty(n, np.int64)
    for c in range(C):
        new_gid[c * NPCR:(c + 1) * NPCR] = c * NPC + rank_c[c]

    # per-core per-node edge lists sorted by wide-row
    per_core_lists = []
    for c in range(C):
        m = core_of_dst == c
        ld = dst[m] - c * NPCR
        g = new_gid[src[m]]
        gs = np.where(g < B3, g, g + 2)
        er = rank_c[c][ld]                       # dst rank of each edge
        eo = np.lexsort((gs, er))
        er_s = er[eo]
        gs_s = gs[eo]
        starts = np.zeros(NPC + 1, np.int64)
        np.cumsum(np.bincount(er_s, minlength=NPC), out=starts[1:])
        per_core_lists.append((starts, gs_s))

    # per-node class counts: s1 [0,B2) strictly-W1; f12 [B2,B3); f123
    # [B3, W) (in all); f23 [W, B2+W); s3 [B2+W, nwide)
    bnds = np.array([B2, B3, W, B2 + W], np.int64)
    cnts = np.zeros((C, 5, NPC), np.int64)
    degp_all = np.zeros((C, NPC), np.int64)
    for c in range(C):
        starts, gs_s = per_core_lists[c]
        d_cnt = np.diff(starts)
        degp_all[c] = d_cnt
        er_of_edge = np.repeat(np.arange(NPC), d_cnt)
        cls = np.searchsorted(bnds, gs_s, side="right")
        for k in range(5):
            cnts[c, k] = np.bincount(
                er_of_edge, weights=(cls == k), minlength=NPC)

    def split_node(T, s1, f12, f123, f23, s3, deg):
        """-> (n1, n3) target-T balanced; n2 = deg - n1 - n3 (all W2-ok)."""
        n1 = np.maximum(s1, np.minimum(T, s1 + f12 + f123))
        f123_left = f123 - np.maximum(0, n1 - (s1 + f12))
        n3 = np.maximum(s3, np.minimum(T, s3 + f23 + f123_left))
        return n1, n3

    kab = []
    for t in range(NT):
        s, e = t * P, (t + 1) * P
        degs = degp_all[:, s:e]
        s1, f12, f123, f23, s3 = (cnts[:, k, s:e] for k in range(5))
        kmax = int(degs.max())
        best = None
        for T in range(kmax + 1):
            n1, n3 = split_node(T, s1, f12, f123, f23, s3, degs)
            K1 = int(n1.max())
            K3 = int(n3.max())
            K2 = int((degs - n1 - n3).max())
            if best is None or K1 + K2 + K3 < best[0]:
                best = (K1 + K2 + K3, T, K1, K2, K3)
        _, T, K1, K2, K3 = best
        kab.append((max(K1, 1), max(K2, 1), max(K3, 1), T))

    # build per-core idx16 inputs
    pad_rel = (cfg.pad_wrow - B1, cfg.pad_wrow - B2, cfg.pad_wrow - B3)
    per_core = []
    for c in range(C):
        starts, gs_s = per_core_lists[c]
        packs = []
        for t in range(NT):
            K1, K2, K3, T = kab[t]
            idx1 = np.full((P, K1), pad_rel[0], np.int64)
            idx2 = np.full((P, K2), pad_rel[1], np.int64)
            idx3 = np.full((P, K3), pad_rel[2], np.int64)
            sl = slice(t * P, (t + 1) * P)
            s1, f12, f123, f23, s3 = (cnts[c, k, sl] for k in range(5))
            degs = degp_all[c, sl]
            n1, n3 = split_node(T, s1, f12, f123, f23, s3, degs)
            n1 = np.minimum(n1, K1)
            n3 = np.minimum(n3, K3)
            for p in range(P):
                node = t * P + p
                lst = gs_s[starts[node]:starts[node + 1]]  # sorted by wrow
                # classes are contiguous runs in the sorted list
                c1 = int(s1[p]); c12 = int(f12[p]); c123 = int(f123[p])
                a = int(n1[p]); b = int(n3[p])
                w1 = lst[:a]                       # s1 + leading f12/f123
                rest = lst[a:]
                w3 = rest[rest.size - b:] if b else rest[:0]
                w2 = rest[:rest.size - b]
                idx1[p, :a] = w1 - B1
                idx2[p, :w2.size] = w2 - B2
                idx3[p, :b] = w3 - B3
            for arr in (idx1, idx2, idx3):
                assert arr.min() >= 0 and arr.max() < W
            packs += [pack_idx16(idx1), pack_idx16(idx2), pack_idx16(idx3)]
        idx16 = np.concatenate(packs, axis=1)

        # x shard: permuted, padded, transposed, f-padded
        xs = x[c * NPCR:(c + 1) * NPCR][order_c[c]]      # [NPCR, f_in]
        xt = np.zeros((cfg.f_pad, NPC), np.float32)
        xt[:cfg.f_in, :NPCR] = xs.T
        per_core.append({"xt": np.ascontiguousarray(xt), "idx16": idx16})

    return per_core, kab, order_c


# --------------------------------------------------------------------------
# device kernel builder
# --------------------------------------------------------------------------

def build_kernel(cfg: Cfg, kab, phases: str = "ABCDE"):
    import concourse.bacc as bacc
    import concourse.tile as tile
    from concourse import bass, mybir
    from concourse.masks import make_identity

    P = cfg.P
    NH, NCO, ROW, WROW = cfg.nh, cfg.nc_out, cfg.row, cfg.wrow
    NPC, NT, KC = cfg.npc, cfg.ntiles, cfg.kchunks
    NFULL, NWIDE, WB, W = cfg.nfull, cfg.nwide, cfg.wb, cfg.win
    f32 = mybir.dt.float32
    bf16 = mybir.dt.bfloat16
    i16 = mybir.dt.int16
    AX = mybir.AxisListType.X
    OP = mybir.AluOpType
    AF = mybir.ActivationFunctionType
    idxw = sum(8 * (k1 + k2 + k3) for (k1, k2, k3, _) in kab)
    WBASES = cfg.wbases

    nc = bacc.Bacc("TRN2", target_bir_lowering=False, debug=False,
                   num_devices=cfg.n_cores)

    xt_d = nc.dram_tensor("xt", [cfg.f_pad, NPC], bf16, kind="ExternalInput")
    idx_d = nc.dram_tensor("idx16", [P, idxw], i16, kind="ExternalInput")
    w1_d = nc.dram_tensor("w1p", [P, KC * NH], bf16, kind="ExternalInput")
    b1_d = nc.dram_tensor("b1r", [P, NH], f32, kind="ExternalInput")
    w4_d = nc.dram_tensor("w4r", [NH, NCO], f32, kind="ExternalInput")
    b4_d = nc.dram_tensor("b4r", [P, NCO], f32, kind="ExternalInput")
    be_d = nc.dram_tensor("beta3r", [P, 1], f32, kind="ExternalInput")
    out_d = nc.dram_tensor("out", [NPC, NCO], f32, kind="ExternalOutput")

    tabA_l = nc.dram_tensor("tabA_l", [NPC, ROW], bf16)
    tabA_f = nc.dram_tensor("tabA_f", [NFULL + 2, ROW], bf16, addr_space="Shared")
    tabB_l = nc.dram_tensor("tabB_l", [NPC, ROW], bf16)
    tabB_f = nc.dram_tensor("tabB_f", [NFULL + 2, ROW], bf16, addr_space="Shared")
    tabA_w = nc.dram_tensor("tabA_w", [NWIDE, WROW], bf16)
    tabB_w = nc.dram_tensor("tabB_w", [NWIDE, WROW], bf16)

    # persistent SBUF (survives across TileContexts)
    def sb(name, shape, dtype):
        return nc.alloc_sbuf_tensor(name, list(shape), dtype)

    ident_f = sb("identf", [P, P], f32)
    zeros = sb("zeros", [P, P], f32)
    w1_sb = sb("w1sb", [P, KC * NH], bf16)
    b1_sb = sb("b1sb", [P, NH], f32)
    w4_sb = sb("w4sb", [NH, NCO], f32)
    b4_sb = sb("b4sb", [P, NCO], f32)
    be_sb = sb("besb", [P, 1], f32)
    bee_sb = sb("beesb", [P, 1], f32)      # exp-ready: beta3 value
    h_sb = sb("hsb", [P, NT * NH], f32)
    rows_sb = sb("rowssb", [P, NT * ROW], bf16)
    sq_sb = sb("sqsb", [P, NT * NH], f32)
    ss_sb = sb("sssb", [P, NT], f32)
    inv_sb = sb("invsb", [P, NT], f32)
    idx_sb = sb("idxsb", [P, idxw], i16)    # all tiles' gather indices

    def epilogue_rows(tab_local):
        """h_sb -> inv norm -> rows_sb -> DMA to tab_local."""
        h3v = h_sb.ap().rearrange("p (t j) -> p t j", t=NT)
        nc.vector.tensor_mul(sq_sb.ap(), h_sb.ap(), h_sb.ap())
        nc.vector.reduce_sum(
            ss_sb.ap(), sq_sb.ap().rearrange("p (t j) -> p t j", t=NT), axis=AX)
        nc.scalar.sqrt(ss_sb.ap(), ss_sb.ap())
        nc.vector.tensor_scalar_add(ss_sb.ap(), ss_sb.ap(), 1.0e-12)
        nc.vector.reciprocal(inv_sb.ap(), ss_sb.ap())
        rv = rows_sb.ap().rearrange("p (t j) -> p t j", t=NT)
        nc.vector.tensor_copy(rv[:, :, 0:16], h3v)
        nc.vector.tensor_copy(rv[:, :, 16], inv_sb.ap())
        nc.vector.tensor_copy(rv[:, :, 17], zeros.ap()[:, 0:NT])
        nc.sync.dma_start(
            out=tab_local[:, :].rearrange("(t p) j -> p t j", p=P),
            in_=rv)

    def allgather(tab_local, tab_full):
        nc.gpsimd.collective_compute(
            "AllGather", OP.bypass,
            replica_groups=[list(range(cfg.n_cores))],
            ins=[tab_local.ap().opt()],
            outs=[tab_full.ap()[0:NFULL, :].opt()])

    def expand(tab_full, tab_wide):
        """packed [NFULL+2, 20] -> 256B-strided [NWIDE, 128] (cols 0:20).

        Regular-pattern SWDGE DMAs (CounterMachine rate ~0.34ns/desc),
        chunked under the 16384-descriptor SWDGE carveout limit.
        wide rows [0, WB) <- packed [0, WB); [WB, WB+2) <- packed pads
        [NFULL, NFULL+2); [WB+2, NWIDE) <- packed [WB, NFULL)."""
        CH = 12288

        def copy(wlo, whi, plo):
            for o in range(0, whi - wlo, CH):
                n = min(CH, whi - wlo - o)
                nc.gpsimd.dma_start(
                    out=tab_wide[wlo + o:wlo + o + n, 0:ROW],
                    in_=tab_full[plo + o:plo + o + n, :])

        copy(0, WB, 0)
        copy(WB, WB + 2, NFULL)
        copy(WB + 2, NWIDE, WB)

    # ---------------- phase A: consts, idx preload, L1, table A, AG1 -------
    with tile.TileContext(nc) as tc:
        make_identity(nc, ident_f.ap())
        nc.gpsimd.memset(zeros.ap(), 0.0)
        nc.gpsimd.memset(rows_sb.ap(), 0.0)
        nc.sync.dma_start(out=idx_sb.ap(), in_=idx_d[:, :])
        nc.sync.dma_start(out=w1_sb.ap(), in_=w1_d[:, :])
        nc.sync.dma_start(out=b1_sb.ap(), in_=b1_d[:, :])
        nc.sync.dma_start(out=w4_sb.ap(), in_=w4_d[:, :])
        nc.sync.dma_start(out=b4_sb.ap(), in_=b4_d[:, :])
        nc.sync.dma_start(out=be_sb.ap(), in_=be_d[:, :])
        with ExitStack() as ctx:
            const = ctx.enter_context(tc.tile_pool(name="pad", bufs=1))
            padrow = const.tile([1, ROW], bf16, tag="padrow")
            nc.gpsimd.memset(padrow[:], 0.0)
            nc.gpsimd.memset(padrow[:1, 17:18], NEG_BIG)
            nc.sync.dma_start(
                out=tabA_f[NFULL:NFULL + 2, :][None, :, :],
                in_=padrow[:1, None, :].to_broadcast([1, 2, ROW]))
            nc.sync.dma_start(
                out=tabB_f[NFULL:NFULL + 2, :][None, :, :],
                in_=padrow[:1, None, :].to_broadcast([1, 2, ROW]))

        with tc.tile_pool(name="l1x", bufs=3) as xp, \
             tc.tile_pool(name="l1p", bufs=4, space="PSUM") as pp:
            for t in range(NT):
                xw = xp.tile([P, KC * P], bf16, tag="xw")
                src = xt_d[:, :].rearrange("(c p) m -> p c m", p=P)[:, :, t * P:(t + 1) * P]
                nc.sync.dma_start(
                    out=xw[:].rearrange("p (c j) -> p c j", c=KC), in_=src)
                ps = pp.tile([P, NH], f32, tag="l1ps")
                for c in range(KC):
                    nc.tensor.matmul(
                        out=ps[:], lhsT=xw[:, c * P:(c + 1) * P],
                        rhs=w1_sb.ap()[:, c * NH:(c + 1) * NH],
                        start=(c == 0), stop=(c == KC - 1))
                hsl = h_sb.ap()[:, t * NH:(t + 1) * NH]
                nc.vector.tensor_add(hsl, ps[:], b1_sb.ap())
                nc.vector.tensor_scalar_max(hsl, hsl, 0.0)
        epilogue_rows(tabA_l)
        allgather(tabA_l, tabA_f)
    with tile.TileContext(nc) as tc:
        expand(tabA_f, tabA_w)

    # ---------------- conv layer -------------------------------------------
    def conv(tab_local, tab_wide, beta_ap):
        """beta_ap: None for beta=1 (conv2), else [P,1] AP with beta value."""
        off = 0
        with tile.TileContext(nc) as tc:
            with tc.tile_pool(name="cv", bufs=3) as cv, \
                 tc.tile_pool(name="cvs", bufs=2) as cvs:
                for t in range(NT):
                    kws = kab[t][:3]
                    K = sum(kws)
                    hs = cv.tile([P, K * ROW], bf16, tag="hs")
                    kacc = 0
                    for (kw, base) in zip(kws, WBASES):
                        dma_gather_raw(
                            nc,
                            out_ap=hs[:, kacc * ROW:(kacc + kw) * ROW]
                            .rearrange("p (c j) -> p c j", j=ROW),
                            in_ap=tab_wide[base:base + W, 0:ROW],
                            idxs_ap=idx_sb.ap()[:, off:off + 8 * kw],
                            num_idxs=P * kw, elem_size=ROW,
                            stride_elems=WROW)
                        off += 8 * kw
                        kacc += kw

                    hd = cvs.tile([P, ROW], bf16, tag="hd")
                    nc.sync.dma_start(
                        out=hd[:], in_=tab_local[t * P:(t + 1) * P, :]
                        .rearrange("(g p) j -> p (g j)", p=P))

                    hs3 = hs[:].rearrange("p (k j) -> p k j", k=K)
                    tmp = cv.tile([P, K * NH], bf16, tag="tmp")
                    tm3 = tmp[:].rearrange("p (k j) -> p k j", k=K)
                    nc.vector.tensor_mul(
                        tm3, hs3[:, :, 0:16],
                        hd[:, None, 0:16].to_broadcast([P, K, 16]))
                    alpha = cv.tile([P, K], f32, tag="alpha")
                    nc.vector.reduce_sum(
                        alpha[:],
                        tmp[:].rearrange("p (k j) -> p k j", j=NH), axis=AX)
                    nc.vector.tensor_mul(alpha[:], alpha[:], hs3[:, :, 16])
                    invd = cvs.tile([P, 1], f32, tag="invd")
                    if beta_ap is None:
                        nc.vector.tensor_copy(invd[:], hd[:, 16:17])
                    else:
                        nc.vector.tensor_mul(invd[:], hd[:, 16:17], beta_ap)
                    nc.vector.tensor_mul(
                        alpha[:], alpha[:],
                        invd[:].to_broadcast([P, K]))
                    # pad slots: hs row is zeros with bias=-1e30 -> alpha=-1e30
                    nc.vector.tensor_add(alpha[:], alpha[:], hs3[:, :, 17])
                    # |alpha| <= |beta| for real slots: exp without max-trick
                    e_bf = cv.tile([P, K], bf16, tag="e")
                    nc.scalar.activation(e_bf[:], alpha[:], AF.Exp)
                    s = cvs.tile([P, 1], f32, tag="s")
                    nc.vector.reduce_sum(
                        s[:], e_bf[:][:, None, :], axis=AX)
                    # analytic self-loop: cos(h,h) = (1 - 1e-12*invn)^2
                    selfa = cvs.tile([P, 1], f32, tag="selfa")
                    nc.vector.tensor_scalar(
                        selfa[:], hd[:, 16:17], -1.0e-12, 1.0,
                        op0=OP.mult, op1=OP.add)
                    nc.vector.tensor_mul(selfa[:], selfa[:], selfa[:])
                    if beta_ap is not None:
                        nc.vector.tensor_mul(selfa[:], selfa[:], beta_ap)
                    es = cvs.tile([P, 1], f32, tag="es")
                    nc.scalar.activation(es[:], selfa[:], AF.Exp)
                    nc.vector.tensor_add(s[:], s[:], es[:])
                    nc.vector.tensor_scalar_add(s[:], s[:], 1.0e-16)
                    r = cvs.tile([P, 1], f32, tag="r")
                    nc.vector.reciprocal(r[:], s[:])
                    coef = cv.tile([P, K], bf16, tag="coef")
                    nc.vector.tensor_mul(
                        coef[:], e_bf[:], r[:].to_broadcast([P, K]))
                    tmp2 = cv.tile([P, K * NH], bf16, tag="tmp2")
                    t23 = tmp2[:].rearrange("p (k j) -> p k j", k=K)
                    nc.vector.tensor_mul(
                        t23, hs3[:, :, 0:16],
                        coef[:][:, :, None].to_broadcast([P, K, 16]))
                    h2v = h_sb.ap()[:, t * NH:(t + 1) * NH]
                    nc.vector.reduce_sum(
                        h2v,
                        tmp2[:].rearrange("p (k j) -> p j k", k=K),
                        axis=AX)
                    # += (exp(self)/s) * h_d
                    rs = cvs.tile([P, 1], f32, tag="rs")
                    nc.vector.tensor_mul(rs[:], es[:], r[:])
                    sh = cvs.tile([P, NH], f32, tag="sh")
                    nc.vector.tensor_mul(
                        sh[:], hd[:, 0:16], rs[:].to_broadcast([P, NH]))
                    nc.vector.tensor_add(h2v, h2v, sh[:])
        assert off == idxw

    if "B" in phases:
        conv(tabA_l, tabA_w, None)

    # ---------------- phase C: table B + AG2 -------------------------------
    if "C" in phases:
        with tile.TileContext(nc) as tc:
            epilogue_rows(tabB_l)
            allgather(tabB_l, tabB_f)
        with tile.TileContext(nc) as tc:
            expand(tabB_f, tabB_w)

    if "D" in phases:
        conv(tabB_l, tabB_w, be_sb.ap()[:, 0:1])

    if "E" not in phases:
        # debug: dump h_sb (and inv_sb) into out
        with tile.TileContext(nc) as tc:
            ov = out_d.ap().rearrange("(p q) j -> p (q j)", p=P)
            nc.sync.dma_start(out=ov[:, 0:NT * NH], in_=h_sb.ap())
            nc.sync.dma_start(out=ov[:, NT * NH:NT * NH + NT], in_=inv_sb.ap())
        nc.compile()
        return nc

    with tile.TileContext(nc) as tc:
        with tc.tile_pool(name="hd", bufs=1) as hp, \
             tc.tile_pool(name="hdp", bufs=4, space="PSUM") as hpp:
            h3t = hp.tile([NH, NT * P], f32, tag="h3t")
            for t in range(NT):
                pst = hpp.tile([NH, P], f32, tag="pst")
                nc.tensor.transpose(
                    out=pst[:], in_=h_sb.ap()[:, t * NH:(t + 1) * NH],
                    identity=ident_f.ap())
                nc.vector.tensor_copy(h3t[:, t * P:(t + 1) * P], pst[:])
            lg = hp.tile([P, NT * NCO], f32, tag="lg")
            for t in range(NT):
                psl = hpp.tile([P, NCO], f32, tag="psl")
                nc.tensor.matmul(
                    out=psl[:], lhsT=h3t[:, t * P:(t + 1) * P], rhs=w4_sb.ap(),
                    start=True, stop=True)
                nc.vector.tensor_add(
                    lg[:, t * NCO:(t + 1) * NCO], psl[:], b4_sb.ap())
            lg3 = lg[:].rearrange("p (t j) -> p t j", t=NT)
            nm = hp.tile([P, NT], f32, tag="hnm")
            nc.vector.reduce_max(nm[:], lg3, axis=AX, negate=True)
            nc.vector.tensor_add(
                lg3, lg3, nm[:][:, :, None].to_broadcast([P, NT, NCO]))
            ex = hp.tile([P, NT * NCO], f32, tag="ex")
            nc.scalar.activation(ex[:], lg[:], AF.Exp)
            s = hp.tile([P, NT], f32, tag="hs_sum")
            nc.vector.reduce_sum(
                s[:], ex[:].rearrange("p (t j) -> p t j", t=NT), axis=AX)
            ls = hp.tile([P, NT], f32, tag="ls")
            nc.scalar.activation(ls[:], s[:], AF.Ln)
            nc.vector.tensor_sub(
                lg3, lg3, ls[:][:, :, None].to_broadcast([P, NT, NCO]))
            nc.sync.dma_start(
                out=out_d[:, :].rearrange("(t p) j -> p t j", p=P),
                in_=lg3)

    nc.compile()
    return nc


# --------------------------------------------------------------------------
# entry point
# --------------------------------------------------------------------------

def run(cfg: Cfg, inputs: dict, trace: bool = False):
    from concourse import bass_utils

    x = np.asarray(inputs["x"], np.float32)
    edge_index = np.asarray(inputs["edge_index"])
    W1 = np.asarray(inputs["W1"], np.float32)
    b1 = np.asarray(inputs["b1"], np.float32)
    W4 = np.asarray(inputs["W4"], np.float32)
    b4 = np.asarray(inputs["b4"], np.float32)
    beta3 = np.asarray(inputs["beta3"], np.float32)

    import ml_dtypes

    per_core, kab, order_c = preprocess(cfg, x, edge_index)
    nc = build_kernel(cfg, kab)

    P, KC, NH = cfg.P, cfg.kchunks, cfg.nh
    w1p = np.zeros((cfg.f_pad, NH), np.float32)
    w1p[:cfg.f_in] = W1
    w1p = np.ascontiguousarray(
        w1p.reshape(KC, P, NH).transpose(1, 0, 2).reshape(P, KC * NH)
    ).astype(ml_dtypes.bfloat16)
    b1r = np.ascontiguousarray(np.broadcast_to(b1[None, :], (P, NH)))
    b4r = np.ascontiguousarray(np.broadcast_to(b4[None, :], (P, cfg.nc_out)))
    ber = np.ascontiguousarray(np.broadcast_to(beta3[None, :], (P, 1)))

    in_maps = []
    for c in range(cfg.n_cores):
        in_maps.append({
            "xt": per_core[c]["xt"].astype(ml_dtypes.bfloat16),
            "idx16": per_core[c]["idx16"],
            "w1p": w1p, "b1r": b1r, "w4r": np.ascontiguousarray(W4),
            "b4r": b4r, "beta3r": ber,
        })

    res = bass_utils.run_bass_kernel_spmd(
        nc, in_maps, core_ids=list(range(cfg.n_cores)), trace=trace)

    out = np.empty((cfg.n_nodes, cfg.nc_out), np.float32)
    for c in range(cfg.n_cores):
        oc = np.asarray(res.results[c]["out"])[:cfg.npc_raw]
        out[c * cfg.npc_raw + order_c[c]] = oc
    return out, res


def kernel(**inputs) -> np.ndarray:
    out, _ = run(FULL, inputs, trace=False)
    return out


# revision 37
# speedup vs baseline: 1.2603x; 1.0410x over previous
"""AGNNet (2-layer AGNN conv + linear head) distributed over 8 trn2 NeuronCores.

Strategy (graph/data parallel, per sharding hint):
  - nodes sharded by dst range: core c owns nodes [c*6250, (c+1)*6250)
  - host groups edges by dst, degree-sorts each core's local nodes (undone on
    output), drops self-loops (handled analytically in the softmax), and packs
    each 128-node tile's in-edge lists into per-tile slot columns
  - device: L1 matmul from host-transposed x shard; build a bf16 row table
    [h(16), inv_norm, bias, pad] per node; AllGather the table; expand it to a
    256B-strided copy in DRAM (SWDGE regular pattern, CounterMachine rate);
    per node-tile ONE dma_gather (ant extended Q7 kernel, ~7.9ns/idx) per
    index window fetches all neighbor rows; DVE softmax (no max-trick needed:
    |alpha| <= |beta|); the self-loop term exp(beta*cos(h,h)) is added
    analytically; second conv identical; head matmul + log_softmax.
  - dma_gather indices are int16, so the 50178-row table is addressed through
    TWO overlapping 32768-row windows (bases 0 and 17410); each node's slots
    are split into window-A columns then window-B columns (host balances the
    split per tile); a pad row at wide-row 17410/17411 (bias = -1e30) backs
    padding slots so they vanish in the softmax.
"""

import math
from contextlib import ExitStack
from dataclasses import dataclass

import numpy as np


@dataclass
class Cfg:
    n_cores: int = 8
    n_nodes: int = 50000
    f_in: int = 2000
    nh: int = 16
    nc_out: int = 20
    row: int = 20           # table row elems: 16 h, [16]=invn, [17]=bias, 18-19 pad
    wrow: int = 128         # wide-table row elems (256B stride for dma_gather)
    P: int = 128
    win: int = 32768        # dma_gather int16 index window (rows per base)

    @property
    def npc_raw(self) -> int:       # real nodes per core
        return self.n_nodes // self.n_cores

    @property
    def npc(self) -> int:           # padded nodes per core (mult of 128)
        return ((self.npc_raw + self.P - 1) // self.P) * self.P

    @property
    def ntiles(self) -> int:
        return self.npc // self.P

    @property
    def kchunks(self) -> int:       # 128-row chunks of the padded f_in
        return (self.f_in + self.P - 1) // self.P

    @property
    def f_pad(self) -> int:
        return self.kchunks * self.P

    @property
    def nfull(self) -> int:
        return self.n_cores * self.npc

    @property
    def wb(self) -> int:            # last window base
        return self.nfull - self.win

    @property
    def wbases(self) -> tuple:      # 3 overlapping 32768-row window bases
        return (0, self.wb // 2, self.wb)


FULL = Cfg()
NEG_BIG = -1.0e30


def pack_idx16(idx_mat: np.ndarray) -> np.ndarray:
    """[128 partitions, nch chunks] window-relative indices -> dma_gather
    int16 index tile [128, nch*8]: list position c*128+j lands on partition j
    chunk c (the ucode lane swizzle applies to both the index read and the
    dst partition, so it cancels); the flat list is wrapped over 16
    partitions and replicated across the 8 16-partition groups."""
    P, nch = idx_mat.shape
    assert P == 128
    flat = np.ascontiguousarray(idx_mat.T).reshape(-1).astype(np.int16)
    tile = np.empty((128, nch * 8), np.int16)
    for p in range(16):
        tile[p] = flat[p::16]
    tile[16:] = np.tile(tile[:16], (7, 1))
    return np.ascontiguousarray(tile)


def dma_gather_raw(nc, out_ap, in_ap, idxs_ap, num_idxs: int,
                   elem_size: int, stride_elems: int):
    """nc.gpsimd.dma_gather minus the elem_size%256 assert (non-transpose
    path in the Q7 ucode has no such restriction; only the stride must be a
    multiple of 256B). single_packet=False: a single packet is capped at 64
    descriptors, large gathers hang with it."""
    from concourse import mybir
    g = nc.gpsimd
    stride_bytes = stride_elems * mybir.dt.size(in_ap.dtype)
    assert stride_bytes % 256 == 0 and stride_bytes // 256 < 256
    _in_ap = g.lower_ap_dma(in_ap, for_custom_bir_dma=True)
    _idxs_ap = g.lower_ap(idxs_ap)
    _out_ap = g.lower_ap(out_ap)
    return g.add_instruction(
        mybir.InstDMAGatherAnt(
            name=g.bass.get_next_instruction_name(),
            ins=[*_in_ap, _idxs_ap, g.lower_val_access(g.to_reg(num_idxs))],
            outs=[_out_ap],
            transpose=False,
            num_idxs=num_idxs,
            elem_size=elem_size,
            stride_bytes_256=stride_bytes // 256,
            gen_mode=0,
            single_packet=False,
            queue_num=0,
            sbuf_tokens_per_rank=0,
            sbuf_free_dim_per_rank=0,
            sbuf_free_dim_pad_per_rank=0,
            sbuf_byte_offset=0,
        ))


# --------------------------------------------------------------------------
# host-side preprocessing
# --------------------------------------------------------------------------

def preprocess(cfg: Cfg, x: np.ndarray, edge_index: np.ndarray):
    """Shard + layout transform.

    Returns (per_core list of dicts, kab, order_c) where kab is the shared
    [(K1_t, K2_t, K3_t)] per tile (same across cores for SPMD): slot columns
    per index window (3 overlapping 32768-row windows of the wide table)."""
    P, NPC, NPCR = cfg.P, cfg.npc, cfg.npc_raw
    n, C, NT = cfg.n_nodes, cfg.n_cores, cfg.ntiles
    W = cfg.win
    B1, B2, B3 = cfg.wbases

    src = edge_index[0].astype(np.int64)
    dst = edge_index[1].astype(np.int64)
    keep = src != dst                     # self-loops handled analytically
    src, dst = src[keep], dst[keep]

    core_of_dst = dst // NPCR

    # pass 1: per-core degree sort -> rank of each node within its core
    order_c, rank_c = [], []
    for c in range(C):
        m = core_of_dst == c
        ld = dst[m] - c * NPCR
        deg = np.bincount(ld, minlength=NPCR)
        order = np.argsort(deg, kind="stable")      # ascending degree
        rank = np.empty(NPCR, np.int64)
        rank[order] = np.arange(NPCR)
        order_c.append(order)
        rank_c.append(rank)

    # new global id after per-core permutation + padding (= wide-table row)
    new_gid = np.empty(n, np.int64)
    for c in range(C):
        new_gid[c * NPCR:(c + 1) * NPCR] = c * NPC + rank_c[c]

    # per-core per-node edge lists sorted by wide-row
    per_core_lists = []
    for c in range(C):
        m = core_of_dst == c
        ld = dst[m] - c * NPCR
        gs = new_gid[src[m]]
        er = rank_c[c][ld]                       # dst rank of each edge
        eo = np.lexsort((gs, er))
        er_s = er[eo]
        gs_s = gs[eo]
        starts = np.zeros(NPC + 1, np.int64)
        np.cumsum(np.bincount(er_s, minlength=NPC), out=starts[1:])
        per_core_lists.append((starts, gs_s))

    # per-node class counts: s1 [0,B2) strictly-W1; f12 [B2,B3); f123
    # [B3, W) (in all); f23 [W, B2+W); s3 [B2+W, nfull)
    bnds = np.array([B2, B3, W, B2 + W], np.int64)
    cnts = np.zeros((C, 5, NPC), np.int64)
    degp_all = np.zeros((C, NPC), np.int64)
    for c in range(C):
        starts, gs_s = per_core_lists[c]
        d_cnt = np.diff(starts)
        degp_all[c] = d_cnt
        er_of_edge = np.repeat(np.arange(NPC), d_cnt)
        cls = np.searchsorted(bnds, gs_s, side="right")
        for k in range(5):
            cnts[c, k] = np.bincount(
                er_of_edge, weights=(cls == k), minlength=NPC)

    def split_node(T, s1, f12, f123, f23, s3, deg):
        """-> (n1, n3) target-T balanced; n2 = deg - n1 - n3 (all W2-ok)."""
        n1 = np.maximum(s1, np.minimum(T, s1 + f12 + f123))
        f123_left = f123 - np.maximum(0, n1 - (s1 + f12))
        n3 = np.maximum(s3, np.minimum(T, s3 + f23 + f123_left))
        return n1, n3

    kab = []
    for t in range(NT):
        s, e = t * P, (t + 1) * P
        degs = degp_all[:, s:e]
        s1, f12, f123, f23, s3 = (cnts[:, k, s:e] for k in range(5))
        kmax = int(degs.max())
        best = None
        for T in range(kmax + 1):
            n1, n3 = split_node(T, s1, f12, f123, f23, s3, degs)
            K1 = int(n1.max())
            K3 = int(n3.max())
            K2 = int((degs - n1 - n3).max())
            if best is None or K1 + K2 + K3 < best[0]:
                best = (K1 + K2 + K3, T, K1, K2, K3)
        _, T, K1, K2, K3 = best
        kab.append((max(K1, 1), max(K2, 1), max(K3, 1), T))

    # build per-core idx16 + per-slot mask inputs (pad slots point at the
    # window base row -- a real row -- and are killed by a -1e30 mask)
    per_core = []
    for c in range(C):
        starts, gs_s = per_core_lists[c]
        packs = []
        masks = []
        for t in range(NT):
            K1, K2, K3, T = kab[t]
            idx1 = np.zeros((P, K1), np.int64)
            idx2 = np.zeros((P, K2), np.int64)
            idx3 = np.zeros((P, K3), np.int64)
            mask = np.full((P, K1 + K2 + K3), NEG_BIG, np.float32)
            sl = slice(t * P, (t + 1) * P)
            s1, f12, f123, f23, s3 = (cnts[c, k, sl] for k in range(5))
            degs = degp_all[c, sl]
            n1, n3 = split_node(T, s1, f12, f123, f23, s3, degs)
            n1 = np.minimum(n1, K1)
            n3 = np.minimum(n3, K3)
            for p in range(P):
                node = t * P + p
                lst = gs_s[starts[node]:starts[node + 1]]  # sorted by wrow
                a = int(n1[p]); b = int(n3[p])
                w1 = lst[:a]                       # s1 + leading f12/f123
                rest = lst[a:]
                w3 = rest[rest.size - b:] if b else rest[:0]
                w2 = rest[:rest.size - b]
                idx1[p, :a] = w1 - B1
                idx2[p, :w2.size] = w2 - B2
                idx3[p, :b] = w3 - B3
                mask[p, :a] = 0.0
                mask[p, K1:K1 + w2.size] = 0.0
                mask[p, K1 + K2:K1 + K2 + b] = 0.0
            for arr in (idx1, idx2, idx3):
                assert arr.min() >= 0 and arr.max() < W
            packs += [pack_idx16(idx1), pack_idx16(idx2), pack_idx16(idx3)]
            masks.append(mask)
        idx16 = np.concatenate(packs, axis=1)
        maskv = np.concatenate(masks, axis=1)

        # x shard: permuted, padded, transposed, f-padded
        xs = x[c * NPCR:(c + 1) * NPCR][order_c[c]]      # [NPCR, f_in]
        xt = np.zeros((cfg.f_pad, NPC), np.float32)
        xt[:cfg.f_in, :NPCR] = xs.T
        per_core.append({"xt": np.ascontiguousarray(xt), "idx16": idx16,
                         "mask": maskv})

    return per_core, kab, order_c


# --------------------------------------------------------------------------
# device kernel builder
# --------------------------------------------------------------------------

def build_kernel(cfg: Cfg, kab, phases: str = "ABCDE"):
    import concourse.bacc as bacc
    import concourse.tile as tile
    from concourse import bass, mybir
    from concourse.masks import make_identity

    P = cfg.P
    NH, NCO, ROW, WROW = cfg.nh, cfg.nc_out, cfg.row, cfg.wrow
    NPC, NT, KC = cfg.npc, cfg.ntiles, cfg.kchunks
    NFULL, WB, W = cfg.nfull, cfg.wb, cfg.win
    f32 = mybir.dt.float32
    bf16 = mybir.dt.bfloat16
    i16 = mybir.dt.int16
    AX = mybir.AxisListType.X
    OP = mybir.AluOpType
    AF = mybir.ActivationFunctionType
    idxw = sum(8 * (k1 + k2 + k3) for (k1, k2, k3, _) in kab)
    sumk = idxw // 8
    WBASES = cfg.wbases
    RPP = NFULL // P                # wide-table rows per partition (bounce)

    nc = bacc.Bacc("TRN2", target_bir_lowering=False, debug=False,
                   num_devices=cfg.n_cores)

    xt_d = nc.dram_tensor("xt", [cfg.f_pad, NPC], bf16, kind="ExternalInput")
    idx_d = nc.dram_tensor("idx16", [P, idxw], i16, kind="ExternalInput")
    mask_d = nc.dram_tensor("mask", [P, sumk], bf16, kind="ExternalInput")
    w1_d = nc.dram_tensor("w1p", [P, KC * NH], bf16, kind="ExternalInput")
    b1_d = nc.dram_tensor("b1r", [P, NH], f32, kind="ExternalInput")
    w4_d = nc.dram_tensor("w4r", [NH, NCO], f32, kind="ExternalInput")
    b4_d = nc.dram_tensor("b4r", [P, NCO], f32, kind="ExternalInput")
    be_d = nc.dram_tensor("beta3r", [P, 1], f32, kind="ExternalInput")
    out_d = nc.dram_tensor("out", [NPC, NCO], f32, kind="ExternalOutput")

    tabA_l = nc.dram_tensor("tabA_l", [NPC, ROW], bf16)
    tabA_f = nc.dram_tensor("tabA_f", [NFULL, ROW], bf16, addr_space="Shared")
    tabB_l = nc.dram_tensor("tabB_l", [NPC, ROW], bf16)
    tabB_f = nc.dram_tensor("tabB_f", [NFULL, ROW], bf16, addr_space="Shared")
    tabA_w = nc.dram_tensor("tabA_w", [NFULL, WROW], bf16)
    tabB_w = nc.dram_tensor("tabB_w", [NFULL, WROW], bf16)

    # persistent SBUF (survives across TileContexts)
    def sb(name, shape, dtype):
        return nc.alloc_sbuf_tensor(name, list(shape), dtype)

    ident_f = sb("identf", [P, P], f32)
    w1_sb = sb("w1sb", [P, KC * NH], bf16)
    b1_sb = sb("b1sb", [P, NH], f32)
    w4_sb = sb("w4sb", [NH, NCO], f32)
    b4_sb = sb("b4sb", [P, NCO], f32)
    be_sb = sb("besb", [P, 1], f32)
    bee_sb = sb("beesb", [P, 1], f32)      # exp-ready: beta3 value
    h_sb = sb("hsb", [P, NT * NH], f32)
    rows_sb = sb("rowssb", [P, NT * ROW], bf16)
    sq_sb = sb("sqsb", [P, NT * NH], f32)
    ss_sb = sb("sssb", [P, NT], f32)
    inv_sb = sb("invsb", [P, NT], f32)
    idx_sb = sb("idxsb", [P, idxw], i16)    # all tiles' gather indices
    mask_sb = sb("masksb", [P, sumk], bf16)  # per-slot pad masks
    wide_sb = sb("widesb", [P, RPP * WROW], bf16)  # table expand bounce

    def epilogue_rows(tab_local):
        """h_sb -> inv norm -> rows_sb -> DMA to tab_local."""
        h3v = h_sb.ap().rearrange("p (t j) -> p t j", t=NT)
        nc.vector.tensor_mul(sq_sb.ap(), h_sb.ap(), h_sb.ap())
        nc.vector.reduce_sum(
            ss_sb.ap(), sq_sb.ap().rearrange("p (t j) -> p t j", t=NT), axis=AX)
        nc.scalar.sqrt(ss_sb.ap(), ss_sb.ap())
        nc.vector.tensor_scalar_add(ss_sb.ap(), ss_sb.ap(), 1.0e-12)
        nc.vector.reciprocal(inv_sb.ap(), ss_sb.ap())
        rv = rows_sb.ap().rearrange("p (t j) -> p t j", t=NT)
        nc.vector.tensor_copy(rv[:, :, 0:16], h3v)
        nc.vector.tensor_copy(rv[:, :, 16], inv_sb.ap())
        nc.sync.dma_start(
            out=tab_local[:, :].rearrange("(t p) j -> p t j", p=P),
            in_=rv)

    def allgather(tab_local, tab_full):
        nc.gpsimd.collective_compute(
            "AllGather", OP.bypass,
            replica_groups=[list(range(cfg.n_cores))],
            ins=[tab_local.ap().opt()],
            outs=[tab_full.ap()[0:NFULL, :].opt()])

    def expand(tab_full, tab_wide):
        """packed [NFULL, 20] -> 256B-strided [NFULL, 128] (cols 0:20).

        Bounced through SBUF: (1) strided DRAM->SBUF load via SWDGE regular
        pattern (avoids the 40B DRAM-write read-modify-write penalty that
        makes a direct DRAM->DRAM expand drain at ~45ns/desc); (2) one bulk
        contiguous SBUF->DRAM store at line rate. Chunked under the
        16384-descriptor SWDGE carveout limit."""
        wv = wide_sb.ap().rearrange("p (r j) -> p r j", j=WROW)
        CH = 96                      # rows-per-partition chunk (<=16k descs)
        for o in range(0, RPP, CH):
            n = min(CH, RPP - o)
            nc.gpsimd.dma_start(
                out=wv[:, o:o + n, 0:ROW],
                in_=tab_full[:, :].rearrange(
                    "(p r) j -> p r j", p=P)[:, o:o + n, :])
        nc.sync.dma_start(
            out=tab_wide[:, :].rearrange("(p r) j -> p (r j)", p=P),
            in_=wide_sb.ap())

    # ---------------- phase A: consts, idx preload, L1, table A, AG1 -------
    with tile.TileContext(nc) as tc:
        make_identity(nc, ident_f.ap())
        nc.gpsimd.memset(rows_sb.ap(), 0.0)
        nc.sync.dma_start(out=idx_sb.ap(), in_=idx_d[:, :])
        nc.sync.dma_start(out=mask_sb.ap(), in_=mask_d[:, :])
        nc.sync.dma_start(out=w1_sb.ap(), in_=w1_d[:, :])
        nc.sync.dma_start(out=b1_sb.ap(), in_=b1_d[:, :])
        nc.sync.dma_start(out=w4_sb.ap(), in_=w4_d[:, :])
        nc.sync.dma_start(out=b4_sb.ap(), in_=b4_d[:, :])
        nc.sync.dma_start(out=be_sb.ap(), in_=be_d[:, :])

        with tc.tile_pool(name="l1x", bufs=3) as xp, \
             tc.tile_pool(name="l1p", bufs=4, space="PSUM") as pp:
            for t in range(NT):
                xw = xp.tile([P, KC * P], bf16, tag="xw")
                src = xt_d[:, :].rearrange("(c p) m -> p c m", p=P)[:, :, t * P:(t + 1) * P]
                nc.sync.dma_start(
                    out=xw[:].rearrange("p (c j) -> p c j", c=KC), in_=src)
                ps = pp.tile([P, NH], f32, tag="l1ps")
                for c in range(KC):
                    nc.tensor.matmul(
                        out=ps[:], lhsT=xw[:, c * P:(c + 1) * P],
                        rhs=w1_sb.ap()[:, c * NH:(c + 1) * NH],
                        start=(c == 0), stop=(c == KC - 1))
                hsl = h_sb.ap()[:, t * NH:(t + 1) * NH]
                nc.vector.tensor_add(hsl, ps[:], b1_sb.ap())
                nc.vector.tensor_scalar_max(hsl, hsl, 0.0)
        epilogue_rows(tabA_l)
        allgather(tabA_l, tabA_f)
    with tile.TileContext(nc) as tc:
        expand(tabA_f, tabA_w)

    # ---------------- conv layer -------------------------------------------
    def conv(tab_local, tab_wide, beta_ap):
        """beta_ap: None for beta=1 (conv2), else [P,1] AP with beta value."""
        off = 0
        moff = 0
        with tile.TileContext(nc) as tc:
            with tc.tile_pool(name="cv", bufs=3) as cv, \
                 tc.tile_pool(name="cvs", bufs=2) as cvs:
                for t in range(NT):
                    kws = kab[t][:3]
                    K = sum(kws)
                    hs = cv.tile([P, K * ROW], bf16, tag="hs")
                    kacc = 0
                    for (kw, base) in zip(kws, WBASES):
                        dma_gather_raw(
                            nc,
                            out_ap=hs[:, kacc * ROW:(kacc + kw) * ROW]
                            .rearrange("p (c j) -> p c j", j=ROW),
                            in_ap=tab_wide[base:base + W, 0:ROW],
                            idxs_ap=idx_sb.ap()[:, off:off + 8 * kw],
                            num_idxs=P * kw, elem_size=ROW,
                            stride_elems=WROW)
                        off += 8 * kw
                        kacc += kw
                    assert kacc == K

                    hd = cvs.tile([P, ROW], bf16, tag="hd")
                    nc.sync.dma_start(
                        out=hd[:], in_=tab_local[t * P:(t + 1) * P, :]
                        .rearrange("(g p) j -> p (g j)", p=P))

                    hs3 = hs[:].rearrange("p (k j) -> p k j", k=K)
                    tmp = cv.tile([P, K * NH], bf16, tag="tmp")
                    tm3 = tmp[:].rearrange("p (k j) -> p k j", k=K)
                    nc.vector.tensor_mul(
                        tm3, hs3[:, :, 0:16],
                        hd[:, None, 0:16].to_broadcast([P, K, 16]))
                    alpha = cv.tile([P, K], f32, tag="alpha")
                    nc.vector.reduce_sum(
                        alpha[:],
                        tmp[:].rearrange("p (k j) -> p k j", j=NH), axis=AX)
                    nc.vector.tensor_mul(alpha[:], alpha[:], hs3[:, :, 16])
                    invd = cvs.tile([P, 1], f32, tag="invd")
                    if beta_ap is None:
                        nc.vector.tensor_copy(invd[:], hd[:, 16:17])
                    else:
                        nc.vector.tensor_mul(invd[:], hd[:, 16:17], beta_ap)
                    nc.vector.tensor_mul(
                        alpha[:], alpha[:],
                        invd[:].to_broadcast([P, K]))
                    # pad slots killed by the -1e30 static mask
                    nc.vector.tensor_add(
                        alpha[:], alpha[:], mask_sb.ap()[:, moff:moff + K])
                    moff += K
                    # |alpha| <= |beta| for real slots: exp without max-trick
                    e_bf = cv.tile([P, K], bf16, tag="e")
                    nc.scalar.activation(e_bf[:], alpha[:], AF.Exp)
                    s = cvs.tile([P, 1], f32, tag="s")
                    nc.vector.reduce_sum(
                        s[:], e_bf[:][:, None, :], axis=AX)
                    # analytic self-loop: cos(h,h) = (1 - 1e-12*invn)^2
                    selfa = cvs.tile([P, 1], f32, tag="selfa")
                    nc.vector.tensor_scalar(
                        selfa[:], hd[:, 16:17], -1.0e-12, 1.0,
                        op0=OP.mult, op1=OP.add)
                    nc.vector.tensor_mul(selfa[:], selfa[:], selfa[:])
                    if beta_ap is not None:
                        nc.vector.tensor_mul(selfa[:], selfa[:], beta_ap)
                    es = cvs.tile([P, 1], f32, tag="es")
                    nc.scalar.activation(es[:], selfa[:], AF.Exp)
                    nc.vector.tensor_add(s[:], s[:], es[:])
                    nc.vector.tensor_scalar_add(s[:], s[:], 1.0e-16)
                    r = cvs.tile([P, 1], f32, tag="r")
                    nc.vector.reciprocal(r[:], s[:])
                    coef = cv.tile([P, K], bf16, tag="coef")
                    nc.vector.tensor_mul(
                        coef[:], e_bf[:], r[:].to_broadcast([P, K]))
                    tmp2 = cv.tile([P, K * NH], bf16, tag="tmp2")
                    t23 = tmp2[:].rearrange("p (k j) -> p k j", k=K)
                    nc.vector.tensor_mul(
                        t23, hs3[:, :, 0:16],
                        coef[:][:, :, None].to_broadcast([P, K, 16]))
                    h2v = h_sb.ap()[:, t * NH:(t + 1) * NH]
                    nc.vector.reduce_sum(
                        h2v,
                        tmp2[:].rearrange("p (k j) -> p j k", k=K),
                        axis=AX)
                    # += (exp(self)/s) * h_d
                    rs = cvs.tile([P, 1], f32, tag="rs")
                    nc.vector.tensor_mul(rs[:], es[:], r[:])
                    sh = cvs.tile([P, NH], f32, tag="sh")
                    nc.vector.tensor_mul(
                        sh[:], hd[:, 0:16], rs[:].to_broadcast([P, NH]))
                    nc.vector.tensor_add(h2v, h2v, sh[:])
        assert off == idxw

    if "B" in phases:
        conv(tabA_l, tabA_w, None)

    # ---------------- phase C: table B + AG2 -------------------------------
    if "C" in phases:
        with tile.TileContext(nc) as tc:
            epilogue_rows(tabB_l)
            allgather(tabB_l, tabB_f)
        with tile.TileContext(nc) as tc:
            expand(tabB_f, tabB_w)

    if "D" in phases:
        conv(tabB_l, tabB_w, be_sb.ap()[:, 0:1])

    if "E" not in phases:
        # debug: dump h_sb (and inv_sb) into out
        with tile.TileContext(nc) as tc:
            ov = out_d.ap().rearrange("(p q) j -> p (q j)", p=P)
            nc.sync.dma_start(out=ov[:, 0:NT * NH], in_=h_sb.ap())
            nc.sync.dma_start(out=ov[:, NT * NH:NT * NH + NT], in_=inv_sb.ap())
        nc.compile()
        return nc

    with tile.TileContext(nc) as tc:
        with tc.tile_pool(name="hd", bufs=1) as hp, \
             tc.tile_pool(name="hdp", bufs=4, space="PSUM") as hpp:
            h3t = hp.tile([NH, NT * P], f32, tag="h3t")
            for t in range(NT):
                pst = hpp.tile([NH, P], f32, tag="pst")
                nc.tensor.transpose(
                    out=pst[:], in_=h_sb.ap()[:, t * NH:(t + 1) * NH],
                    identity=ident_f.ap())
                nc.vector.tensor_copy(h3t[:, t * P:(t + 1) * P], pst[:])
            lg = hp.tile([P, NT * NCO], f32, tag="lg")
            for t in range(NT):
                psl = hpp.tile([P, NCO], f32, tag="psl")
                nc.tensor.matmul(
                    out=psl[:], lhsT=h3t[:, t * P:(t + 1) * P], rhs=w4_sb.ap(),
                    start=True, stop=True)
                nc.vector.tensor_add(
                    lg[:, t * NCO:(t + 1) * NCO], psl[:], b4_sb.ap())
            lg3 = lg[:].rearrange("p (t j) -> p t j", t=NT)
            nm = hp.tile([P, NT], f32, tag="hnm")
            nc.vector.reduce_max(nm[:], lg3, axis=AX, negate=True)
            nc.vector.tensor_add(
                lg3, lg3, nm[:][:, :, None].to_broadcast([P, NT, NCO]))
            ex = hp.tile([P, NT * NCO], f32, tag="ex")
            nc.scalar.activation(ex[:], lg[:], AF.Exp)
            s = hp.tile([P, NT], f32, tag="hs_sum")
            nc.vector.reduce_sum(
                s[:], ex[:].rearrange("p (t j) -> p t j", t=NT), axis=AX)
            ls = hp.tile([P, NT], f32, tag="ls")
            nc.scalar.activation(ls[:], s[:], AF.Ln)
            nc.vector.tensor_sub(
                lg3, lg3, ls[:][:, :, None].to_broadcast([P, NT, NCO]))
            nc.sync.dma_start(
                out=out_d[:, :].rearrange("(t p) j -> p t j", p=P),
                in_=lg3)

    nc.compile()
    return nc


# --------------------------------------------------------------------------
# entry point
# --------------------------------------------------------------------------

def run(cfg: Cfg, inputs: dict, trace: bool = False):
    from concourse import bass_utils

    x = np.asarray(inputs["x"], np.float32)
    edge_index = np.asarray(inputs["edge_index"])
    W1 = np.asarray(inputs["W1"], np.float32)
    b1 = np.asarray(inputs["b1"], np.float32)
    W4 = np.asarray(inputs["W4"], np.float32)
    b4 = np.asarray(inputs["b4"], np.float32)
    beta3 = np.asarray(inputs["beta3"], np.float32)

    import ml_dtypes

    per_core, kab, order_c = preprocess(cfg, x, edge_index)
    nc = build_kernel(cfg, kab)

    P, KC, NH = cfg.P, cfg.kchunks, cfg.nh
    w1p = np.zeros((cfg.f_pad, NH), np.float32)
    w1p[:cfg.f_in] = W1
    w1p = np.ascontiguousarray(
        w1p.reshape(KC, P, NH).transpose(1, 0, 2).reshape(P, KC * NH)
    ).astype(ml_dtypes.bfloat16)
    b1r = np.ascontiguousarray(np.broadcast_to(b1[None, :], (P, NH)))
    b4r = np.ascontiguousarray(np.broadcast_to(b4[None, :], (P, cfg.nc_out)))
    ber = np.ascontiguousarray(np.broadcast_to(beta3[None, :], (P, 1)))

    in_maps = []
    for c in range(cfg.n_cores):
        in_maps.append({
            "xt": per_core[c]["xt"].astype(ml_dtypes.bfloat16),
            "idx16": per_core[c]["idx16"],
            "mask": per_core[c]["mask"].astype(ml_dtypes.bfloat16),
            "w1p": w1p, "b1r": b1r, "w4r": np.ascontiguousarray(W4),
            "b4r": b4r, "beta3r": ber,
        })

    res = bass_utils.run_bass_kernel_spmd(
        nc, in_maps, core_ids=list(range(cfg.n_cores)), trace=trace)

    out = np.empty((cfg.n_nodes, cfg.nc_out), np.float32)
    for c in range(cfg.n_cores):
        oc = np.asarray(res.results[c]["out"])[:cfg.npc_raw]
        out[c * cfg.npc_raw + order_c[c]] = oc
    return out, res


def kernel(**inputs) -> np.ndarray:
    out, _ = run(FULL, inputs, trace=False)
    return out


# revision 46
# speedup vs baseline: 1.3218x; 1.0488x over previous
"""AGNNet (2-layer AGNN conv + linear head) distributed over 8 trn2 NeuronCores.

Strategy (graph/data parallel, per sharding hint):
  - nodes sharded by dst range: core c owns nodes [c*6250, (c+1)*6250)
  - host groups edges by dst, degree-sorts each core's local nodes (undone on
    output), drops self-loops (handled analytically in the softmax), and packs
    each 128-node tile's in-edge lists into per-tile slot columns
  - device: L1 matmul from host-transposed x shard; build a bf16 row table
    [h(16), inv_norm, bias, pad] per node; AllGather the table; expand it to a
    256B-strided copy in DRAM (SWDGE regular pattern, CounterMachine rate);
    per node-tile ONE dma_gather (ant extended Q7 kernel, ~7.9ns/idx) per
    index window fetches all neighbor rows; DVE softmax (no max-trick needed:
    |alpha| <= |beta|); the self-loop term exp(beta*cos(h,h)) is added
    analytically; second conv identical; head matmul + log_softmax.
  - dma_gather indices are int16, so the 50178-row table is addressed through
    TWO overlapping 32768-row windows (bases 0 and 17410); each node's slots
    are split into window-A columns then window-B columns (host balances the
    split per tile); a pad row at wide-row 17410/17411 (bias = -1e30) backs
    padding slots so they vanish in the softmax.
"""

import math
from contextlib import ExitStack
from dataclasses import dataclass

import numpy as np


@dataclass
class Cfg:
    n_cores: int = 8
    n_nodes: int = 50000
    f_in: int = 2000
    nh: int = 16
    nc_out: int = 20
    row: int = 20           # table row elems: 16 h, [16]=invn, [17]=bias, 18-19 pad
    wrow: int = 128         # wide-table row elems (256B stride for dma_gather)
    P: int = 128
    win: int = 32768        # dma_gather int16 index window (rows per base)

    @property
    def npc_raw(self) -> int:       # real nodes per core
        return self.n_nodes // self.n_cores

    @property
    def npc(self) -> int:           # padded nodes per core (mult of 128)
        return ((self.npc_raw + self.P - 1) // self.P) * self.P

    @property
    def ntiles(self) -> int:
        return self.npc // self.P

    @property
    def kchunks(self) -> int:       # 128-row chunks of the padded f_in
        return (self.f_in + self.P - 1) // self.P

    @property
    def f_pad(self) -> int:
        return self.kchunks * self.P

    @property
    def nfull(self) -> int:
        return self.n_cores * self.npc

    @property
    def wb(self) -> int:            # last window base
        return self.nfull - self.win

    @property
    def wbases(self) -> tuple:      # 3 overlapping 32768-row window bases
        return (0, self.wb // 2, self.wb)


FULL = Cfg()
NEG_BIG = -1.0e30
G_DO_GATHER = True


def pack_idx16(idx_mat: np.ndarray) -> np.ndarray:
    """[128 partitions, nch chunks] window-relative indices -> dma_gather
    int16 index tile [128, nch*8]: list position c*128+j lands on partition j
    chunk c (the ucode lane swizzle applies to both the index read and the
    dst partition, so it cancels); the flat list is wrapped over 16
    partitions and replicated across the 8 16-partition groups."""
    P, nch = idx_mat.shape
    assert P == 128
    flat = np.ascontiguousarray(idx_mat.T).reshape(-1).astype(np.int16)
    tile = np.empty((128, nch * 8), np.int16)
    for p in range(16):
        tile[p] = flat[p::16]
    tile[16:] = np.tile(tile[:16], (7, 1))
    return np.ascontiguousarray(tile)


def dma_gather_raw(nc, out_ap, in_ap, idxs_ap, num_idxs: int,
                   elem_size: int, stride_elems: int = 0,
                   sbuf_tokens_per_rank: int = 0,
                   sbuf_free_dim_per_rank: int = 0):
    """nc.gpsimd.dma_gather minus the elem_size%256 assert (non-transpose
    path in the Q7 ucode has no such restriction; only the DRAM stride must
    be a multiple of 256B) and minus the SBUF-source transpose-only assert
    (gen_descs handles src_is_sbuf in the plain path too). For SBUF source:
    idx i reads partition i % tokens_per_rank, free-byte offset
    (i // tokens_per_rank) * free_dim_per_rank. single_packet=False: a
    single packet is capped at 64 descriptors, large gathers hang with it."""
    from concourse import mybir
    g = nc.gpsimd
    src_is_sbuf = sbuf_tokens_per_rank > 0
    if src_is_sbuf:
        stride_bytes_256 = 0
        if in_ap.dtype != out_ap.dtype:
            in_ap = in_ap.bitcast(out_ap.dtype)
        _in_ap = [g.lower_ap(in_ap)]
    else:
        stride_bytes = stride_elems * mybir.dt.size(in_ap.dtype)
        assert stride_bytes % 256 == 0 and stride_bytes // 256 < 256
        stride_bytes_256 = stride_bytes // 256
        _in_ap = g.lower_ap_dma(in_ap, for_custom_bir_dma=True)
    _idxs_ap = g.lower_ap(idxs_ap)
    _out_ap = g.lower_ap(out_ap)
    return g.add_instruction(
        mybir.InstDMAGatherAnt(
            name=g.bass.get_next_instruction_name(),
            ins=[*_in_ap, _idxs_ap, g.lower_val_access(g.to_reg(num_idxs))],
            outs=[_out_ap],
            transpose=False,
            num_idxs=num_idxs,
            elem_size=elem_size,
            stride_bytes_256=stride_bytes_256,
            gen_mode=0,
            single_packet=False,
            queue_num=0,
            sbuf_tokens_per_rank=sbuf_tokens_per_rank,
            sbuf_free_dim_per_rank=sbuf_free_dim_per_rank,
            sbuf_free_dim_pad_per_rank=0,
            sbuf_byte_offset=0,
        ))


# --------------------------------------------------------------------------
# host-side preprocessing
# --------------------------------------------------------------------------

def preprocess(cfg: Cfg, x: np.ndarray, edge_index: np.ndarray):
    """Shard + layout transform.

    Returns (per_core list of dicts, kab, order_c) where kab is the shared
    [(K1_t, K2_t, K3_t)] per tile (same across cores for SPMD): slot columns
    per index window (3 overlapping 32768-row windows of the wide table)."""
    P, NPC, NPCR = cfg.P, cfg.npc, cfg.npc_raw
    n, C, NT = cfg.n_nodes, cfg.n_cores, cfg.ntiles
    W = cfg.win
    B1, B2, B3 = cfg.wbases

    src = edge_index[0].astype(np.int64)
    dst = edge_index[1].astype(np.int64)
    keep = src != dst                     # self-loops handled analytically
    src, dst = src[keep], dst[keep]

    core_of_dst = dst // NPCR

    # pass 1: per-core degree sort -> rank of each node within its core
    order_c, rank_c = [], []
    for c in range(C):
        m = core_of_dst == c
        ld = dst[m] - c * NPCR
        deg = np.bincount(ld, minlength=NPCR)
        order = np.argsort(deg, kind="stable")      # ascending degree
        rank = np.empty(NPCR, np.int64)
        rank[order] = np.arange(NPCR)
        order_c.append(order)
        rank_c.append(rank)

    # new global id after per-core permutation + padding (= wide-table row)
    new_gid = np.empty(n, np.int64)
    for c in range(C):
        new_gid[c * NPCR:(c + 1) * NPCR] = c * NPC + rank_c[c]

    # per-core per-node edge lists sorted by wide-row
    per_core_lists = []
    for c in range(C):
        m = core_of_dst == c
        ld = dst[m] - c * NPCR
        gs = new_gid[src[m]]
        er = rank_c[c][ld]                       # dst rank of each edge
        eo = np.lexsort((gs, er))
        er_s = er[eo]
        gs_s = gs[eo]
        starts = np.zeros(NPC + 1, np.int64)
        np.cumsum(np.bincount(er_s, minlength=NPC), out=starts[1:])
        per_core_lists.append((starts, gs_s))

    # per-node class counts: s1 [0,B2) strictly-W1; f12 [B2,B3); f123
    # [B3, W) (in all); f23 [W, B2+W); s3 [B2+W, nfull)
    bnds = np.array([B2, B3, W, B2 + W], np.int64)
    cnts = np.zeros((C, 5, NPC), np.int64)
    degp_all = np.zeros((C, NPC), np.int64)
    for c in range(C):
        starts, gs_s = per_core_lists[c]
        d_cnt = np.diff(starts)
        degp_all[c] = d_cnt
        er_of_edge = np.repeat(np.arange(NPC), d_cnt)
        cls = np.searchsorted(bnds, gs_s, side="right")
        for k in range(5):
            cnts[c, k] = np.bincount(
                er_of_edge, weights=(cls == k), minlength=NPC)

    def split_node(T, s1, f12, f123, f23, s3, deg):
        """-> (n1, n3) target-T balanced; n2 = deg - n1 - n3 (all W2-ok)."""
        n1 = np.maximum(s1, np.minimum(T, s1 + f12 + f123))
        f123_left = f123 - np.maximum(0, n1 - (s1 + f12))
        n3 = np.maximum(s3, np.minimum(T, s3 + f23 + f123_left))
        return n1, n3

    kab = []
    for t in range(NT):
        s, e = t * P, (t + 1) * P
        degs = degp_all[:, s:e]
        s1, f12, f123, f23, s3 = (cnts[:, k, s:e] for k in range(5))
        kmax = int(degs.max())
        best = None
        for T in range(kmax + 1):
            n1, n3 = split_node(T, s1, f12, f123, f23, s3, degs)
            K1 = int(n1.max())
            K3 = int(n3.max())
            K2 = int((degs - n1 - n3).max())
            if best is None or K1 + K2 + K3 < best[0]:
                best = (K1 + K2 + K3, T, K1, K2, K3)
        _, T, K1, K2, K3 = best
        kab.append((max(K1, 1), max(K2, 1), max(K3, 1), T))

    # build per-core idx16 + per-slot mask inputs (pad slots point at the
    # window base row -- a real row -- and are killed by a -1e30 mask)
    per_core = []
    for c in range(C):
        starts, gs_s = per_core_lists[c]
        packs = []
        masks = []
        for t in range(NT):
            K1, K2, K3, T = kab[t]
            idx1 = np.zeros((P, K1), np.int64)
            idx2 = np.zeros((P, K2), np.int64)
            idx3 = np.zeros((P, K3), np.int64)
            mask = np.full((P, K1 + K2 + K3), NEG_BIG, np.float32)
            sl = slice(t * P, (t + 1) * P)
            s1, f12, f123, f23, s3 = (cnts[c, k, sl] for k in range(5))
            degs = degp_all[c, sl]
            n1, n3 = split_node(T, s1, f12, f123, f23, s3, degs)
            n1 = np.minimum(n1, K1)
            n3 = np.minimum(n3, K3)
            for p in range(P):
                node = t * P + p
                lst = gs_s[starts[node]:starts[node + 1]]  # sorted by wrow
                a = int(n1[p]); b = int(n3[p])
                w1 = lst[:a]                       # s1 + leading f12/f123
                rest = lst[a:]
                w3 = rest[rest.size - b:] if b else rest[:0]
                w2 = rest[:rest.size - b]
                idx1[p, :a] = w1 - B1
                idx2[p, :w2.size] = w2 - B2
                idx3[p, :b] = w3 - B3
                mask[p, :a] = 0.0
                mask[p, K1:K1 + w2.size] = 0.0
                mask[p, K1 + K2:K1 + K2 + b] = 0.0
            for arr in (idx1, idx2, idx3):
                assert arr.min() >= 0 and arr.max() < W
            packs += [pack_idx16(idx1), pack_idx16(idx2), pack_idx16(idx3)]
            masks.append(mask)
        idx16 = np.concatenate(packs, axis=1)
        maskv = np.concatenate(masks, axis=1)

        # x shard: permuted, padded, transposed, f-padded
        xs = x[c * NPCR:(c + 1) * NPCR][order_c[c]]      # [NPCR, f_in]
        xt = np.zeros((cfg.f_pad, NPC), np.float32)
        xt[:cfg.f_in, :NPCR] = xs.T
        per_core.append({"xt": np.ascontiguousarray(xt), "idx16": idx16,
                         "mask": maskv})

    return per_core, kab, order_c


# --------------------------------------------------------------------------
# device kernel builder
# --------------------------------------------------------------------------

def build_kernel(cfg: Cfg, kab, phases: str = "ABCDE"):
    import concourse.bacc as bacc
    import concourse.tile as tile
    from concourse import bass, mybir
    from concourse.masks import make_identity

    P = cfg.P
    NH, NCO, ROW, WROW = cfg.nh, cfg.nc_out, cfg.row, cfg.wrow
    NPC, NT, KC = cfg.npc, cfg.ntiles, cfg.kchunks
    NFULL, WB, W = cfg.nfull, cfg.wb, cfg.win
    f32 = mybir.dt.float32
    bf16 = mybir.dt.bfloat16
    i16 = mybir.dt.int16
    AX = mybir.AxisListType.X
    OP = mybir.AluOpType
    AF = mybir.ActivationFunctionType
    idxw = sum(8 * (k1 + k2 + k3) for (k1, k2, k3, _) in kab)
    sumk = idxw // 8
    WBASES = cfg.wbases
    RPP = NFULL // P                # wide-table rows per partition (bounce)

    nc = bacc.Bacc("TRN2", target_bir_lowering=False, debug=False,
                   num_devices=cfg.n_cores)

    xt_d = nc.dram_tensor("xt", [cfg.f_pad, NPC], bf16, kind="ExternalInput")
    idx_d = nc.dram_tensor("idx16", [P, idxw], i16, kind="ExternalInput")
    mask_d = nc.dram_tensor("mask", [P, sumk], bf16, kind="ExternalInput")
    w1_d = nc.dram_tensor("w1p", [P, KC * NH], bf16, kind="ExternalInput")
    b1_d = nc.dram_tensor("b1r", [P, NH], f32, kind="ExternalInput")
    w4_d = nc.dram_tensor("w4r", [NH, NCO], f32, kind="ExternalInput")
    b4_d = nc.dram_tensor("b4r", [P, NCO], f32, kind="ExternalInput")
    be_d = nc.dram_tensor("beta3r", [P, 1], f32, kind="ExternalInput")
    out_d = nc.dram_tensor("out", [NPC, NCO], f32, kind="ExternalOutput")

    tabA_l = nc.dram_tensor("tabA_l", [NPC, ROW], bf16)
    tabA_f = nc.dram_tensor("tabA_f", [NFULL, ROW], bf16, addr_space="Shared")
    tabB_l = nc.dram_tensor("tabB_l", [NPC, ROW], bf16)
    tabB_f = nc.dram_tensor("tabB_f", [NFULL, ROW], bf16, addr_space="Shared")
    tabA_w = nc.dram_tensor("tabA_w", [NFULL, WROW], bf16)  # G-phase only

    # persistent SBUF (survives across TileContexts)
    def sb(name, shape, dtype):
        return nc.alloc_sbuf_tensor(name, list(shape), dtype)

    ident_f = sb("identf", [P, P], f32)
    w1_sb = sb("w1sb", [P, KC * NH], bf16)
    b1_sb = sb("b1sb", [P, NH], f32)
    w4_sb = sb("w4sb", [NH, NCO], f32)
    b4_sb = sb("b4sb", [P, NCO], f32)
    be_sb = sb("besb", [P, 1], f32)
    bee_sb = sb("beesb", [P, 1], f32)      # exp-ready: beta3 value
    h_sb = sb("hsb", [P, NT * NH], f32)
    rows_sb = sb("rowssb", [P, NT * ROW], bf16)
    sq_sb = sb("sqsb", [P, NT * NH], f32)
    ss_sb = sb("sssb", [P, NT], f32)
    inv_sb = sb("invsb", [P, NT], f32)
    idx_sb = sb("idxsb", [P, idxw], i16)    # all tiles' gather indices
    mask_sb = sb("masksb", [P, sumk], bf16)  # per-slot pad masks
    # SBUF-resident gather table: row r at partition r%128, stripe r//128
    table_sb = sb("tablesb", [P, RPP * ROW], bf16)

    def epilogue_rows(tab_local):
        """h_sb -> inv norm -> rows_sb -> DMA to tab_local."""
        h3v = h_sb.ap().rearrange("p (t j) -> p t j", t=NT)
        nc.vector.tensor_mul(sq_sb.ap(), h_sb.ap(), h_sb.ap())
        nc.vector.reduce_sum(
            ss_sb.ap(), sq_sb.ap().rearrange("p (t j) -> p t j", t=NT), axis=AX)
        nc.scalar.sqrt(ss_sb.ap(), ss_sb.ap())
        nc.vector.tensor_scalar_add(ss_sb.ap(), ss_sb.ap(), 1.0e-12)
        nc.vector.reciprocal(inv_sb.ap(), ss_sb.ap())
        rv = rows_sb.ap().rearrange("p (t j) -> p t j", t=NT)
        nc.vector.tensor_copy(rv[:, :, 0:16], h3v)
        nc.vector.tensor_copy(rv[:, :, 16], inv_sb.ap())
        nc.sync.dma_start(
            out=tab_local[:, :].rearrange("(t p) j -> p t j", p=P),
            in_=rv)

    def allgather(tab_local, tab_full):
        nc.gpsimd.collective_compute(
            "AllGather", OP.bypass,
            replica_groups=[list(range(cfg.n_cores))],
            ins=[tab_local.ap().opt()],
            outs=[tab_full.ap()[0:NFULL, :].opt()])

    def load_table(tab_full):
        """packed DRAM [NFULL, 20] -> SBUF table (row r at partition r%128,
        stripe r//128). SWDGE regular-pattern DMAs (~0.34ns/desc), chunked
        under the 16384-descriptor carveout limit."""
        tv = table_sb.ap().rearrange("p (r j) -> p r j", j=ROW)
        src = tab_full[:, :].rearrange("(r p) j -> p r j", p=P)
        CH = 96                      # stripes per chunk (<=16k descs)
        for o in range(0, RPP, CH):
            n = min(CH, RPP - o)
            nc.gpsimd.dma_start(out=tv[:, o:o + n, :], in_=src[:, o:o + n, :])

    # ---------------- phase A: consts, idx preload, L1, table A, AG1 -------
    with tile.TileContext(nc) as tc:
        make_identity(nc, ident_f.ap())
        nc.gpsimd.memset(rows_sb.ap(), 0.0)
        nc.sync.dma_start(out=idx_sb.ap(), in_=idx_d[:, :])
        nc.sync.dma_start(out=mask_sb.ap(), in_=mask_d[:, :])
        nc.sync.dma_start(out=w1_sb.ap(), in_=w1_d[:, :])
        nc.sync.dma_start(out=b1_sb.ap(), in_=b1_d[:, :])
        nc.sync.dma_start(out=w4_sb.ap(), in_=w4_d[:, :])
        nc.sync.dma_start(out=b4_sb.ap(), in_=b4_d[:, :])
        nc.sync.dma_start(out=be_sb.ap(), in_=be_d[:, :])

        with tc.tile_pool(name="l1x", bufs=3) as xp, \
             tc.tile_pool(name="l1p", bufs=4, space="PSUM") as pp:
            for t in range(NT):
                xw = xp.tile([P, KC * P], bf16, tag="xw")
                src = xt_d[:, :].rearrange("(c p) m -> p c m", p=P)[:, :, t * P:(t + 1) * P]
                nc.sync.dma_start(
                    out=xw[:].rearrange("p (c j) -> p c j", c=KC), in_=src)
                ps = pp.tile([P, NH], f32, tag="l1ps")
                for c in range(KC):
                    nc.tensor.matmul(
                        out=ps[:], lhsT=xw[:, c * P:(c + 1) * P],
                        rhs=w1_sb.ap()[:, c * NH:(c + 1) * NH],
                        start=(c == 0), stop=(c == KC - 1))
                hsl = h_sb.ap()[:, t * NH:(t + 1) * NH]
                nc.vector.tensor_add(hsl, ps[:], b1_sb.ap())
                nc.vector.tensor_scalar_max(hsl, hsl, 0.0)
        epilogue_rows(tabA_l)
        allgather(tabA_l, tabA_f)
    with tile.TileContext(nc) as tc:
        load_table(tabA_f)

    # ---------------- conv layer -------------------------------------------
    def conv(tab_local, beta_ap):
        """beta_ap: None for beta=1 (conv2), else [P,1] AP with beta value."""
        off = 0
        moff = 0
        with tile.TileContext(nc) as tc:
            with tc.tile_pool(name="cv", bufs=3) as cv, \
                 tc.tile_pool(name="cvs", bufs=2) as cvs:
                for t in range(NT):
                    kws = kab[t][:3]
                    K = sum(kws)
                    hs = cv.tile([P, K * ROW], bf16, tag="hs")
                    kacc = 0
                    for (kw, base) in zip(kws, WBASES):
                        s0 = (base // P) * ROW
                        dma_gather_raw(
                            nc,
                            out_ap=hs[:, kacc * ROW:(kacc + kw) * ROW]
                            .rearrange("p (c j) -> p c j", j=ROW),
                            in_ap=table_sb.ap()[:, s0:s0 + 256 * ROW],
                            idxs_ap=idx_sb.ap()[:, off:off + 8 * kw],
                            num_idxs=P * kw, elem_size=ROW,
                            sbuf_tokens_per_rank=P,
                            sbuf_free_dim_per_rank=ROW * 2)
                        off += 8 * kw
                        kacc += kw
                    assert kacc == K

                    hd = cvs.tile([P, ROW], bf16, tag="hd")
                    nc.sync.dma_start(
                        out=hd[:], in_=tab_local[t * P:(t + 1) * P, :]
                        .rearrange("(g p) j -> p (g j)", p=P))

                    hs3 = hs[:].rearrange("p (k j) -> p k j", k=K)
                    tmp = cv.tile([P, K * NH], bf16, tag="tmp")
                    tm3 = tmp[:].rearrange("p (k j) -> p k j", k=K)
                    nc.vector.tensor_mul(
                        tm3, hs3[:, :, 0:16],
                        hd[:, None, 0:16].to_broadcast([P, K, 16]))
                    alpha = cv.tile([P, K], f32, tag="alpha")
                    nc.vector.reduce_sum(
                        alpha[:],
                        tmp[:].rearrange("p (k j) -> p k j", j=NH), axis=AX)
                    nc.vector.tensor_mul(alpha[:], alpha[:], hs3[:, :, 16])
                    invd = cvs.tile([P, 1], f32, tag="invd")
                    if beta_ap is None:
                        nc.vector.tensor_copy(invd[:], hd[:, 16:17])
                    else:
                        nc.vector.tensor_mul(invd[:], hd[:, 16:17], beta_ap)
                    nc.vector.tensor_mul(
                        alpha[:], alpha[:],
                        invd[:].to_broadcast([P, K]))
                    # pad slots killed by the -1e30 static mask
                    nc.vector.tensor_add(
                        alpha[:], alpha[:], mask_sb.ap()[:, moff:moff + K])
                    moff += K
                    # |alpha| <= |beta| for real slots: exp without max-trick
                    e_bf = cv.tile([P, K], bf16, tag="e")
                    nc.scalar.activation(e_bf[:], alpha[:], AF.Exp)
                    s = cvs.tile([P, 1], f32, tag="s")
                    nc.vector.reduce_sum(
                        s[:], e_bf[:][:, None, :], axis=AX)
                    # analytic self-loop: cos(h,h) = (1 - 1e-12*invn)^2
                    selfa = cvs.tile([P, 1], f32, tag="selfa")
                    nc.vector.tensor_scalar(
                        selfa[:], hd[:, 16:17], -1.0e-12, 1.0,
                        op0=OP.mult, op1=OP.add)
                    nc.vector.tensor_mul(selfa[:], selfa[:], selfa[:])
                    if beta_ap is not None:
                        nc.vector.tensor_mul(selfa[:], selfa[:], beta_ap)
                    es = cvs.tile([P, 1], f32, tag="es")
                    nc.scalar.activation(es[:], selfa[:], AF.Exp)
                    nc.vector.tensor_add(s[:], s[:], es[:])
                    nc.vector.tensor_scalar_add(s[:], s[:], 1.0e-16)
                    r = cvs.tile([P, 1], f32, tag="r")
                    nc.vector.reciprocal(r[:], s[:])
                    coef = cv.tile([P, K], bf16, tag="coef")
                    nc.vector.tensor_mul(
                        coef[:], e_bf[:], r[:].to_broadcast([P, K]))
                    tmp2 = cv.tile([P, K * NH], bf16, tag="tmp2")
                    t23 = tmp2[:].rearrange("p (k j) -> p k j", k=K)
                    nc.vector.tensor_mul(
                        t23, hs3[:, :, 0:16],
                        coef[:][:, :, None].to_broadcast([P, K, 16]))
                    h2v = h_sb.ap()[:, t * NH:(t + 1) * NH]
                    nc.vector.reduce_sum(
                        h2v,
                        tmp2[:].rearrange("p (k j) -> p j k", k=K),
                        axis=AX)
                    # += (exp(self)/s) * h_d
                    rs = cvs.tile([P, 1], f32, tag="rs")
                    nc.vector.tensor_mul(rs[:], es[:], r[:])
                    sh = cvs.tile([P, NH], f32, tag="sh")
                    nc.vector.tensor_mul(
                        sh[:], hd[:, 0:16], rs[:].to_broadcast([P, NH]))
                    nc.vector.tensor_add(h2v, h2v, sh[:])
        assert off == idxw

    if "G" in phases:
        # debug: dma_gather NCH chunks of table rows via the ant gather and
        # dump raw rows (as f32). G_MODE: dram (256B-strided) | sbuf
        import os
        NCH = int(os.environ.get("G_NCH", "49"))
        G_NINST = int(os.environ.get("G_NINST", "1"))
        G_MODE = os.environ.get("G_MODE", "dram")
        nidx = NCH * P
        widx = nidx // 16
        idx16_d = nc.dram_tensor("idx16g", [P, widx], mybir.dt.int16,
                                 kind="ExternalInput")
        with tile.TileContext(nc) as tc:
            if G_MODE == "dram":
                nc.sync.dma_start(out=tabA_w[:, 0:ROW], in_=tabA_f[:, :])
            else:
                # sbuf layout: row r -> partition r%128, stripe r//128
                load_table(tabA_f)
            with tc.tile_pool(name="dbg", bufs=1) as dbg:
                idx_sbg = dbg.tile([P, widx], mybir.dt.int16, tag="idx16")
                nc.sync.dma_start(out=idx_sbg[:], in_=idx16_d[:, :])
                hs = dbg.tile([P, NCH * ROW], bf16, tag="hs")
                nc.gpsimd.memset(hs[:], 0.0)
                if G_DO_GATHER:
                    for _ in range(G_NINST):
                        if G_MODE == "dram":
                            dma_gather_raw(
                                nc,
                                out_ap=hs[:].rearrange(
                                    "p (c j) -> p c j", j=ROW),
                                in_ap=tabA_w[:, 0:ROW], idxs_ap=idx_sbg[:],
                                num_idxs=nidx, elem_size=ROW,
                                stride_elems=128)
                        else:
                            dma_gather_raw(
                                nc,
                                out_ap=hs[:].rearrange(
                                    "p (c j) -> p c j", j=ROW),
                                in_ap=table_sb.ap()[:, 0:RPP * ROW],
                                idxs_ap=idx_sbg[:],
                                num_idxs=nidx, elem_size=ROW,
                                sbuf_tokens_per_rank=P,
                                sbuf_free_dim_per_rank=ROW * 2)
                ncols = min(NCH * ROW, 980)
                hf = dbg.tile([P, ncols], f32, tag="hf")
                nc.vector.tensor_copy(hf[:], hs[:, 0:ncols])
                ov = out_d.ap().rearrange("(p q) j -> p (q j)", p=P)
                nc.sync.dma_start(out=ov[:, 0:ncols], in_=hf[:])
        nc.compile()
        return nc

    if "B" in phases:
        conv(tabA_l, None)

    # ---------------- phase C: table B + AG2 -------------------------------
    if "C" in phases:
        with tile.TileContext(nc) as tc:
            epilogue_rows(tabB_l)
            allgather(tabB_l, tabB_f)
        with tile.TileContext(nc) as tc:
            load_table(tabB_f)

    if "D" in phases:
        conv(tabB_l, be_sb.ap()[:, 0:1])

    if "E" not in phases:
        # debug: dump h_sb (and inv_sb) into out
        with tile.TileContext(nc) as tc:
            ov = out_d.ap().rearrange("(p q) j -> p (q j)", p=P)
            nc.sync.dma_start(out=ov[:, 0:NT * NH], in_=h_sb.ap())
            nc.sync.dma_start(out=ov[:, NT * NH:NT * NH + NT], in_=inv_sb.ap())
        nc.compile()
        return nc

    with tile.TileContext(nc) as tc:
        with tc.tile_pool(name="hd", bufs=1) as hp, \
             tc.tile_pool(name="hdp", bufs=4, space="PSUM") as hpp:
            h3t = hp.tile([NH, NT * P], f32, tag="h3t")
            for t in range(NT):
                pst = hpp.tile([NH, P], f32, tag="pst")
                nc.tensor.transpose(
                    out=pst[:], in_=h_sb.ap()[:, t * NH:(t + 1) * NH],
                    identity=ident_f.ap())
                nc.vector.tensor_copy(h3t[:, t * P:(t + 1) * P], pst[:])
            lg = hp.tile([P, NT * NCO], f32, tag="lg")
            for t in range(NT):
                psl = hpp.tile([P, NCO], f32, tag="psl")
                nc.tensor.matmul(
                    out=psl[:], lhsT=h3t[:, t * P:(t + 1) * P], rhs=w4_sb.ap(),
                    start=True, stop=True)
                nc.vector.tensor_add(
                    lg[:, t * NCO:(t + 1) * NCO], psl[:], b4_sb.ap())
            lg3 = lg[:].rearrange("p (t j) -> p t j", t=NT)
            nm = hp.tile([P, NT], f32, tag="hnm")
            nc.vector.reduce_max(nm[:], lg3, axis=AX, negate=True)
            nc.vector.tensor_add(
                lg3, lg3, nm[:][:, :, None].to_broadcast([P, NT, NCO]))
            ex = hp.tile([P, NT * NCO], f32, tag="ex")
            nc.scalar.activation(ex[:], lg[:], AF.Exp)
            s = hp.tile([P, NT], f32, tag="hs_sum")
            nc.vector.reduce_sum(
                s[:], ex[:].rearrange("p (t j) -> p t j", t=NT), axis=AX)
            ls = hp.tile([P, NT], f32, tag="ls")
            nc.scalar.activation(ls[:], s[:], AF.Ln)
            nc.vector.tensor_sub(
                lg3, lg3, ls[:][:, :, None].to_broadcast([P, NT, NCO]))
            nc.sync.dma_start(
                out=out_d[:, :].rearrange("(t p) j -> p t j", p=P),
                in_=lg3)

    nc.compile()
    return nc


# --------------------------------------------------------------------------
# entry point
# --------------------------------------------------------------------------

def run(cfg: Cfg, inputs: dict, trace: bool = False):
    from concourse import bass_utils

    x = np.asarray(inputs["x"], np.float32)
    edge_index = np.asarray(inputs["edge_index"])
    W1 = np.asarray(inputs["W1"], np.float32)
    b1 = np.asarray(inputs["b1"], np.float32)
    W4 = np.asarray(inputs["W4"], np.float32)
    b4 = np.asarray(inputs["b4"], np.float32)
    beta3 = np.asarray(inputs["beta3"], np.float32)

    import ml_dtypes

    per_core, kab, order_c = preprocess(cfg, x, edge_index)
    nc = build_kernel(cfg, kab)

    P, KC, NH = cfg.P, cfg.kchunks, cfg.nh
    w1p = np.zeros((cfg.f_pad, NH), np.float32)
    w1p[:cfg.f_in] = W1
    w1p = np.ascontiguousarray(
        w1p.reshape(KC, P, NH).transpose(1, 0, 2).reshape(P, KC * NH)
    ).astype(ml_dtypes.bfloat16)
    b1r = np.ascontiguousarray(np.broadcast_to(b1[None, :], (P, NH)))
    b4r = np.ascontiguousarray(np.broadcast_to(b4[None, :], (P, cfg.nc_out)))
    ber = np.ascontiguousarray(np.broadcast_to(beta3[None, :], (P, 1)))

    in_maps = []
    for c in range(cfg.n_cores):
        in_maps.append({
            "xt": per_core[c]["xt"].astype(ml_dtypes.bfloat16),
            "idx16": per_core[c]["idx16"],
            "mask": per_core[c]["mask"].astype(ml_dtypes.bfloat16),
            "w1p": w1p, "b1r": b1r, "w4r": np.ascontiguousarray(W4),
            "b4r": b4r, "beta3r": ber,
        })

    res = bass_utils.run_bass_kernel_spmd(
        nc, in_maps, core_ids=list(range(cfg.n_cores)), trace=trace)

    out = np.empty((cfg.n_nodes, cfg.nc_out), np.float32)
    for c in range(cfg.n_cores):
        oc = np.asarray(res.results[c]["out"])[:cfg.npc_raw]
        out[c * cfg.npc_raw + order_c[c]] = oc
    return out, res


def kernel(**inputs) -> np.ndarray:
    out, _ = run(FULL, inputs, trace=False)
    return out


# revision 54
# speedup vs baseline: 1.3455x; 1.0180x over previous
"""AGNNet (2-layer AGNN conv + linear head) distributed over 8 trn2 NeuronCores.

Strategy (graph/data parallel, per sharding hint):
  - nodes sharded by dst range: core c owns nodes [c*6250, (c+1)*6250)
  - host groups edges by dst, degree-sorts each core's local nodes (undone on
    output), drops self-loops (handled analytically in the softmax), and packs
    each 128-node tile's in-edge lists into per-tile slot columns
  - device: L1 matmul from host-transposed x shard; build a bf16 row table
    [h(16), inv_norm, bias, pad] per node; AllGather the table; expand it to a
    256B-strided copy in DRAM (SWDGE regular pattern, CounterMachine rate);
    per node-tile ONE dma_gather (ant extended Q7 kernel, ~7.9ns/idx) per
    index window fetches all neighbor rows; DVE softmax (no max-trick needed:
    |alpha| <= |beta|); the self-loop term exp(beta*cos(h,h)) is added
    analytically; second conv identical; head matmul + log_softmax.
  - dma_gather indices are int16, so the 50178-row table is addressed through
    TWO overlapping 32768-row windows (bases 0 and 17410); each node's slots
    are split into window-A columns then window-B columns (host balances the
    split per tile); a pad row at wide-row 17410/17411 (bias = -1e30) backs
    padding slots so they vanish in the softmax.
"""

import math
from contextlib import ExitStack
from dataclasses import dataclass

import numpy as np


@dataclass
class Cfg:
    n_cores: int = 8
    n_nodes: int = 50000
    f_in: int = 2000
    nh: int = 16
    nc_out: int = 20
    row: int = 20           # table row elems: 16 h, [16]=invn, [17]=bias, 18-19 pad
    wrow: int = 128         # wide-table row elems (256B stride for dma_gather)
    P: int = 128
    win: int = 32768        # dma_gather int16 index window (rows per base)

    @property
    def npc_raw(self) -> int:       # real nodes per core
        return self.n_nodes // self.n_cores

    @property
    def npc(self) -> int:           # padded nodes per core (mult of 128)
        return ((self.npc_raw + self.P - 1) // self.P) * self.P

    @property
    def ntiles(self) -> int:
        return self.npc // self.P

    @property
    def kchunks(self) -> int:       # 128-row chunks of the padded f_in
        return (self.f_in + self.P - 1) // self.P

    @property
    def f_pad(self) -> int:
        return self.kchunks * self.P

    @property
    def nfull(self) -> int:
        return self.n_cores * self.npc

    @property
    def wb(self) -> int:            # last window base
        return self.nfull - self.win

    @property
    def wbases(self) -> tuple:      # 3 overlapping 32768-row window bases
        return (0, self.wb // 2, self.wb)


FULL = Cfg()
NEG_BIG = -1.0e30
G_DO_GATHER = True


def pack_idx16(idx_mat: np.ndarray) -> np.ndarray:
    """[128 partitions, nch chunks] window-relative indices -> dma_gather
    int16 index tile [128, nch*8]: list position c*128+j lands on partition j
    chunk c (the ucode lane swizzle applies to both the index read and the
    dst partition, so it cancels); the flat list is wrapped over 16
    partitions and replicated across the 8 16-partition groups."""
    P, nch = idx_mat.shape
    assert P == 128
    flat = np.ascontiguousarray(idx_mat.T).reshape(-1).astype(np.int16)
    tile = np.empty((128, nch * 8), np.int16)
    for p in range(16):
        tile[p] = flat[p::16]
    tile[16:] = np.tile(tile[:16], (7, 1))
    return np.ascontiguousarray(tile)


def dma_gather_raw(nc, out_ap, in_ap, idxs_ap, num_idxs: int,
                   elem_size: int, stride_elems: int = 0,
                   sbuf_tokens_per_rank: int = 0,
                   sbuf_free_dim_per_rank: int = 0):
    """nc.gpsimd.dma_gather minus the elem_size%256 assert (non-transpose
    path in the Q7 ucode has no such restriction; only the DRAM stride must
    be a multiple of 256B) and minus the SBUF-source transpose-only assert
    (gen_descs handles src_is_sbuf in the plain path too). For SBUF source:
    idx i reads partition i % tokens_per_rank, free-byte offset
    (i // tokens_per_rank) * free_dim_per_rank. single_packet=False: a
    single packet is capped at 64 descriptors, large gathers hang with it."""
    from concourse import mybir
    g = nc.gpsimd
    src_is_sbuf = sbuf_tokens_per_rank > 0
    if src_is_sbuf:
        stride_bytes_256 = 0
        if in_ap.dtype != out_ap.dtype:
            in_ap = in_ap.bitcast(out_ap.dtype)
        _in_ap = [g.lower_ap(in_ap)]
    else:
        stride_bytes = stride_elems * mybir.dt.size(in_ap.dtype)
        assert stride_bytes % 256 == 0 and stride_bytes // 256 < 256
        stride_bytes_256 = stride_bytes // 256
        _in_ap = g.lower_ap_dma(in_ap, for_custom_bir_dma=True)
    _idxs_ap = g.lower_ap(idxs_ap)
    _out_ap = g.lower_ap(out_ap)
    return g.add_instruction(
        mybir.InstDMAGatherAnt(
            name=g.bass.get_next_instruction_name(),
            ins=[*_in_ap, _idxs_ap, g.lower_val_access(g.to_reg(num_idxs))],
            outs=[_out_ap],
            transpose=False,
            num_idxs=num_idxs,
            elem_size=elem_size,
            stride_bytes_256=stride_bytes_256,
            gen_mode=0,
            single_packet=False,
            queue_num=0,
            sbuf_tokens_per_rank=sbuf_tokens_per_rank,
            sbuf_free_dim_per_rank=sbuf_free_dim_per_rank,
            sbuf_free_dim_pad_per_rank=0,
            sbuf_byte_offset=0,
        ))


# --------------------------------------------------------------------------
# host-side preprocessing
# --------------------------------------------------------------------------

def preprocess(cfg: Cfg, x: np.ndarray, edge_index: np.ndarray):
    """Shard + layout transform.

    Returns (per_core list of dicts, kab, order_c) where kab is the shared
    [(K1_t, K2_t, K3_t)] per tile (same across cores for SPMD): slot columns
    per index window (3 overlapping 32768-row windows of the wide table)."""
    P, NPC, NPCR = cfg.P, cfg.npc, cfg.npc_raw
    n, C, NT = cfg.n_nodes, cfg.n_cores, cfg.ntiles
    W = cfg.win
    B1, B2, B3 = cfg.wbases

    src = edge_index[0].astype(np.int64)
    dst = edge_index[1].astype(np.int64)
    keep = src != dst                     # self-loops handled analytically
    src, dst = src[keep], dst[keep]

    core_of_dst = dst // NPCR

    # pass 1: per-core degree sort -> rank of each node within its core
    order_c, rank_c = [], []
    for c in range(C):
        m = core_of_dst == c
        ld = dst[m] - c * NPCR
        deg = np.bincount(ld, minlength=NPCR)
        order = np.argsort(deg, kind="stable")      # ascending degree
        rank = np.empty(NPCR, np.int64)
        rank[order] = np.arange(NPCR)
        order_c.append(order)
        rank_c.append(rank)

    # new global id after per-core permutation + padding (= wide-table row)
    new_gid = np.empty(n, np.int64)
    for c in range(C):
        new_gid[c * NPCR:(c + 1) * NPCR] = c * NPC + rank_c[c]

    # per-core per-node edge lists sorted by wide-row
    per_core_lists = []
    for c in range(C):
        m = core_of_dst == c
        ld = dst[m] - c * NPCR
        gs = new_gid[src[m]]
        er = rank_c[c][ld]                       # dst rank of each edge
        eo = np.lexsort((gs, er))
        er_s = er[eo]
        gs_s = gs[eo]
        starts = np.zeros(NPC + 1, np.int64)
        np.cumsum(np.bincount(er_s, minlength=NPC), out=starts[1:])
        per_core_lists.append((starts, gs_s))

    # per-node class counts: s1 [0,B2) strictly-W1; f12 [B2,B3); f123
    # [B3, W) (in all); f23 [W, B2+W); s3 [B2+W, nfull)
    bnds = np.array([B2, B3, W, B2 + W], np.int64)
    cnts = np.zeros((C, 5, NPC), np.int64)
    degp_all = np.zeros((C, NPC), np.int64)
    for c in range(C):
        starts, gs_s = per_core_lists[c]
        d_cnt = np.diff(starts)
        degp_all[c] = d_cnt
        er_of_edge = np.repeat(np.arange(NPC), d_cnt)
        cls = np.searchsorted(bnds, gs_s, side="right")
        for k in range(5):
            cnts[c, k] = np.bincount(
                er_of_edge, weights=(cls == k), minlength=NPC)

    def split_node(T, s1, f12, f123, f23, s3, deg):
        """-> (n1, n3) target-T balanced; n2 = deg - n1 - n3 (all W2-ok)."""
        n1 = np.maximum(s1, np.minimum(T, s1 + f12 + f123))
        f123_left = f123 - np.maximum(0, n1 - (s1 + f12))
        n3 = np.maximum(s3, np.minimum(T, s3 + f23 + f123_left))
        return n1, n3

    kab = []
    for t in range(NT):
        s, e = t * P, (t + 1) * P
        degs = degp_all[:, s:e]
        s1, f12, f123, f23, s3 = (cnts[:, k, s:e] for k in range(5))
        kmax = int(degs.max())
        best = None
        for T in range(kmax + 1):
            n1, n3 = split_node(T, s1, f12, f123, f23, s3, degs)
            K1 = int(n1.max())
            K3 = int(n3.max())
            K2 = int((degs - n1 - n3).max())
            if best is None or K1 + K2 + K3 < best[0]:
                best = (K1 + K2 + K3, T, K1, K2, K3)
        _, T, K1, K2, K3 = best
        kab.append((max(K1, 1), max(K2, 1), max(K3, 1), T))

    # build per-core idx16 + per-slot mask inputs (pad slots point at the
    # window base row -- a real row -- and are killed by a -1e30 mask)
    per_core = []
    for c in range(C):
        starts, gs_s = per_core_lists[c]
        packs = []
        masks = []
        for t in range(NT):
            K1, K2, K3, T = kab[t]
            idx1 = np.zeros((P, K1), np.int64)
            idx2 = np.zeros((P, K2), np.int64)
            idx3 = np.zeros((P, K3), np.int64)
            mask = np.full((P, K1 + K2 + K3), NEG_BIG, np.float32)
            sl = slice(t * P, (t + 1) * P)
            s1, f12, f123, f23, s3 = (cnts[c, k, sl] for k in range(5))
            degs = degp_all[c, sl]
            n1, n3 = split_node(T, s1, f12, f123, f23, s3, degs)
            n1 = np.minimum(n1, K1)
            n3 = np.minimum(n3, K3)
            for p in range(P):
                node = t * P + p
                lst = gs_s[starts[node]:starts[node + 1]]  # sorted by wrow
                a = int(n1[p]); b = int(n3[p])
                w1 = lst[:a]                       # s1 + leading f12/f123
                rest = lst[a:]
                w3 = rest[rest.size - b:] if b else rest[:0]
                w2 = rest[:rest.size - b]
                idx1[p, :a] = w1 - B1
                idx2[p, :w2.size] = w2 - B2
                idx3[p, :b] = w3 - B3
                mask[p, :a] = 0.0
                mask[p, K1:K1 + w2.size] = 0.0
                mask[p, K1 + K2:K1 + K2 + b] = 0.0
            for arr in (idx1, idx2, idx3):
                assert arr.min() >= 0 and arr.max() < W
            packs += [pack_idx16(idx1), pack_idx16(idx2), pack_idx16(idx3)]
            masks.append(mask)
        idx16 = np.concatenate(packs, axis=1)
        maskv = np.concatenate(masks, axis=1)

        # x shard: permuted, padded, transposed, f-padded, then pre-blocked
        # per tile as [t][p][chunk][m] so each tile's load is one contiguous
        # 4KB-per-partition DMA: xt2[t*128+p, c*128+m] = x^T[c*128+p, t*128+m]
        xs = x[c * NPCR:(c + 1) * NPCR][order_c[c]]      # [NPCR, f_in]
        xt = np.zeros((cfg.f_pad, NPC), np.float32)
        xt[:cfg.f_in, :NPCR] = xs.T
        KCh = cfg.kchunks
        xt2 = (xt.reshape(KCh, P, NT, P)
               .transpose(2, 1, 0, 3)                    # [t, p, c, m]
               .reshape(NT * P, KCh * P))
        per_core.append({"xt": np.ascontiguousarray(xt2), "idx16": idx16,
                         "mask": maskv})

    return per_core, kab, order_c


# --------------------------------------------------------------------------
# device kernel builder
# --------------------------------------------------------------------------

def build_kernel(cfg: Cfg, kab, phases: str = "ABCDE"):
    import concourse.bacc as bacc
    import concourse.tile as tile
    from concourse import bass, mybir
    from concourse.masks import make_identity

    P = cfg.P
    NH, NCO, ROW, WROW = cfg.nh, cfg.nc_out, cfg.row, cfg.wrow
    NPC, NT, KC = cfg.npc, cfg.ntiles, cfg.kchunks
    NFULL, WB, W = cfg.nfull, cfg.wb, cfg.win
    f32 = mybir.dt.float32
    bf16 = mybir.dt.bfloat16
    i16 = mybir.dt.int16
    AX = mybir.AxisListType.X
    OP = mybir.AluOpType
    AF = mybir.ActivationFunctionType
    idxw = sum(8 * (k1 + k2 + k3) for (k1, k2, k3, _) in kab)
    sumk = idxw // 8
    WBASES = cfg.wbases
    RPP = NFULL // P                # wide-table rows per partition (bounce)

    nc = bacc.Bacc("TRN2", target_bir_lowering=False, debug=False,
                   num_devices=cfg.n_cores)

    xt_d = nc.dram_tensor("xt", [NT * P, KC * P], bf16, kind="ExternalInput")
    idx_d = nc.dram_tensor("idx16", [P, idxw], i16, kind="ExternalInput")
    mask_d = nc.dram_tensor("mask", [P, sumk], bf16, kind="ExternalInput")
    w1_d = nc.dram_tensor("w1p", [P, KC * NH], bf16, kind="ExternalInput")
    b1_d = nc.dram_tensor("b1r", [P, NH], f32, kind="ExternalInput")
    w4_d = nc.dram_tensor("w4r", [NH, NCO], f32, kind="ExternalInput")
    b4_d = nc.dram_tensor("b4r", [P, NCO], f32, kind="ExternalInput")
    be_d = nc.dram_tensor("beta3r", [P, 1], f32, kind="ExternalInput")
    out_d = nc.dram_tensor("out", [NPC, NCO], f32, kind="ExternalOutput")

    tabA_l = nc.dram_tensor("tabA_l", [NPC, ROW], bf16)
    tabA_f = nc.dram_tensor("tabA_f", [NFULL, ROW], bf16, addr_space="Shared")
    tabB_l = nc.dram_tensor("tabB_l", [NPC, ROW], bf16)
    tabB_f = nc.dram_tensor("tabB_f", [NFULL, ROW], bf16, addr_space="Shared")
    tabA_w = nc.dram_tensor("tabA_w", [NFULL, WROW], bf16)  # G-phase only

    # persistent SBUF (survives across TileContexts)
    def sb(name, shape, dtype):
        return nc.alloc_sbuf_tensor(name, list(shape), dtype)

    ident_b = sb("identb", [P, P], bf16)
    w1_sb = sb("w1sb", [P, KC * NH], bf16)
    b1_sb = sb("b1sb", [P, NH], f32)
    w4_sb = sb("w4sb", [NH, NCO], f32)
    b4_sb = sb("b4sb", [P, NCO], f32)
    be_sb = sb("besb", [P, 1], f32)
    bee_sb = sb("beesb", [P, 1], f32)      # exp-ready: beta3 value
    h_sb = sb("hsb", [P, NT * NH], f32)
    rows_sb = sb("rowssb", [P, NT * ROW], bf16)
    sq_sb = sb("sqsb", [P, NT * NH], f32)
    ss_sb = sb("sssb", [P, NT], f32)
    inv_sb = sb("invsb", [P, NT], f32)
    idx_sb = sb("idxsb", [P, idxw], i16)    # all tiles' gather indices
    mask_sb = sb("masksb", [P, sumk], bf16)  # per-slot pad masks
    # SBUF-resident gather table: row r at partition r%128, stripe r//128
    table_sb = sb("tablesb", [P, RPP * ROW], bf16)

    def epilogue_rows(tab_local):
        """h_sb -> inv norm -> rows_sb -> DMA to tab_local."""
        h3v = h_sb.ap().rearrange("p (t j) -> p t j", t=NT)
        nc.vector.tensor_mul(sq_sb.ap(), h_sb.ap(), h_sb.ap())
        nc.vector.reduce_sum(
            ss_sb.ap(), sq_sb.ap().rearrange("p (t j) -> p t j", t=NT), axis=AX)
        nc.scalar.sqrt(ss_sb.ap(), ss_sb.ap())
        nc.vector.tensor_scalar_add(ss_sb.ap(), ss_sb.ap(), 1.0e-12)
        nc.vector.reciprocal(inv_sb.ap(), ss_sb.ap())
        rv = rows_sb.ap().rearrange("p (t j) -> p t j", t=NT)
        nc.vector.tensor_copy(rv[:, :, 0:16], h3v)
        nc.vector.tensor_copy(rv[:, :, 16], inv_sb.ap())
        nc.sync.dma_start(
            out=tab_local[:, :].rearrange("(t p) j -> p t j", p=P),
            in_=rv)

    def allgather(tab_local, tab_full):
        nc.gpsimd.collective_compute(
            "AllGather", OP.bypass,
            replica_groups=[list(range(cfg.n_cores))],
            ins=[tab_local.ap().opt()],
            outs=[tab_full.ap()[0:NFULL, :].opt()])

    def load_table(tab_full):
        """packed DRAM [NFULL, 20] -> SBUF table (row r at partition r%128,
        stripe r//128). SWDGE regular-pattern DMAs (~0.34ns/desc), chunked
        under the 16384-descriptor carveout limit."""
        tv = table_sb.ap().rearrange("p (r j) -> p r j", j=ROW)
        src = tab_full[:, :].rearrange("(r p) j -> p r j", p=P)
        CH = 96                      # stripes per chunk (<=16k descs)
        for o in range(0, RPP, CH):
            n = min(CH, RPP - o)
            nc.gpsimd.dma_start(out=tv[:, o:o + n, :], in_=src[:, o:o + n, :])

    # ---------------- phase A: consts, idx preload, L1, table A, AG1 -------
    with tile.TileContext(nc) as tc:
        make_identity(nc, ident_b.ap())
        nc.gpsimd.memset(rows_sb.ap(), 0.0)
        nc.sync.dma_start(out=idx_sb.ap(), in_=idx_d[:, :])
        nc.sync.dma_start(out=mask_sb.ap(), in_=mask_d[:, :])
        nc.sync.dma_start(out=w1_sb.ap(), in_=w1_d[:, :])
        nc.sync.dma_start(out=b1_sb.ap(), in_=b1_d[:, :])
        nc.sync.dma_start(out=w4_sb.ap(), in_=w4_d[:, :])
        nc.sync.dma_start(out=b4_sb.ap(), in_=b4_d[:, :])
        nc.sync.dma_start(out=be_sb.ap(), in_=be_d[:, :])

        with tc.tile_pool(name="l1x", bufs=3) as xp, \
             tc.tile_pool(name="l1p", bufs=4, space="PSUM") as pp:
            for t in range(NT):
                xw = xp.tile([P, KC * P], bf16, tag="xw")
                nc.sync.dma_start(
                    out=xw[:], in_=xt_d[t * P:(t + 1) * P, :])
                ps = pp.tile([P, NH], f32, tag="l1ps")
                for c in range(KC):
                    nc.tensor.matmul(
                        out=ps[:], lhsT=xw[:, c * P:(c + 1) * P],
                        rhs=w1_sb.ap()[:, c * NH:(c + 1) * NH],
                        start=(c == 0), stop=(c == KC - 1))
                hsl = h_sb.ap()[:, t * NH:(t + 1) * NH]
                nc.vector.tensor_add(hsl, ps[:], b1_sb.ap())
                nc.vector.tensor_scalar_max(hsl, hsl, 0.0)
        epilogue_rows(tabA_l)
        allgather(tabA_l, tabA_f)
    with tile.TileContext(nc) as tc:
        load_table(tabA_f)

    # ---------------- conv layer -------------------------------------------
    def conv(tab_local, beta_ap):
        """beta_ap: None for beta=1 (conv2), else [P,1] AP with beta value."""
        off = 0
        moff = 0
        with tile.TileContext(nc) as tc:
            with tc.tile_pool(name="cv", bufs=3) as cv, \
                 tc.tile_pool(name="cvs", bufs=2) as cvs:
                for t in range(NT):
                    kws = kab[t][:3]
                    K = sum(kws)
                    hs = cv.tile([P, K * ROW], bf16, tag="hs")
                    kacc = 0
                    for (kw, base) in zip(kws, WBASES):
                        s0 = (base // P) * ROW
                        dma_gather_raw(
                            nc,
                            out_ap=hs[:, kacc * ROW:(kacc + kw) * ROW]
                            .rearrange("p (c j) -> p c j", j=ROW),
                            in_ap=table_sb.ap()[:, s0:s0 + 256 * ROW],
                            idxs_ap=idx_sb.ap()[:, off:off + 8 * kw],
                            num_idxs=P * kw, elem_size=ROW,
                            sbuf_tokens_per_rank=P,
                            sbuf_free_dim_per_rank=ROW * 2)
                        off += 8 * kw
                        kacc += kw
                    assert kacc == K

                    hd = cvs.tile([P, ROW], bf16, tag="hd")
                    nc.sync.dma_start(
                        out=hd[:], in_=tab_local[t * P:(t + 1) * P, :]
                        .rearrange("(g p) j -> p (g j)", p=P))

                    hs3 = hs[:].rearrange("p (k j) -> p k j", k=K)
                    tmp = cv.tile([P, K * NH], bf16, tag="tmp")
                    tm3 = tmp[:].rearrange("p (k j) -> p k j", k=K)
                    nc.vector.tensor_mul(
                        tm3, hs3[:, :, 0:16],
                        hd[:, None, 0:16].to_broadcast([P, K, 16]))
                    alpha = cv.tile([P, K], f32, tag="alpha")
                    nc.vector.reduce_sum(
                        alpha[:],
                        tmp[:].rearrange("p (k j) -> p k j", j=NH), axis=AX)
                    nc.vector.tensor_mul(alpha[:], alpha[:], hs3[:, :, 16])
                    invd = cvs.tile([P, 1], f32, tag="invd")
                    if beta_ap is None:
                        nc.vector.tensor_copy(invd[:], hd[:, 16:17])
                    else:
                        nc.vector.tensor_mul(invd[:], hd[:, 16:17], beta_ap)
                    nc.vector.tensor_mul(
                        alpha[:], alpha[:],
                        invd[:].to_broadcast([P, K]))
                    # pad slots killed by the -1e30 static mask
                    nc.vector.tensor_add(
                        alpha[:], alpha[:], mask_sb.ap()[:, moff:moff + K])
                    moff += K
                    # |alpha| <= |beta| for real slots: exp without max-trick
                    e_bf = cv.tile([P, K], bf16, tag="e")
                    nc.scalar.activation(e_bf[:], alpha[:], AF.Exp)
                    s = cvs.tile([P, 1], f32, tag="s")
                    nc.vector.reduce_sum(
                        s[:], e_bf[:][:, None, :], axis=AX)
                    # analytic self-loop: cos(h,h) = (1 - 1e-12*invn)^2
                    selfa = cvs.tile([P, 1], f32, tag="selfa")
                    nc.vector.tensor_scalar(
                        selfa[:], hd[:, 16:17], -1.0e-12, 1.0,
                        op0=OP.mult, op1=OP.add)
                    nc.vector.tensor_mul(selfa[:], selfa[:], selfa[:])
                    if beta_ap is not None:
                        nc.vector.tensor_mul(selfa[:], selfa[:], beta_ap)
                    es = cvs.tile([P, 1], f32, tag="es")
                    nc.scalar.activation(es[:], selfa[:], AF.Exp)
                    nc.vector.tensor_add(s[:], s[:], es[:])
                    nc.vector.tensor_scalar_add(s[:], s[:], 1.0e-16)
                    r = cvs.tile([P, 1], f32, tag="r")
                    nc.vector.reciprocal(r[:], s[:])
                    coef = cv.tile([P, K], bf16, tag="coef")
                    nc.vector.tensor_mul(
                        coef[:], e_bf[:], r[:].to_broadcast([P, K]))
                    tmp2 = cv.tile([P, K * NH], bf16, tag="tmp2")
                    t23 = tmp2[:].rearrange("p (k j) -> p k j", k=K)
                    nc.vector.tensor_mul(
                        t23, hs3[:, :, 0:16],
                        coef[:][:, :, None].to_broadcast([P, K, 16]))
                    h2v = h_sb.ap()[:, t * NH:(t + 1) * NH]
                    nc.vector.reduce_sum(
                        h2v,
                        tmp2[:].rearrange("p (k j) -> p j k", k=K),
                        axis=AX)
                    # += (exp(self)/s) * h_d
                    rs = cvs.tile([P, 1], f32, tag="rs")
                    nc.vector.tensor_mul(rs[:], es[:], r[:])
                    sh = cvs.tile([P, NH], f32, tag="sh")
                    nc.vector.tensor_mul(
                        sh[:], hd[:, 0:16], rs[:].to_broadcast([P, NH]))
                    nc.vector.tensor_add(h2v, h2v, sh[:])
        assert off == idxw

    if "G" in phases:
        # debug: dma_gather NCH chunks of table rows via the ant gather and
        # dump raw rows (as f32). G_MODE: dram (256B-strided) | sbuf
        import os
        NCH = int(os.environ.get("G_NCH", "49"))
        G_NINST = int(os.environ.get("G_NINST", "1"))
        G_MODE = os.environ.get("G_MODE", "dram")
        nidx = NCH * P
        widx = nidx // 16
        idx16_d = nc.dram_tensor("idx16g", [P, widx], mybir.dt.int16,
                                 kind="ExternalInput")
        with tile.TileContext(nc) as tc:
            if G_MODE == "dram":
                nc.sync.dma_start(out=tabA_w[:, 0:ROW], in_=tabA_f[:, :])
            else:
                # sbuf layout: row r -> partition r%128, stripe r//128
                load_table(tabA_f)
            with tc.tile_pool(name="dbg", bufs=1) as dbg:
                idx_sbg = dbg.tile([P, widx], mybir.dt.int16, tag="idx16")
                nc.sync.dma_start(out=idx_sbg[:], in_=idx16_d[:, :])
                hs = dbg.tile([P, NCH * ROW], bf16, tag="hs")
                nc.gpsimd.memset(hs[:], 0.0)
                if G_DO_GATHER:
                    for _ in range(G_NINST):
                        if G_MODE == "dram":
                            dma_gather_raw(
                                nc,
                                out_ap=hs[:].rearrange(
                                    "p (c j) -> p c j", j=ROW),
                                in_ap=tabA_w[:, 0:ROW], idxs_ap=idx_sbg[:],
                                num_idxs=nidx, elem_size=ROW,
                                stride_elems=128)
                        else:
                            dma_gather_raw(
                                nc,
                                out_ap=hs[:].rearrange(
                                    "p (c j) -> p c j", j=ROW),
                                in_ap=table_sb.ap()[:, 0:RPP * ROW],
                                idxs_ap=idx_sbg[:],
                                num_idxs=nidx, elem_size=ROW,
                                sbuf_tokens_per_rank=P,
                                sbuf_free_dim_per_rank=ROW * 2)
                ncols = min(NCH * ROW, 980)
                hf = dbg.tile([P, ncols], f32, tag="hf")
                nc.vector.tensor_copy(hf[:], hs[:, 0:ncols])
                ov = out_d.ap().rearrange("(p q) j -> p (q j)", p=P)
                nc.sync.dma_start(out=ov[:, 0:ncols], in_=hf[:])
        nc.compile()
        return nc

    if "B" in phases:
        conv(tabA_l, None)

    # ---------------- phase C: table B + AG2 -------------------------------
    if "C" in phases:
        with tile.TileContext(nc) as tc:
            epilogue_rows(tabB_l)
            allgather(tabB_l, tabB_f)
        with tile.TileContext(nc) as tc:
            load_table(tabB_f)

    if "D" in phases:
        conv(tabB_l, be_sb.ap()[:, 0:1])

    if "E" not in phases:
        # debug: dump h_sb (and inv_sb) into out
        with tile.TileContext(nc) as tc:
            ov = out_d.ap().rearrange("(p q) j -> p (q j)", p=P)
            nc.sync.dma_start(out=ov[:, 0:NT * NH], in_=h_sb.ap())
            nc.sync.dma_start(out=ov[:, NT * NH:NT * NH + NT], in_=inv_sb.ap())
        nc.compile()
        return nc

    with tile.TileContext(nc) as tc:
        with tc.tile_pool(name="hd", bufs=1) as hp, \
             tc.tile_pool(name="hdp", bufs=4, space="PSUM") as hpp:
            hb = hp.tile([P, NT * NH], bf16, tag="hb")
            nc.vector.tensor_copy(hb[:], h_sb.ap())
            h3t = hp.tile([NH, NT * P], bf16, tag="h3t")
            for t in range(NT):
                pst = hpp.tile([NH, P], bf16, tag="pst")
                nc.tensor.transpose(
                    out=pst[:], in_=hb[:, t * NH:(t + 1) * NH],
                    identity=ident_b.ap())
                nc.vector.tensor_copy(h3t[:, t * P:(t + 1) * P], pst[:])
            w4b = hp.tile([NH, NCO], bf16, tag="w4b")
            nc.vector.tensor_copy(w4b[:], w4_sb.ap())
            lg = hp.tile([P, NT * NCO], f32, tag="lg")
            for t in range(NT):
                psl = hpp.tile([P, NCO], f32, tag="psl")
                nc.tensor.matmul(
                    out=psl[:], lhsT=h3t[:, t * P:(t + 1) * P], rhs=w4b[:],
                    start=True, stop=True)
                nc.vector.tensor_add(
                    lg[:, t * NCO:(t + 1) * NCO], psl[:], b4_sb.ap())
            lg3 = lg[:].rearrange("p (t j) -> p t j", t=NT)
            nm = hp.tile([P, NT], f32, tag="hnm")
            nc.vector.reduce_max(nm[:], lg3, axis=AX, negate=True)
            nc.vector.tensor_add(
                lg3, lg3, nm[:][:, :, None].to_broadcast([P, NT, NCO]))
            ex = hp.tile([P, NT * NCO], f32, tag="ex")
            nc.scalar.activation(ex[:], lg[:], AF.Exp)
            s = hp.tile([P, NT], f32, tag="hs_sum")
            nc.vector.reduce_sum(
                s[:], ex[:].rearrange("p (t j) -> p t j", t=NT), axis=AX)
            ls = hp.tile([P, NT], f32, tag="ls")
            nc.scalar.activation(ls[:], s[:], AF.Ln)
            nc.vector.tensor_sub(
                lg3, lg3, ls[:][:, :, None].to_broadcast([P, NT, NCO]))
            nc.sync.dma_start(
                out=out_d[:, :].rearrange("(t p) j -> p t j", p=P),
                in_=lg3)

    nc.compile()
    return nc


# --------------------------------------------------------------------------
# entry point
# --------------------------------------------------------------------------

def run(cfg: Cfg, inputs: dict, trace: bool = False):
    from concourse import bass_utils

    x = np.asarray(inputs["x"], np.float32)
    edge_index = np.asarray(inputs["edge_index"])
    W1 = np.asarray(inputs["W1"], np.float32)
    b1 = np.asarray(inputs["b1"], np.float32)
    W4 = np.asarray(inputs["W4"], np.float32)
    b4 = np.asarray(inputs["b4"], np.float32)
    beta3 = np.asarray(inputs["beta3"], np.float32)

    import ml_dtypes

    per_core, kab, order_c = preprocess(cfg, x, edge_index)
    nc = build_kernel(cfg, kab)

    P, KC, NH = cfg.P, cfg.kchunks, cfg.nh
    w1p = np.zeros((cfg.f_pad, NH), np.float32)
    w1p[:cfg.f_in] = W1
    w1p = np.ascontiguousarray(
        w1p.reshape(KC, P, NH).transpose(1, 0, 2).reshape(P, KC * NH)
    ).astype(ml_dtypes.bfloat16)
    b1r = np.ascontiguousarray(np.broadcast_to(b1[None, :], (P, NH)))
    b4r = np.ascontiguousarray(np.broadcast_to(b4[None, :], (P, cfg.nc_out)))
    ber = np.ascontiguousarray(np.broadcast_to(beta3[None, :], (P, 1)))

    in_maps = []
    for c in range(cfg.n_cores):
        in_maps.append({
            "xt": per_core[c]["xt"].astype(ml_dtypes.bfloat16),
            "idx16": per_core[c]["idx16"],
            "mask": per_core[c]["mask"].astype(ml_dtypes.bfloat16),
            "w1p": w1p, "b1r": b1r, "w4r": np.ascontiguousarray(W4),
            "b4r": b4r, "beta3r": ber,
        })

    res = bass_utils.run_bass_kernel_spmd(
        nc, in_maps, core_ids=list(range(cfg.n_cores)), trace=trace)

    out = np.empty((cfg.n_nodes, cfg.nc_out), np.float32)
    for c in range(cfg.n_cores):
        oc = np.asarray(res.results[c]["out"])[:cfg.npc_raw]
        out[c * cfg.npc_raw + order_c[c]] = oc
    return out, res


def kernel(**inputs) -> np.ndarray:
    out, _ = run(FULL, inputs, trace=False)
    return out


# revision 56
# speedup vs baseline: 1.3591x; 1.0101x over previous
"""AGNNet (2-layer AGNN conv + linear head) distributed over 8 trn2 NeuronCores.

Strategy (graph/data parallel, per sharding hint):
  - nodes sharded by dst range: core c owns nodes [c*6250, (c+1)*6250)
  - host groups edges by dst, degree-sorts each core's local nodes (undone on
    output), drops self-loops (handled analytically in the softmax), and packs
    each 128-node tile's in-edge lists into per-tile slot columns
  - device: L1 matmul from host pre-blocked x shard; build a bf16 row table
    [h(16), inv_norm, 0, 0] per node; AllGather the packed table (2MB);
    redistribute it into SBUF (row r -> partition r%128, stripe r//128,
    15.7KB/partition); per node-tile one SBUF-source dma_gather (ant extended
    Q7 kernel, ~7.7ns/idx -- the Q7 descriptor-generation rate is THE
    bottleneck of this problem; indirect1d would cost ~11ns/edge) per index
    window fetches all neighbor rows; DVE softmax (no max-trick needed:
    |alpha| <= |beta|); the self-loop term exp(beta*cos(h,h)) is added
    analytically from inv_norm alone; second conv identical; bf16 head
    matmul + log_softmax.
  - dma_gather indices are int16, so the 50176-row table is addressed through
    THREE overlapping 32768-row windows (bases 0/8704/17408 as free-dim
    offsets into the SBUF table); the host balances each node's slots across
    windows per tile (~+7%% padding); padding slots point at the window base
    row and are killed by a -1e30 static per-slot mask added to alpha.
"""

import math
from contextlib import ExitStack
from dataclasses import dataclass

import numpy as np


@dataclass
class Cfg:
    n_cores: int = 8
    n_nodes: int = 50000
    f_in: int = 2000
    nh: int = 16
    nc_out: int = 20
    row: int = 20           # table row elems: 16 h, [16]=invn, [17]=bias, 18-19 pad
    wrow: int = 128         # wide-table row elems (256B stride for dma_gather)
    P: int = 128
    win: int = 32768        # dma_gather int16 index window (rows per base)

    @property
    def npc_raw(self) -> int:       # real nodes per core
        return self.n_nodes // self.n_cores

    @property
    def npc(self) -> int:           # padded nodes per core (mult of 128)
        return ((self.npc_raw + self.P - 1) // self.P) * self.P

    @property
    def ntiles(self) -> int:
        return self.npc // self.P

    @property
    def kchunks(self) -> int:       # 128-row chunks of the padded f_in
        return (self.f_in + self.P - 1) // self.P

    @property
    def f_pad(self) -> int:
        return self.kchunks * self.P

    @property
    def nfull(self) -> int:
        return self.n_cores * self.npc

    @property
    def wb(self) -> int:            # last window base
        return self.nfull - self.win

    @property
    def wbases(self) -> tuple:      # 3 overlapping 32768-row window bases
        return (0, self.wb // 2, self.wb)


FULL = Cfg()
NEG_BIG = -1.0e30
G_DO_GATHER = True


def pack_idx16(idx_mat: np.ndarray) -> np.ndarray:
    """[128 partitions, nch chunks] window-relative indices -> dma_gather
    int16 index tile [128, nch*8]: list position c*128+j lands on partition j
    chunk c (the ucode lane swizzle applies to both the index read and the
    dst partition, so it cancels); the flat list is wrapped over 16
    partitions and replicated across the 8 16-partition groups."""
    P, nch = idx_mat.shape
    assert P == 128
    flat = np.ascontiguousarray(idx_mat.T).reshape(-1).astype(np.int16)
    tile = np.empty((128, nch * 8), np.int16)
    for p in range(16):
        tile[p] = flat[p::16]
    tile[16:] = np.tile(tile[:16], (7, 1))
    return np.ascontiguousarray(tile)


def dma_gather_raw(nc, out_ap, in_ap, idxs_ap, num_idxs: int,
                   elem_size: int, stride_elems: int = 0,
                   sbuf_tokens_per_rank: int = 0,
                   sbuf_free_dim_per_rank: int = 0):
    """nc.gpsimd.dma_gather minus the elem_size%256 assert (non-transpose
    path in the Q7 ucode has no such restriction; only the DRAM stride must
    be a multiple of 256B) and minus the SBUF-source transpose-only assert
    (gen_descs handles src_is_sbuf in the plain path too). For SBUF source:
    idx i reads partition i % tokens_per_rank, free-byte offset
    (i // tokens_per_rank) * free_dim_per_rank. single_packet=False: a
    single packet is capped at 64 descriptors, large gathers hang with it."""
    from concourse import mybir
    g = nc.gpsimd
    src_is_sbuf = sbuf_tokens_per_rank > 0
    if src_is_sbuf:
        stride_bytes_256 = 0
        if in_ap.dtype != out_ap.dtype:
            in_ap = in_ap.bitcast(out_ap.dtype)
        _in_ap = [g.lower_ap(in_ap)]
    else:
        stride_bytes = stride_elems * mybir.dt.size(in_ap.dtype)
        assert stride_bytes % 256 == 0 and stride_bytes // 256 < 256
        stride_bytes_256 = stride_bytes // 256
        _in_ap = g.lower_ap_dma(in_ap, for_custom_bir_dma=True)
    _idxs_ap = g.lower_ap(idxs_ap)
    _out_ap = g.lower_ap(out_ap)
    return g.add_instruction(
        mybir.InstDMAGatherAnt(
            name=g.bass.get_next_instruction_name(),
            ins=[*_in_ap, _idxs_ap, g.lower_val_access(g.to_reg(num_idxs))],
            outs=[_out_ap],
            transpose=False,
            num_idxs=num_idxs,
            elem_size=elem_size,
            stride_bytes_256=stride_bytes_256,
            gen_mode=0,
            single_packet=False,
            queue_num=0,
            sbuf_tokens_per_rank=sbuf_tokens_per_rank,
            sbuf_free_dim_per_rank=sbuf_free_dim_per_rank,
            sbuf_free_dim_pad_per_rank=0,
            sbuf_byte_offset=0,
        ))


# --------------------------------------------------------------------------
# host-side preprocessing
# --------------------------------------------------------------------------

def preprocess(cfg: Cfg, x: np.ndarray, edge_index: np.ndarray):
    """Shard + layout transform.

    Returns (per_core list of dicts, kab, order_c) where kab is the shared
    [(K1_t, K2_t, K3_t)] per tile (same across cores for SPMD): slot columns
    per index window (3 overlapping 32768-row windows of the wide table)."""
    P, NPC, NPCR = cfg.P, cfg.npc, cfg.npc_raw
    n, C, NT = cfg.n_nodes, cfg.n_cores, cfg.ntiles
    W = cfg.win
    B1, B2, B3 = cfg.wbases

    src = edge_index[0].astype(np.int64)
    dst = edge_index[1].astype(np.int64)
    keep = src != dst                     # self-loops handled analytically
    src, dst = src[keep], dst[keep]

    core_of_dst = dst // NPCR

    # pass 1: per-core degree sort -> rank of each node within its core
    order_c, rank_c = [], []
    for c in range(C):
        m = core_of_dst == c
        ld = dst[m] - c * NPCR
        deg = np.bincount(ld, minlength=NPCR)
        order = np.argsort(deg, kind="stable")      # ascending degree
        rank = np.empty(NPCR, np.int64)
        rank[order] = np.arange(NPCR)
        order_c.append(order)
        rank_c.append(rank)

    # new global id after per-core permutation + padding (= wide-table row)
    new_gid = np.empty(n, np.int64)
    for c in range(C):
        new_gid[c * NPCR:(c + 1) * NPCR] = c * NPC + rank_c[c]

    # per-core per-node edge lists sorted by wide-row
    per_core_lists = []
    for c in range(C):
        m = core_of_dst == c
        ld = dst[m] - c * NPCR
        gs = new_gid[src[m]]
        er = rank_c[c][ld]                       # dst rank of each edge
        eo = np.lexsort((gs, er))
        er_s = er[eo]
        gs_s = gs[eo]
        starts = np.zeros(NPC + 1, np.int64)
        np.cumsum(np.bincount(er_s, minlength=NPC), out=starts[1:])
        per_core_lists.append((starts, gs_s))

    # per-node class counts: s1 [0,B2) strictly-W1; f12 [B2,B3); f123
    # [B3, W) (in all); f23 [W, B2+W); s3 [B2+W, nfull)
    bnds = np.array([B2, B3, W, B2 + W], np.int64)
    cnts = np.zeros((C, 5, NPC), np.int64)
    degp_all = np.zeros((C, NPC), np.int64)
    for c in range(C):
        starts, gs_s = per_core_lists[c]
        d_cnt = np.diff(starts)
        degp_all[c] = d_cnt
        er_of_edge = np.repeat(np.arange(NPC), d_cnt)
        cls = np.searchsorted(bnds, gs_s, side="right")
        for k in range(5):
            cnts[c, k] = np.bincount(
                er_of_edge, weights=(cls == k), minlength=NPC)

    def split_node(T, s1, f12, f123, f23, s3, deg):
        """-> (n1, n3) target-T balanced; n2 = deg - n1 - n3 (all W2-ok)."""
        n1 = np.maximum(s1, np.minimum(T, s1 + f12 + f123))
        f123_left = f123 - np.maximum(0, n1 - (s1 + f12))
        n3 = np.maximum(s3, np.minimum(T, s3 + f23 + f123_left))
        return n1, n3

    kab = []
    for t in range(NT):
        s, e = t * P, (t + 1) * P
        degs = degp_all[:, s:e]
        s1, f12, f123, f23, s3 = (cnts[:, k, s:e] for k in range(5))
        kmax = int(degs.max())
        best = None
        for T in range(kmax + 1):
            n1, n3 = split_node(T, s1, f12, f123, f23, s3, degs)
            K1 = int(n1.max())
            K3 = int(n3.max())
            K2 = int((degs - n1 - n3).max())
            if best is None or K1 + K2 + K3 < best[0]:
                best = (K1 + K2 + K3, T, K1, K2, K3)
        _, T, K1, K2, K3 = best
        kab.append((max(K1, 1), max(K2, 1), max(K3, 1), T))

    # build per-core idx16 + per-slot mask inputs (pad slots point at the
    # window base row -- a real row -- and are killed by a -1e30 mask)
    per_core = []
    for c in range(C):
        starts, gs_s = per_core_lists[c]
        packs = []
        masks = []
        for t in range(NT):
            K1, K2, K3, T = kab[t]
            idx1 = np.zeros((P, K1), np.int64)
            idx2 = np.zeros((P, K2), np.int64)
            idx3 = np.zeros((P, K3), np.int64)
            mask = np.full((P, K1 + K2 + K3), NEG_BIG, np.float32)
            sl = slice(t * P, (t + 1) * P)
            s1, f12, f123, f23, s3 = (cnts[c, k, sl] for k in range(5))
            degs = degp_all[c, sl]
            n1, n3 = split_node(T, s1, f12, f123, f23, s3, degs)
            n1 = np.minimum(n1, K1)
            n3 = np.minimum(n3, K3)
            for p in range(P):
                node = t * P + p
                lst = gs_s[starts[node]:starts[node + 1]]  # sorted by wrow
                a = int(n1[p]); b = int(n3[p])
                w1 = lst[:a]                       # s1 + leading f12/f123
                rest = lst[a:]
                w3 = rest[rest.size - b:] if b else rest[:0]
                w2 = rest[:rest.size - b]
                idx1[p, :a] = w1 - B1
                idx2[p, :w2.size] = w2 - B2
                idx3[p, :b] = w3 - B3
                mask[p, :a] = 0.0
                mask[p, K1:K1 + w2.size] = 0.0
                mask[p, K1 + K2:K1 + K2 + b] = 0.0
            for arr in (idx1, idx2, idx3):
                assert arr.min() >= 0 and arr.max() < W
            packs += [pack_idx16(idx1), pack_idx16(idx2), pack_idx16(idx3)]
            masks.append(mask)
        idx16 = np.concatenate(packs, axis=1)
        maskv = np.concatenate(masks, axis=1)

        # x shard: permuted, padded, transposed, f-padded, then pre-blocked
        # per tile as [t][p][chunk][m] so each tile's load is one contiguous
        # 4KB-per-partition DMA: xt2[t*128+p, c*128+m] = x^T[c*128+p, t*128+m]
        xs = x[c * NPCR:(c + 1) * NPCR][order_c[c]]      # [NPCR, f_in]
        xt = np.zeros((cfg.f_pad, NPC), np.float32)
        xt[:cfg.f_in, :NPCR] = xs.T
        KCh = cfg.kchunks
        xt2 = (xt.reshape(KCh, P, NT, P)
               .transpose(2, 1, 0, 3)                    # [t, p, c, m]
               .reshape(NT * P, KCh * P))
        per_core.append({"xt": np.ascontiguousarray(xt2), "idx16": idx16,
                         "mask": maskv})

    return per_core, kab, order_c


# --------------------------------------------------------------------------
# device kernel builder
# --------------------------------------------------------------------------

def build_kernel(cfg: Cfg, kab, phases: str = "ABCDE"):
    import concourse.bacc as bacc
    import concourse.tile as tile
    from concourse import bass, mybir
    from concourse.masks import make_identity

    P = cfg.P
    NH, NCO, ROW, WROW = cfg.nh, cfg.nc_out, cfg.row, cfg.wrow
    NPC, NT, KC = cfg.npc, cfg.ntiles, cfg.kchunks
    NFULL, WB, W = cfg.nfull, cfg.wb, cfg.win
    f32 = mybir.dt.float32
    bf16 = mybir.dt.bfloat16
    i16 = mybir.dt.int16
    AX = mybir.AxisListType.X
    OP = mybir.AluOpType
    AF = mybir.ActivationFunctionType
    idxw = sum(8 * (k1 + k2 + k3) for (k1, k2, k3, _) in kab)
    sumk = idxw // 8
    WBASES = cfg.wbases
    RPP = NFULL // P                # wide-table rows per partition (bounce)

    nc = bacc.Bacc("TRN2", target_bir_lowering=False, debug=False,
                   num_devices=cfg.n_cores)

    xt_d = nc.dram_tensor("xt", [NT * P, KC * P], bf16, kind="ExternalInput")
    idx_d = nc.dram_tensor("idx16", [P, idxw], i16, kind="ExternalInput")
    mask_d = nc.dram_tensor("mask", [P, sumk], bf16, kind="ExternalInput")
    w1_d = nc.dram_tensor("w1p", [P, KC * NH], bf16, kind="ExternalInput")
    b1_d = nc.dram_tensor("b1r", [P, NH], f32, kind="ExternalInput")
    w4_d = nc.dram_tensor("w4r", [NH, NCO], f32, kind="ExternalInput")
    b4_d = nc.dram_tensor("b4r", [P, NCO], f32, kind="ExternalInput")
    be_d = nc.dram_tensor("beta3r", [P, 1], f32, kind="ExternalInput")
    out_d = nc.dram_tensor("out", [NPC, NCO], f32, kind="ExternalOutput")

    tabA_l = nc.dram_tensor("tabA_l", [NPC, ROW], bf16)
    tabA_f = nc.dram_tensor("tabA_f", [NFULL, ROW], bf16, addr_space="Shared")
    tabB_l = nc.dram_tensor("tabB_l", [NPC, ROW], bf16)
    tabB_f = nc.dram_tensor("tabB_f", [NFULL, ROW], bf16, addr_space="Shared")
    tabA_w = nc.dram_tensor("tabA_w", [NFULL, WROW], bf16)  # G-phase only

    # persistent SBUF (survives across TileContexts)
    def sb(name, shape, dtype):
        return nc.alloc_sbuf_tensor(name, list(shape), dtype)

    ident_b = sb("identb", [P, P], bf16)
    w1_sb = sb("w1sb", [P, KC * NH], bf16)
    b1_sb = sb("b1sb", [P, NH], f32)
    w4_sb = sb("w4sb", [NH, NCO], f32)
    b4_sb = sb("b4sb", [P, NCO], f32)
    be_sb = sb("besb", [P, 1], f32)
    bee_sb = sb("beesb", [P, 1], f32)      # exp-ready: beta3 value
    h_sb = sb("hsb", [P, NT * NH], f32)
    rows_sb = sb("rowssb", [P, NT * ROW], bf16)
    sq_sb = sb("sqsb", [P, NT * NH], f32)
    ss_sb = sb("sssb", [P, NT], f32)
    inv_sb = sb("invsb", [P, NT], f32)
    idx_sb = sb("idxsb", [P, idxw], i16)    # all tiles' gather indices
    mask_sb = sb("masksb", [P, sumk], bf16)  # per-slot pad masks
    # SBUF-resident gather table: row r at partition r%128, stripe r//128
    table_sb = sb("tablesb", [P, RPP * ROW], bf16)

    def epilogue_rows(tab_local):
        """h_sb -> inv norm -> rows_sb -> DMA to tab_local."""
        h3v = h_sb.ap().rearrange("p (t j) -> p t j", t=NT)
        nc.vector.tensor_mul(sq_sb.ap(), h_sb.ap(), h_sb.ap())
        nc.vector.reduce_sum(
            ss_sb.ap(), sq_sb.ap().rearrange("p (t j) -> p t j", t=NT), axis=AX)
        nc.scalar.sqrt(ss_sb.ap(), ss_sb.ap())
        nc.vector.tensor_scalar_add(ss_sb.ap(), ss_sb.ap(), 1.0e-12)
        nc.vector.reciprocal(inv_sb.ap(), ss_sb.ap())
        rv = rows_sb.ap().rearrange("p (t j) -> p t j", t=NT)
        nc.vector.tensor_copy(rv[:, :, 0:16], h3v)
        nc.vector.tensor_copy(rv[:, :, 16], inv_sb.ap())
        nc.sync.dma_start(
            out=tab_local[:, :].rearrange("(t p) j -> p t j", p=P),
            in_=rv)

    def allgather(tab_local, tab_full):
        nc.gpsimd.collective_compute(
            "AllGather", OP.bypass,
            replica_groups=[list(range(cfg.n_cores))],
            ins=[tab_local.ap().opt()],
            outs=[tab_full.ap()[0:NFULL, :].opt()])

    def load_table(tab_full):
        """packed DRAM [NFULL, 20] -> SBUF table (row r at partition r%128,
        stripe r//128). SWDGE regular-pattern DMAs (~0.34ns/desc), chunked
        under the 16384-descriptor carveout limit."""
        tv = table_sb.ap().rearrange("p (r j) -> p r j", j=ROW)
        src = tab_full[:, :].rearrange("(r p) j -> p r j", p=P)
        CH = 56          # stripes per chunk: 2 chunks fit the 16k-desc ring,
        for o in range(0, RPP, CH):  # so generation overlaps drain
            n = min(CH, RPP - o)
            nc.gpsimd.dma_start(out=tv[:, o:o + n, :], in_=src[:, o:o + n, :])

    # ---------------- phase A: consts, idx preload, L1, table A, AG1 -------
    with tile.TileContext(nc) as tc:
        make_identity(nc, ident_b.ap())
        nc.gpsimd.memset(rows_sb.ap(), 0.0)
        nc.sync.dma_start(out=idx_sb.ap(), in_=idx_d[:, :])
        nc.sync.dma_start(out=mask_sb.ap(), in_=mask_d[:, :])
        nc.sync.dma_start(out=w1_sb.ap(), in_=w1_d[:, :])
        nc.sync.dma_start(out=b1_sb.ap(), in_=b1_d[:, :])
        nc.sync.dma_start(out=w4_sb.ap(), in_=w4_d[:, :])
        nc.sync.dma_start(out=b4_sb.ap(), in_=b4_d[:, :])
        nc.sync.dma_start(out=be_sb.ap(), in_=be_d[:, :])

        with tc.tile_pool(name="l1x", bufs=3) as xp, \
             tc.tile_pool(name="l1p", bufs=4, space="PSUM") as pp:
            for t in range(NT):
                xw = xp.tile([P, KC * P], bf16, tag="xw")
                nc.sync.dma_start(
                    out=xw[:], in_=xt_d[t * P:(t + 1) * P, :])
                ps = pp.tile([P, NH], f32, tag="l1ps")
                for c in range(KC):
                    nc.tensor.matmul(
                        out=ps[:], lhsT=xw[:, c * P:(c + 1) * P],
                        rhs=w1_sb.ap()[:, c * NH:(c + 1) * NH],
                        start=(c == 0), stop=(c == KC - 1))
                hsl = h_sb.ap()[:, t * NH:(t + 1) * NH]
                nc.vector.tensor_add(hsl, ps[:], b1_sb.ap())
                nc.vector.tensor_scalar_max(hsl, hsl, 0.0)
        epilogue_rows(tabA_l)
        allgather(tabA_l, tabA_f)
    with tile.TileContext(nc) as tc:
        load_table(tabA_f)

    # ---------------- conv layer -------------------------------------------
    def conv(tab_local, beta_ap):
        """beta_ap: None for beta=1 (conv2), else [P,1] AP with beta value."""
        off = 0
        moff = 0
        with tile.TileContext(nc) as tc:
            with tc.tile_pool(name="cv", bufs=3) as cv, \
                 tc.tile_pool(name="cvs", bufs=2) as cvs:
                for t in range(NT):
                    kws = kab[t][:3]
                    K = sum(kws)
                    hs = cv.tile([P, K * ROW], bf16, tag="hs")
                    kacc = 0
                    for (kw, base) in zip(kws, WBASES):
                        s0 = (base // P) * ROW
                        dma_gather_raw(
                            nc,
                            out_ap=hs[:, kacc * ROW:(kacc + kw) * ROW]
                            .rearrange("p (c j) -> p c j", j=ROW),
                            in_ap=table_sb.ap()[:, s0:s0 + 256 * ROW],
                            idxs_ap=idx_sb.ap()[:, off:off + 8 * kw],
                            num_idxs=P * kw, elem_size=ROW,
                            sbuf_tokens_per_rank=P,
                            sbuf_free_dim_per_rank=ROW * 2)
                        off += 8 * kw
                        kacc += kw
                    assert kacc == K

                    hd = cvs.tile([P, ROW], bf16, tag="hd")
                    nc.sync.dma_start(
                        out=hd[:], in_=tab_local[t * P:(t + 1) * P, :]
                        .rearrange("(g p) j -> p (g j)", p=P))

                    hs3 = hs[:].rearrange("p (k j) -> p k j", k=K)
                    tmp = cv.tile([P, K * NH], bf16, tag="tmp")
                    tm3 = tmp[:].rearrange("p (k j) -> p k j", k=K)
                    nc.vector.tensor_mul(
                        tm3, hs3[:, :, 0:16],
                        hd[:, None, 0:16].to_broadcast([P, K, 16]))
                    alpha = cv.tile([P, K], f32, tag="alpha")
                    nc.vector.reduce_sum(
                        alpha[:],
                        tmp[:].rearrange("p (k j) -> p k j", j=NH), axis=AX)
                    nc.vector.tensor_mul(alpha[:], alpha[:], hs3[:, :, 16])
                    invd = cvs.tile([P, 1], f32, tag="invd")
                    if beta_ap is None:
                        nc.vector.tensor_copy(invd[:], hd[:, 16:17])
                    else:
                        nc.vector.tensor_mul(invd[:], hd[:, 16:17], beta_ap)
                    nc.vector.tensor_mul(
                        alpha[:], alpha[:],
                        invd[:].to_broadcast([P, K]))
                    # pad slots killed by the -1e30 static mask
                    nc.vector.tensor_add(
                        alpha[:], alpha[:], mask_sb.ap()[:, moff:moff + K])
                    moff += K
                    # |alpha| <= |beta| for real slots: exp without max-trick
                    e_bf = cv.tile([P, K], bf16, tag="e")
                    nc.scalar.activation(e_bf[:], alpha[:], AF.Exp)
                    s = cvs.tile([P, 1], f32, tag="s")
                    nc.vector.reduce_sum(
                        s[:], e_bf[:][:, None, :], axis=AX)
                    # analytic self-loop: cos(h,h) = (1 - 1e-12*invn)^2
                    selfa = cvs.tile([P, 1], f32, tag="selfa")
                    nc.vector.tensor_scalar(
                        selfa[:], hd[:, 16:17], -1.0e-12, 1.0,
                        op0=OP.mult, op1=OP.add)
                    nc.vector.tensor_mul(selfa[:], selfa[:], selfa[:])
                    if beta_ap is not None:
                        nc.vector.tensor_mul(selfa[:], selfa[:], beta_ap)
                    es = cvs.tile([P, 1], f32, tag="es")
                    nc.scalar.activation(es[:], selfa[:], AF.Exp)
                    nc.vector.tensor_add(s[:], s[:], es[:])
                    nc.vector.tensor_scalar_add(s[:], s[:], 1.0e-16)
                    r = cvs.tile([P, 1], f32, tag="r")
                    nc.vector.reciprocal(r[:], s[:])
                    coef = cv.tile([P, K], bf16, tag="coef")
                    nc.vector.tensor_mul(
                        coef[:], e_bf[:], r[:].to_broadcast([P, K]))
                    tmp2 = cv.tile([P, K * NH], bf16, tag="tmp2")
                    t23 = tmp2[:].rearrange("p (k j) -> p k j", k=K)
                    nc.vector.tensor_mul(
                        t23, hs3[:, :, 0:16],
                        coef[:][:, :, None].to_broadcast([P, K, 16]))
                    h2v = h_sb.ap()[:, t * NH:(t + 1) * NH]
                    nc.vector.reduce_sum(
                        h2v,
                        tmp2[:].rearrange("p (k j) -> p j k", k=K),
                        axis=AX)
                    # += (exp(self)/s) * h_d
                    rs = cvs.tile([P, 1], f32, tag="rs")
                    nc.vector.tensor_mul(rs[:], es[:], r[:])
                    sh = cvs.tile([P, NH], f32, tag="sh")
                    nc.vector.tensor_mul(
                        sh[:], hd[:, 0:16], rs[:].to_broadcast([P, NH]))
                    nc.vector.tensor_add(h2v, h2v, sh[:])
        assert off == idxw

    if "G" in phases:
        # debug: dma_gather NCH chunks of table rows via the ant gather and
        # dump raw rows (as f32). G_MODE: dram (256B-strided) | sbuf
        import os
        NCH = int(os.environ.get("G_NCH", "49"))
        G_NINST = int(os.environ.get("G_NINST", "1"))
        G_MODE = os.environ.get("G_MODE", "dram")
        nidx = NCH * P
        widx = nidx // 16
        idx16_d = nc.dram_tensor("idx16g", [P, widx], mybir.dt.int16,
                                 kind="ExternalInput")
        with tile.TileContext(nc) as tc:
            if G_MODE == "dram":
                nc.sync.dma_start(out=tabA_w[:, 0:ROW], in_=tabA_f[:, :])
            else:
                # sbuf layout: row r -> partition r%128, stripe r//128
                load_table(tabA_f)
            with tc.tile_pool(name="dbg", bufs=1) as dbg:
                idx_sbg = dbg.tile([P, widx], mybir.dt.int16, tag="idx16")
                nc.sync.dma_start(out=idx_sbg[:], in_=idx16_d[:, :])
                hs = dbg.tile([P, NCH * ROW], bf16, tag="hs")
                nc.gpsimd.memset(hs[:], 0.0)
                if G_DO_GATHER:
                    for _ in range(G_NINST):
                        if G_MODE == "dram":
                            dma_gather_raw(
                                nc,
                                out_ap=hs[:].rearrange(
                                    "p (c j) -> p c j", j=ROW),
                                in_ap=tabA_w[:, 0:ROW], idxs_ap=idx_sbg[:],
                                num_idxs=nidx, elem_size=ROW,
                                stride_elems=128)
                        else:
                            dma_gather_raw(
                                nc,
                                out_ap=hs[:].rearrange(
                                    "p (c j) -> p c j", j=ROW),
                                in_ap=table_sb.ap()[:, 0:RPP * ROW],
                                idxs_ap=idx_sbg[:],
                                num_idxs=nidx, elem_size=ROW,
                                sbuf_tokens_per_rank=P,
                                sbuf_free_dim_per_rank=ROW * 2)
                ncols = min(NCH * ROW, 980)
                hf = dbg.tile([P, ncols], f32, tag="hf")
                nc.vector.tensor_copy(hf[:], hs[:, 0:ncols])
                ov = out_d.ap().rearrange("(p q) j -> p (q j)", p=P)
                nc.sync.dma_start(out=ov[:, 0:ncols], in_=hf[:])
        nc.compile()
        return nc

    if "B" in phases:
        conv(tabA_l, None)

    # ---------------- phase C: table B + AG2 -------------------------------
    if "C" in phases:
        with tile.TileContext(nc) as tc:
            epilogue_rows(tabB_l)
            allgather(tabB_l, tabB_f)
        with tile.TileContext(nc) as tc:
            load_table(tabB_f)

    if "D" in phases:
        conv(tabB_l, be_sb.ap()[:, 0:1])

    if "E" not in phases:
        # debug: dump h_sb (and inv_sb) into out
        with tile.TileContext(nc) as tc:
            ov = out_d.ap().rearrange("(p q) j -> p (q j)", p=P)
            nc.sync.dma_start(out=ov[:, 0:NT * NH], in_=h_sb.ap())
            nc.sync.dma_start(out=ov[:, NT * NH:NT * NH + NT], in_=inv_sb.ap())
        nc.compile()
        return nc

    with tile.TileContext(nc) as tc:
        with tc.tile_pool(name="hd", bufs=1) as hp, \
             tc.tile_pool(name="hdp", bufs=4, space="PSUM") as hpp:
            hb = hp.tile([P, NT * NH], bf16, tag="hb")
            nc.vector.tensor_copy(hb[:], h_sb.ap())
            h3t = hp.tile([NH, NT * P], bf16, tag="h3t")
            for t in range(NT):
                pst = hpp.tile([NH, P], bf16, tag="pst")
                nc.tensor.transpose(
                    out=pst[:], in_=hb[:, t * NH:(t + 1) * NH],
                    identity=ident_b.ap())
                nc.vector.tensor_copy(h3t[:, t * P:(t + 1) * P], pst[:])
            w4b = hp.tile([NH, NCO], bf16, tag="w4b")
            nc.vector.tensor_copy(w4b[:], w4_sb.ap())
            lg = hp.tile([P, NT * NCO], f32, tag="lg")
            for t in range(NT):
                psl = hpp.tile([P, NCO], f32, tag="psl")
                nc.tensor.matmul(
                    out=psl[:], lhsT=h3t[:, t * P:(t + 1) * P], rhs=w4b[:],
                    start=True, stop=True)
                nc.vector.tensor_add(
                    lg[:, t * NCO:(t + 1) * NCO], psl[:], b4_sb.ap())
            lg3 = lg[:].rearrange("p (t j) -> p t j", t=NT)
            nm = hp.tile([P, NT], f32, tag="hnm")
            nc.vector.reduce_max(nm[:], lg3, axis=AX, negate=True)
            nc.vector.tensor_add(
                lg3, lg3, nm[:][:, :, None].to_broadcast([P, NT, NCO]))
            ex = hp.tile([P, NT * NCO], f32, tag="ex")
            nc.scalar.activation(ex[:], lg[:], AF.Exp)
            s = hp.tile([P, NT], f32, tag="hs_sum")
            nc.vector.reduce_sum(
                s[:], ex[:].rearrange("p (t j) -> p t j", t=NT), axis=AX)
            ls = hp.tile([P, NT], f32, tag="ls")
            nc.scalar.activation(ls[:], s[:], AF.Ln)
            nc.vector.tensor_sub(
                lg3, lg3, ls[:][:, :, None].to_broadcast([P, NT, NCO]))
            nc.sync.dma_start(
                out=out_d[:, :].rearrange("(t p) j -> p t j", p=P),
                in_=lg3)

    nc.compile()
    return nc


# --------------------------------------------------------------------------
# entry point
# --------------------------------------------------------------------------

def run(cfg: Cfg, inputs: dict, trace: bool = False):
    from concourse import bass_utils

    x = np.asarray(inputs["x"], np.float32)
    edge_index = np.asarray(inputs["edge_index"])
    W1 = np.asarray(inputs["W1"], np.float32)
    b1 = np.asarray(inputs["b1"], np.float32)
    W4 = np.asarray(inputs["W4"], np.float32)
    b4 = np.asarray(inputs["b4"], np.float32)
    beta3 = np.asarray(inputs["beta3"], np.float32)

    import ml_dtypes

    per_core, kab, order_c = preprocess(cfg, x, edge_index)
    nc = build_kernel(cfg, kab)

    P, KC, NH = cfg.P, cfg.kchunks, cfg.nh
    w1p = np.zeros((cfg.f_pad, NH), np.float32)
    w1p[:cfg.f_in] = W1
    w1p = np.ascontiguousarray(
        w1p.reshape(KC, P, NH).transpose(1, 0, 2).reshape(P, KC * NH)
    ).astype(ml_dtypes.bfloat16)
    b1r = np.ascontiguousarray(np.broadcast_to(b1[None, :], (P, NH)))
    b4r = np.ascontiguousarray(np.broadcast_to(b4[None, :], (P, cfg.nc_out)))
    ber = np.ascontiguousarray(np.broadcast_to(beta3[None, :], (P, 1)))

    in_maps = []
    for c in range(cfg.n_cores):
        in_maps.append({
            "xt": per_core[c]["xt"].astype(ml_dtypes.bfloat16),
            "idx16": per_core[c]["idx16"],
            "mask": per_core[c]["mask"].astype(ml_dtypes.bfloat16),
            "w1p": w1p, "b1r": b1r, "w4r": np.ascontiguousarray(W4),
            "b4r": b4r, "beta3r": ber,
        })

    res = bass_utils.run_bass_kernel_spmd(
        nc, in_maps, core_ids=list(range(cfg.n_cores)), trace=trace)

    out = np.empty((cfg.n_nodes, cfg.nc_out), np.float32)
    for c in range(cfg.n_cores):
        oc = np.asarray(res.results[c]["out"])[:cfg.npc_raw]
        out[c * cfg.npc_raw + order_c[c]] = oc
    return out, res


def kernel(**inputs) -> np.ndarray:
    out, _ = run(FULL, inputs, trace=False)
    return out


# revision 60
# speedup vs baseline: 1.3889x; 1.0219x over previous
"""AGNNet (2-layer AGNN conv + linear head) distributed over 8 trn2 NeuronCores.

Strategy (graph/data parallel, per sharding hint):
  - nodes sharded by dst range: core c owns nodes [c*6250, (c+1)*6250)
  - host groups edges by dst, degree-sorts each core's local nodes (undone on
    output), drops self-loops (handled analytically in the softmax), and packs
    each 128-node tile's in-edge lists into per-tile slot columns
  - device: L1 matmul from host pre-blocked x shard; build a bf16 row table
    [h(16), inv_norm, 0, 0] per node; AllGather the packed table (2MB);
    redistribute it into SBUF (row r -> partition r%128, stripe r//128,
    15.7KB/partition); per node-tile one SBUF-source dma_gather (ant extended
    Q7 kernel, ~7.7ns/idx -- the Q7 descriptor-generation rate is THE
    bottleneck of this problem; indirect1d would cost ~11ns/edge) per index
    window fetches all neighbor rows; DVE softmax (no max-trick needed:
    |alpha| <= |beta|); the self-loop term exp(beta*cos(h,h)) is added
    analytically from inv_norm alone; second conv identical; bf16 head
    matmul + log_softmax.
  - dma_gather indices are int16, so the 50176-row table is addressed through
    THREE overlapping 32768-row windows (bases 0/8704/17408 as free-dim
    offsets into the SBUF table); the host balances each node's slots across
    windows per tile (~+7%% padding); padding slots point at the window base
    row and are killed by a -1e30 static per-slot mask added to alpha.
"""

import math
from contextlib import ExitStack
from dataclasses import dataclass

import numpy as np


@dataclass
class Cfg:
    n_cores: int = 8
    n_nodes: int = 50000
    f_in: int = 2000
    nh: int = 16
    nc_out: int = 20
    row: int = 20           # table row elems: 16 h, [16]=invn, [17]=bias, 18-19 pad
    wrow: int = 128         # wide-table row elems (256B stride for dma_gather)
    P: int = 128
    win: int = 32768        # dma_gather int16 index window (rows per base)

    @property
    def npc_raw(self) -> int:       # real nodes per core
        return self.n_nodes // self.n_cores

    @property
    def npc(self) -> int:           # padded nodes per core (mult of 128)
        return ((self.npc_raw + self.P - 1) // self.P) * self.P

    @property
    def ntiles(self) -> int:
        return self.npc // self.P

    @property
    def kchunks(self) -> int:       # 128-row chunks of the padded f_in
        return (self.f_in + self.P - 1) // self.P

    @property
    def f_pad(self) -> int:
        return self.kchunks * self.P

    @property
    def nfull(self) -> int:
        return self.n_cores * self.npc

    @property
    def wb(self) -> int:            # last window base
        return self.nfull - self.win

    @property
    def wbases(self) -> tuple:      # 3 overlapping 32768-row window bases
        return (0, self.wb // 2, self.wb)


FULL = Cfg()
NEG_BIG = -1.0e30
G_DO_GATHER = True
GRP = 4          # tiles per merged gather group


def pack_idx16(idx_mat: np.ndarray) -> np.ndarray:
    """[128 partitions, nch chunks] window-relative indices -> dma_gather
    int16 index tile [128, nch*8]: list position c*128+j lands on partition j
    chunk c (the ucode lane swizzle applies to both the index read and the
    dst partition, so it cancels); the flat list is wrapped over 16
    partitions and replicated across the 8 16-partition groups."""
    P, nch = idx_mat.shape
    assert P == 128
    flat = np.ascontiguousarray(idx_mat.T).reshape(-1).astype(np.int16)
    tile = np.empty((128, nch * 8), np.int16)
    for p in range(16):
        tile[p] = flat[p::16]
    tile[16:] = np.tile(tile[:16], (7, 1))
    return np.ascontiguousarray(tile)


def dma_gather_raw(nc, out_ap, in_ap, idxs_ap, num_idxs: int,
                   elem_size: int, stride_elems: int = 0,
                   sbuf_tokens_per_rank: int = 0,
                   sbuf_free_dim_per_rank: int = 0):
    """nc.gpsimd.dma_gather minus the elem_size%256 assert (non-transpose
    path in the Q7 ucode has no such restriction; only the DRAM stride must
    be a multiple of 256B) and minus the SBUF-source transpose-only assert
    (gen_descs handles src_is_sbuf in the plain path too). For SBUF source:
    idx i reads partition i % tokens_per_rank, free-byte offset
    (i // tokens_per_rank) * free_dim_per_rank. single_packet=False: a
    single packet is capped at 64 descriptors, large gathers hang with it."""
    from concourse import mybir
    g = nc.gpsimd
    src_is_sbuf = sbuf_tokens_per_rank > 0
    if src_is_sbuf:
        stride_bytes_256 = 0
        if in_ap.dtype != out_ap.dtype:
            in_ap = in_ap.bitcast(out_ap.dtype)
        _in_ap = [g.lower_ap(in_ap)]
    else:
        stride_bytes = stride_elems * mybir.dt.size(in_ap.dtype)
        assert stride_bytes % 256 == 0 and stride_bytes // 256 < 256
        stride_bytes_256 = stride_bytes // 256
        _in_ap = g.lower_ap_dma(in_ap, for_custom_bir_dma=True)
    _idxs_ap = g.lower_ap(idxs_ap)
    _out_ap = g.lower_ap(out_ap)
    return g.add_instruction(
        mybir.InstDMAGatherAnt(
            name=g.bass.get_next_instruction_name(),
            ins=[*_in_ap, _idxs_ap, g.lower_val_access(g.to_reg(num_idxs))],
            outs=[_out_ap],
            transpose=False,
            num_idxs=num_idxs,
            elem_size=elem_size,
            stride_bytes_256=stride_bytes_256,
            gen_mode=0,
            single_packet=False,
            queue_num=0,
            sbuf_tokens_per_rank=sbuf_tokens_per_rank,
            sbuf_free_dim_per_rank=sbuf_free_dim_per_rank,
            sbuf_free_dim_pad_per_rank=0,
            sbuf_byte_offset=0,
        ))


# --------------------------------------------------------------------------
# host-side preprocessing
# --------------------------------------------------------------------------

def preprocess(cfg: Cfg, x: np.ndarray, edge_index: np.ndarray):
    """Shard + layout transform.

    Returns (per_core list of dicts, kab, order_c) where kab is the shared
    [(K1_t, K2_t, K3_t)] per tile (same across cores for SPMD): slot columns
    per index window (3 overlapping 32768-row windows of the wide table)."""
    P, NPC, NPCR = cfg.P, cfg.npc, cfg.npc_raw
    n, C, NT = cfg.n_nodes, cfg.n_cores, cfg.ntiles
    W = cfg.win
    B1, B2, B3 = cfg.wbases

    src = edge_index[0].astype(np.int64)
    dst = edge_index[1].astype(np.int64)
    keep = src != dst                     # self-loops handled analytically
    src, dst = src[keep], dst[keep]

    core_of_dst = dst // NPCR

    # pass 1: per-core degree sort -> rank of each node within its core
    order_c, rank_c = [], []
    for c in range(C):
        m = core_of_dst == c
        ld = dst[m] - c * NPCR
        deg = np.bincount(ld, minlength=NPCR)
        order = np.argsort(deg, kind="stable")      # ascending degree
        rank = np.empty(NPCR, np.int64)
        rank[order] = np.arange(NPCR)
        order_c.append(order)
        rank_c.append(rank)

    # new global id after per-core permutation + padding (= wide-table row)
    new_gid = np.empty(n, np.int64)
    for c in range(C):
        new_gid[c * NPCR:(c + 1) * NPCR] = c * NPC + rank_c[c]

    # per-core per-node edge lists sorted by wide-row
    per_core_lists = []
    for c in range(C):
        m = core_of_dst == c
        ld = dst[m] - c * NPCR
        gs = new_gid[src[m]]
        er = rank_c[c][ld]                       # dst rank of each edge
        eo = np.lexsort((gs, er))
        er_s = er[eo]
        gs_s = gs[eo]
        starts = np.zeros(NPC + 1, np.int64)
        np.cumsum(np.bincount(er_s, minlength=NPC), out=starts[1:])
        per_core_lists.append((starts, gs_s))

    # per-node class counts: s1 [0,B2) strictly-W1; f12 [B2,B3); f123
    # [B3, W) (in all); f23 [W, B2+W); s3 [B2+W, nfull)
    bnds = np.array([B2, B3, W, B2 + W], np.int64)
    cnts = np.zeros((C, 5, NPC), np.int64)
    degp_all = np.zeros((C, NPC), np.int64)
    for c in range(C):
        starts, gs_s = per_core_lists[c]
        d_cnt = np.diff(starts)
        degp_all[c] = d_cnt
        er_of_edge = np.repeat(np.arange(NPC), d_cnt)
        cls = np.searchsorted(bnds, gs_s, side="right")
        for k in range(5):
            cnts[c, k] = np.bincount(
                er_of_edge, weights=(cls == k), minlength=NPC)

    def split_node(T, s1, f12, f123, f23, s3, deg):
        """-> (n1, n3) target-T balanced; n2 = deg - n1 - n3 (all W2-ok)."""
        n1 = np.maximum(s1, np.minimum(T, s1 + f12 + f123))
        f123_left = f123 - np.maximum(0, n1 - (s1 + f12))
        n3 = np.maximum(s3, np.minimum(T, s3 + f23 + f123_left))
        return n1, n3

    kab = []
    for t in range(NT):
        s, e = t * P, (t + 1) * P
        degs = degp_all[:, s:e]
        s1, f12, f123, f23, s3 = (cnts[:, k, s:e] for k in range(5))
        kmax = int(degs.max())
        best = None
        for T in range(kmax + 1):
            n1, n3 = split_node(T, s1, f12, f123, f23, s3, degs)
            K1 = int(n1.max())
            K3 = int(n3.max())
            K2 = int((degs - n1 - n3).max())
            if best is None or K1 + K2 + K3 < best[0]:
                best = (K1 + K2 + K3, T, K1, K2, K3)
        _, T, K1, K2, K3 = best
        kab.append((max(K1, 1), max(K2, 1), max(K3, 1), T))

    # build per-core idx16 + per-slot mask inputs (pad slots point at the
    # window base row -- a real row -- and are killed by a -1e30 mask).
    # Gathers are merged per GRP-tile group: one instruction per window,
    # slot layout [W1 t0..t3 | W2 t0..t3 | W3 t0..t3].
    per_core = []
    for c in range(C):
        starts, gs_s = per_core_lists[c]
        tiles = []
        for t in range(NT):
            K1, K2, K3, T = kab[t]
            idx1 = np.zeros((P, K1), np.int64)
            idx2 = np.zeros((P, K2), np.int64)
            idx3 = np.zeros((P, K3), np.int64)
            mask = np.full((P, K1 + K2 + K3), NEG_BIG, np.float32)
            sl = slice(t * P, (t + 1) * P)
            s1, f12, f123, f23, s3 = (cnts[c, k, sl] for k in range(5))
            degs = degp_all[c, sl]
            n1, n3 = split_node(T, s1, f12, f123, f23, s3, degs)
            n1 = np.minimum(n1, K1)
            n3 = np.minimum(n3, K3)
            for p in range(P):
                node = t * P + p
                lst = gs_s[starts[node]:starts[node + 1]]  # sorted by wrow
                a = int(n1[p]); b = int(n3[p])
                w1 = lst[:a]                       # s1 + leading f12/f123
                rest = lst[a:]
                w3 = rest[rest.size - b:] if b else rest[:0]
                w2 = rest[:rest.size - b]
                idx1[p, :a] = w1 - B1
                idx2[p, :w2.size] = w2 - B2
                idx3[p, :b] = w3 - B3
                mask[p, :a] = 0.0
                mask[p, K1:K1 + w2.size] = 0.0
                mask[p, K1 + K2:K1 + K2 + b] = 0.0
            for arr in (idx1, idx2, idx3):
                assert arr.min() >= 0 and arr.max() < W
            tiles.append((idx1, idx2, idx3, mask[:, :K1],
                          mask[:, K1:K1 + K2], mask[:, K1 + K2:]))
        packs = []
        masks = []
        for g0 in range(0, NT, GRP):
            grp = range(g0, min(g0 + GRP, NT))
            for w in range(3):
                packs.append(pack_idx16(np.concatenate(
                    [tiles[t][w] for t in grp], axis=1)))
                masks.append(np.concatenate(
                    [tiles[t][3 + w] for t in grp], axis=1))
        idx16 = np.concatenate(packs, axis=1)
        maskv = np.concatenate(masks, axis=1)

        # x shard: permuted, padded, transposed, f-padded, then pre-blocked
        # per tile as [t][p][chunk][m] so each tile's load is one contiguous
        # 4KB-per-partition DMA: xt2[t*128+p, c*128+m] = x^T[c*128+p, t*128+m]
        xs = x[c * NPCR:(c + 1) * NPCR][order_c[c]]      # [NPCR, f_in]
        xt = np.zeros((cfg.f_pad, NPC), np.float32)
        xt[:cfg.f_in, :NPCR] = xs.T
        KCh = cfg.kchunks
        xt2 = (xt.reshape(KCh, P, NT, P)
               .transpose(2, 1, 0, 3)                    # [t, p, c, m]
               .reshape(NT * P, KCh * P))
        per_core.append({"xt": np.ascontiguousarray(xt2), "idx16": idx16,
                         "mask": maskv})

    return per_core, kab, order_c


# --------------------------------------------------------------------------
# device kernel builder
# --------------------------------------------------------------------------

def build_kernel(cfg: Cfg, kab, phases: str = "ABCDE"):
    import concourse.bacc as bacc
    import concourse.tile as tile
    from concourse import bass, mybir
    from concourse.masks import make_identity

    P = cfg.P
    NH, NCO, ROW, WROW = cfg.nh, cfg.nc_out, cfg.row, cfg.wrow
    NPC, NT, KC = cfg.npc, cfg.ntiles, cfg.kchunks
    NFULL, WB, W = cfg.nfull, cfg.wb, cfg.win
    f32 = mybir.dt.float32
    bf16 = mybir.dt.bfloat16
    i16 = mybir.dt.int16
    AX = mybir.AxisListType.X
    OP = mybir.AluOpType
    AF = mybir.ActivationFunctionType
    idxw = sum(8 * (k1 + k2 + k3) for (k1, k2, k3, _) in kab)
    sumk = idxw // 8
    WBASES = cfg.wbases
    RPP = NFULL // P                # wide-table rows per partition (bounce)

    nc = bacc.Bacc("TRN2", target_bir_lowering=False, debug=False,
                   num_devices=cfg.n_cores)

    xt_d = nc.dram_tensor("xt", [NT * P, KC * P], bf16, kind="ExternalInput")
    idx_d = nc.dram_tensor("idx16", [P, idxw], i16, kind="ExternalInput")
    mask_d = nc.dram_tensor("mask", [P, sumk], bf16, kind="ExternalInput")
    w1_d = nc.dram_tensor("w1p", [P, KC * NH], bf16, kind="ExternalInput")
    b1_d = nc.dram_tensor("b1r", [P, NH], f32, kind="ExternalInput")
    w4_d = nc.dram_tensor("w4r", [NH, NCO], f32, kind="ExternalInput")
    b4_d = nc.dram_tensor("b4r", [P, NCO], f32, kind="ExternalInput")
    be_d = nc.dram_tensor("beta3r", [P, 1], f32, kind="ExternalInput")
    out_d = nc.dram_tensor("out", [NPC, NCO], f32, kind="ExternalOutput")

    tabA_l = nc.dram_tensor("tabA_l", [NPC, ROW], bf16)
    tabA_f = nc.dram_tensor("tabA_f", [NFULL, ROW], bf16, addr_space="Shared")
    tabB_l = nc.dram_tensor("tabB_l", [NPC, ROW], bf16)
    tabB_f = nc.dram_tensor("tabB_f", [NFULL, ROW], bf16, addr_space="Shared")
    tabA_w = nc.dram_tensor("tabA_w", [NFULL, WROW], bf16)  # G-phase only

    # persistent SBUF (survives across TileContexts)
    def sb(name, shape, dtype):
        return nc.alloc_sbuf_tensor(name, list(shape), dtype)

    ident_b = sb("identb", [P, P], bf16)
    w1_sb = sb("w1sb", [P, KC * NH], bf16)
    b1_sb = sb("b1sb", [P, NH], f32)
    w4_sb = sb("w4sb", [NH, NCO], f32)
    b4_sb = sb("b4sb", [P, NCO], f32)
    be_sb = sb("besb", [P, 1], f32)
    bee_sb = sb("beesb", [P, 1], f32)      # exp-ready: beta3 value
    h_sb = sb("hsb", [P, NT * NH], f32)
    rows_sb = sb("rowssb", [P, NT * ROW], bf16)
    sq_sb = sb("sqsb", [P, NT * NH], f32)
    ss_sb = sb("sssb", [P, NT], f32)
    inv_sb = sb("invsb", [P, NT], f32)
    idx_sb = sb("idxsb", [P, idxw], i16)    # all tiles' gather indices
    mask_sb = sb("masksb", [P, sumk], bf16)  # per-slot pad masks
    # SBUF-resident gather table: row r at partition r%128, stripe r//128
    table_sb = sb("tablesb", [P, RPP * ROW], bf16)

    def epilogue_rows(tab_local):
        """h_sb -> inv norm -> rows_sb -> DMA to tab_local."""
        h3v = h_sb.ap().rearrange("p (t j) -> p t j", t=NT)
        nc.vector.tensor_mul(sq_sb.ap(), h_sb.ap(), h_sb.ap())
        nc.vector.reduce_sum(
            ss_sb.ap(), sq_sb.ap().rearrange("p (t j) -> p t j", t=NT), axis=AX)
        nc.scalar.sqrt(ss_sb.ap(), ss_sb.ap())
        nc.vector.tensor_scalar_add(ss_sb.ap(), ss_sb.ap(), 1.0e-12)
        nc.vector.reciprocal(inv_sb.ap(), ss_sb.ap())
        rv = rows_sb.ap().rearrange("p (t j) -> p t j", t=NT)
        nc.vector.tensor_copy(rv[:, :, 0:16], h3v)
        nc.vector.tensor_copy(rv[:, :, 16], inv_sb.ap())
        nc.sync.dma_start(
            out=tab_local[:, :].rearrange("(t p) j -> p t j", p=P),
            in_=rv)

    def allgather(tab_local, tab_full):
        nc.gpsimd.collective_compute(
            "AllGather", OP.bypass,
            replica_groups=[list(range(cfg.n_cores))],
            ins=[tab_local.ap().opt()],
            outs=[tab_full.ap()[0:NFULL, :].opt()])

    def load_table(tab_full):
        """packed DRAM [NFULL, 20] -> SBUF table (row r at partition r%128,
        stripe r//128). SWDGE regular-pattern DMAs (~0.34ns/desc), chunked
        under the 16384-descriptor carveout limit."""
        tv = table_sb.ap().rearrange("p (r j) -> p r j", j=ROW)
        src = tab_full[:, :].rearrange("(r p) j -> p r j", p=P)
        CH = 56          # stripes per chunk: 2 chunks fit the 16k-desc ring,
        for o in range(0, RPP, CH):  # so generation overlaps drain
            n = min(CH, RPP - o)
            nc.gpsimd.dma_start(out=tv[:, o:o + n, :], in_=src[:, o:o + n, :])

    # ---------------- phase A: consts, idx preload, L1, table A, AG1 -------
    with tile.TileContext(nc) as tc:
        make_identity(nc, ident_b.ap())
        nc.gpsimd.memset(rows_sb.ap(), 0.0)
        nc.sync.dma_start(out=idx_sb.ap(), in_=idx_d[:, :])
        nc.sync.dma_start(out=mask_sb.ap(), in_=mask_d[:, :])
        nc.sync.dma_start(out=w1_sb.ap(), in_=w1_d[:, :])
        nc.sync.dma_start(out=b1_sb.ap(), in_=b1_d[:, :])
        nc.sync.dma_start(out=w4_sb.ap(), in_=w4_d[:, :])
        nc.sync.dma_start(out=b4_sb.ap(), in_=b4_d[:, :])
        nc.sync.dma_start(out=be_sb.ap(), in_=be_d[:, :])

        with tc.tile_pool(name="l1x", bufs=3) as xp, \
             tc.tile_pool(name="l1p", bufs=4, space="PSUM") as pp:
            for t in range(NT):
                xw = xp.tile([P, KC * P], bf16, tag="xw")
                nc.sync.dma_start(
                    out=xw[:], in_=xt_d[t * P:(t + 1) * P, :])
                ps = pp.tile([P, NH], f32, tag="l1ps")
                for c in range(KC):
                    nc.tensor.matmul(
                        out=ps[:], lhsT=xw[:, c * P:(c + 1) * P],
                        rhs=w1_sb.ap()[:, c * NH:(c + 1) * NH],
                        start=(c == 0), stop=(c == KC - 1))
                hsl = h_sb.ap()[:, t * NH:(t + 1) * NH]
                nc.vector.tensor_add(hsl, ps[:], b1_sb.ap())
                nc.vector.tensor_scalar_max(hsl, hsl, 0.0)
        epilogue_rows(tabA_l)
        allgather(tabA_l, tabA_f)
    with tile.TileContext(nc) as tc:
        load_table(tabA_f)

    # ---------------- conv layer -------------------------------------------
    def conv(tab_local, beta_ap):
        """beta_ap: None for beta=1 (conv2), else [P,1] AP with beta value.

        One dma_gather per (GRP-tile group, window); slot layout per group =
        [W1 t0..t3 | W2 t0..t3 | W3 t0..t3]; DVE ops run per ragged
        (tile, window) column range."""
        off = 0
        moff = 0
        with tile.TileContext(nc) as tc:
            with tc.tile_pool(name="cv", bufs=3) as cv, \
                 tc.tile_pool(name="cvs", bufs=2) as cvs:
                for g0 in range(0, NT, GRP):
                    grp = list(range(g0, min(g0 + GRP, NT)))
                    gsz = len(grp)
                    kws = [kab[t][:3] for t in grp]
                    swid = [sum(k[w] for k in kws) for w in range(3)]
                    K = sum(swid)
                    hs = cv.tile([P, K * ROW], bf16, tag="hs")
                    boff = [0, swid[0], swid[0] + swid[1]]
                    for w, base in enumerate(WBASES):
                        s0 = (base // P) * ROW
                        dma_gather_raw(
                            nc,
                            out_ap=hs[:, boff[w] * ROW:
                                      (boff[w] + swid[w]) * ROW]
                            .rearrange("p (c j) -> p c j", j=ROW),
                            in_ap=table_sb.ap()[:, s0:s0 + 256 * ROW],
                            idxs_ap=idx_sb.ap()[:, off:off + 8 * swid[w]],
                            num_idxs=P * swid[w], elem_size=ROW,
                            sbuf_tokens_per_rank=P,
                            sbuf_free_dim_per_rank=ROW * 2)
                        off += 8 * swid[w]

                    hd = cvs.tile([P, gsz * ROW], bf16, tag="hd")
                    nc.sync.dma_start(
                        out=hd[:].rearrange("p (g j) -> p g j", g=gsz),
                        in_=tab_local[g0 * P:(g0 + gsz) * P, :]
                        .rearrange("(g p) j -> p g j", p=P))
                    hd3 = hd[:].rearrange("p (g j) -> p g j", g=gsz)

                    # ragged (tile-in-group, column start, width) ranges in
                    # hs/alpha order
                    rng = []
                    for w in range(3):
                        c = boff[w]
                        for ti in range(gsz):
                            rng.append((ti, c, kws[ti][w]))
                            c += kws[ti][w]

                    invd = cvs.tile([P, gsz], f32, tag="invd")
                    if beta_ap is None:
                        nc.vector.tensor_copy(invd[:], hd3[:, :, 16])
                    else:
                        nc.vector.tensor_mul(
                            invd[:], hd3[:, :, 16],
                            beta_ap.to_broadcast([P, gsz]))

                    alpha = cv.tile([P, K], f32, tag="alpha")
                    tmp = cv.tile([P, K * NH], bf16, tag="tmp")
                    for (ti, c0, kw) in rng:
                        hsw = hs[:, c0 * ROW:(c0 + kw) * ROW].rearrange(
                            "p (k j) -> p k j", k=kw)
                        tm = tmp[:, c0 * NH:(c0 + kw) * NH].rearrange(
                            "p (k j) -> p k j", k=kw)
                        hdr = hd[:, ti * ROW:ti * ROW + 16]
                        nc.vector.tensor_mul(
                            tm, hsw[:, :, 0:16],
                            hdr[:, None, :].to_broadcast([P, kw, 16]))
                        asl = alpha[:, c0:c0 + kw]
                        nc.vector.reduce_sum(
                            asl, tmp[:, c0 * NH:(c0 + kw) * NH].rearrange(
                                "p (k j) -> p k j", j=NH), axis=AX)
                        nc.vector.tensor_mul(asl, asl, hsw[:, :, 16])
                        nc.vector.tensor_mul(
                            asl, asl,
                            invd[:, ti:ti + 1].to_broadcast([P, kw]))
                    # pad slots killed by the -1e30 static mask (group-wide)
                    nc.vector.tensor_add(
                        alpha[:], alpha[:], mask_sb.ap()[:, moff:moff + K])
                    moff += K
                    # |alpha| <= |beta| for real slots: exp without max-trick
                    e_bf = cv.tile([P, K], bf16, tag="e")
                    nc.scalar.activation(e_bf[:], alpha[:], AF.Exp)
                    # per-(tile,window) partial sums -> s [P, gsz]
                    se = cvs.tile([P, gsz * 3], f32, tag="se")
                    for i, (ti, c0, kw) in enumerate(rng):
                        w = i // gsz
                        nc.vector.reduce_sum(
                            se[:, ti * 3 + w:ti * 3 + w + 1],
                            e_bf[:, None, c0:c0 + kw], axis=AX)
                    s = cvs.tile([P, gsz], f32, tag="s")
                    nc.vector.reduce_sum(
                        s[:], se[:].rearrange("p (g w) -> p g w", g=gsz),
                        axis=AX)
                    # analytic self-loop: cos(h,h) = (1 - 1e-12*invn)^2
                    selfa = cvs.tile([P, gsz], f32, tag="selfa")
                    nc.vector.tensor_scalar(
                        selfa[:], hd3[:, :, 16], -1.0e-12, 1.0,
                        op0=OP.mult, op1=OP.add)
                    nc.vector.tensor_mul(selfa[:], selfa[:], selfa[:])
                    if beta_ap is not None:
                        nc.vector.tensor_mul(
                            selfa[:], selfa[:],
                            beta_ap.to_broadcast([P, gsz]))
                    es = cvs.tile([P, gsz], f32, tag="es")
                    nc.scalar.activation(es[:], selfa[:], AF.Exp)
                    nc.vector.tensor_add(s[:], s[:], es[:])
                    nc.vector.tensor_scalar_add(s[:], s[:], 1.0e-16)
                    r = cvs.tile([P, gsz], f32, tag="r")
                    nc.vector.reciprocal(r[:], s[:])
                    coef = cv.tile([P, K], bf16, tag="coef")
                    tmp2 = cv.tile([P, K * NH], bf16, tag="tmp2")
                    hp = cvs.tile([P, NH], f32, tag="hp")
                    for i, (ti, c0, kw) in enumerate(rng):
                        hsw = hs[:, c0 * ROW:(c0 + kw) * ROW].rearrange(
                            "p (k j) -> p k j", k=kw)
                        csl = coef[:, c0:c0 + kw]
                        nc.vector.tensor_mul(
                            csl, e_bf[:, c0:c0 + kw],
                            r[:, ti:ti + 1].to_broadcast([P, kw]))
                        t23 = tmp2[:, c0 * NH:(c0 + kw) * NH].rearrange(
                            "p (k j) -> p k j", k=kw)
                        nc.vector.tensor_mul(
                            t23, hsw[:, :, 0:16],
                            csl[:, :, None].to_broadcast([P, kw, 16]))
                        h2v = h_sb.ap()[:, grp[ti] * NH:(grp[ti] + 1) * NH]
                        if i < gsz:        # first window block for this tile
                            nc.vector.reduce_sum(
                                h2v, tmp2[:, c0 * NH:(c0 + kw) * NH]
                                .rearrange("p (k j) -> p j k", k=kw), axis=AX)
                        else:
                            nc.vector.reduce_sum(
                                hp[:], tmp2[:, c0 * NH:(c0 + kw) * NH]
                                .rearrange("p (k j) -> p j k", k=kw), axis=AX)
                            nc.vector.tensor_add(h2v, h2v, hp[:])
                    # += (exp(self)/s) * h_d  (group-wide)
                    rs = cvs.tile([P, gsz], f32, tag="rs")
                    nc.vector.tensor_mul(rs[:], es[:], r[:])
                    sh = cvs.tile([P, gsz * NH], f32, tag="sh")
                    nc.vector.tensor_mul(
                        sh[:].rearrange("p (g j) -> p g j", g=gsz),
                        hd3[:, :, 0:16],
                        rs[:][:, :, None].to_broadcast([P, gsz, NH]))
                    h2g = h_sb.ap()[:, g0 * NH:(g0 + gsz) * NH]
                    nc.vector.tensor_add(h2g, h2g, sh[:])
        assert off == idxw

    if "G" in phases:
        # debug: dma_gather NCH chunks of table rows via the ant gather and
        # dump raw rows (as f32). G_MODE: dram (256B-strided) | sbuf
        import os
        NCH = int(os.environ.get("G_NCH", "49"))
        G_NINST = int(os.environ.get("G_NINST", "1"))
        G_MODE = os.environ.get("G_MODE", "dram")
        nidx = NCH * P
        widx = nidx // 16
        idx16_d = nc.dram_tensor("idx16g", [P, widx], mybir.dt.int16,
                                 kind="ExternalInput")
        with tile.TileContext(nc) as tc:
            if G_MODE == "dram":
                nc.sync.dma_start(out=tabA_w[:, 0:ROW], in_=tabA_f[:, :])
            else:
                # sbuf layout: row r -> partition r%128, stripe r//128
                load_table(tabA_f)
            with tc.tile_pool(name="dbg", bufs=1) as dbg:
                idx_sbg = dbg.tile([P, widx], mybir.dt.int16, tag="idx16")
                nc.sync.dma_start(out=idx_sbg[:], in_=idx16_d[:, :])
                hs = dbg.tile([P, NCH * ROW], bf16, tag="hs")
                nc.gpsimd.memset(hs[:], 0.0)
                if G_DO_GATHER:
                    for _ in range(G_NINST):
                        if G_MODE == "dram":
                            dma_gather_raw(
                                nc,
                                out_ap=hs[:].rearrange(
                                    "p (c j) -> p c j", j=ROW),
                                in_ap=tabA_w[:, 0:ROW], idxs_ap=idx_sbg[:],
                                num_idxs=nidx, elem_size=ROW,
                                stride_elems=128)
                        else:
                            dma_gather_raw(
                                nc,
                                out_ap=hs[:].rearrange(
                                    "p (c j) -> p c j", j=ROW),
                                in_ap=table_sb.ap()[:, 0:RPP * ROW],
                                idxs_ap=idx_sbg[:],
                                num_idxs=nidx, elem_size=ROW,
                                sbuf_tokens_per_rank=P,
                                sbuf_free_dim_per_rank=ROW * 2)
                ncols = min(NCH * ROW, 980)
                hf = dbg.tile([P, ncols], f32, tag="hf")
                nc.vector.tensor_copy(hf[:], hs[:, 0:ncols])
                ov = out_d.ap().rearrange("(p q) j -> p (q j)", p=P)
                nc.sync.dma_start(out=ov[:, 0:ncols], in_=hf[:])
        nc.compile()
        return nc

    if "B" in phases:
        conv(tabA_l, None)

    # ---------------- phase C: table B + AG2 -------------------------------
    if "C" in phases:
        with tile.TileContext(nc) as tc:
            epilogue_rows(tabB_l)
            allgather(tabB_l, tabB_f)
        with tile.TileContext(nc) as tc:
            load_table(tabB_f)

    if "D" in phases:
        conv(tabB_l, be_sb.ap()[:, 0:1])

    if "E" not in phases:
        # debug: dump h_sb (and inv_sb) into out
        with tile.TileContext(nc) as tc:
            ov = out_d.ap().rearrange("(p q) j -> p (q j)", p=P)
            nc.sync.dma_start(out=ov[:, 0:NT * NH], in_=h_sb.ap())
            nc.sync.dma_start(out=ov[:, NT * NH:NT * NH + NT], in_=inv_sb.ap())
        nc.compile()
        return nc

    with tile.TileContext(nc) as tc:
        with tc.tile_pool(name="hd", bufs=1) as hp, \
             tc.tile_pool(name="hdp", bufs=4, space="PSUM") as hpp:
            hb = hp.tile([P, NT * NH], bf16, tag="hb")
            nc.vector.tensor_copy(hb[:], h_sb.ap())
            h3t = hp.tile([NH, NT * P], bf16, tag="h3t")
            for t in range(NT):
                pst = hpp.tile([NH, P], bf16, tag="pst")
                nc.tensor.transpose(
                    out=pst[:], in_=hb[:, t * NH:(t + 1) * NH],
                    identity=ident_b.ap())
                nc.vector.tensor_copy(h3t[:, t * P:(t + 1) * P], pst[:])
            w4b = hp.tile([NH, NCO], bf16, tag="w4b")
            nc.vector.tensor_copy(w4b[:], w4_sb.ap())
            lg = hp.tile([P, NT * NCO], f32, tag="lg")
            for t in range(NT):
                psl = hpp.tile([P, NCO], f32, tag="psl")
                nc.tensor.matmul(
                    out=psl[:], lhsT=h3t[:, t * P:(t + 1) * P], rhs=w4b[:],
                    start=True, stop=True)
                nc.vector.tensor_add(
                    lg[:, t * NCO:(t + 1) * NCO], psl[:], b4_sb.ap())
            lg3 = lg[:].rearrange("p (t j) -> p t j", t=NT)
            nm = hp.tile([P, NT], f32, tag="hnm")
            nc.vector.reduce_max(nm[:], lg3, axis=AX, negate=True)
            nc.vector.tensor_add(
                lg3, lg3, nm[:][:, :, None].to_broadcast([P, NT, NCO]))
            ex = hp.tile([P, NT * NCO], f32, tag="ex")
            nc.scalar.activation(ex[:], lg[:], AF.Exp)
            s = hp.tile([P, NT], f32, tag="hs_sum")
            nc.vector.reduce_sum(
                s[:], ex[:].rearrange("p (t j) -> p t j", t=NT), axis=AX)
            ls = hp.tile([P, NT], f32, tag="ls")
            nc.scalar.activation(ls[:], s[:], AF.Ln)
            nc.vector.tensor_sub(
                lg3, lg3, ls[:][:, :, None].to_broadcast([P, NT, NCO]))
            nc.sync.dma_start(
                out=out_d[:, :].rearrange("(t p) j -> p t j", p=P),
                in_=lg3)

    nc.compile()
    return nc


# --------------------------------------------------------------------------
# entry point
# --------------------------------------------------------------------------

def run(cfg: Cfg, inputs: dict, trace: bool = False):
    from concourse import bass_utils

    x = np.asarray(inputs["x"], np.float32)
    edge_index = np.asarray(inputs["edge_index"])
    W1 = np.asarray(inputs["W1"], np.float32)
    b1 = np.asarray(inputs["b1"], np.float32)
    W4 = np.asarray(inputs["W4"], np.float32)
    b4 = np.asarray(inputs["b4"], np.float32)
    beta3 = np.asarray(inputs["beta3"], np.float32)

    import ml_dtypes

    per_core, kab, order_c = preprocess(cfg, x, edge_index)
    nc = build_kernel(cfg, kab)

    P, KC, NH = cfg.P, cfg.kchunks, cfg.nh
    w1p = np.zeros((cfg.f_pad, NH), np.float32)
    w1p[:cfg.f_in] = W1
    w1p = np.ascontiguousarray(
        w1p.reshape(KC, P, NH).transpose(1, 0, 2).reshape(P, KC * NH)
    ).astype(ml_dtypes.bfloat16)
    b1r = np.ascontiguousarray(np.broadcast_to(b1[None, :], (P, NH)))
    b4r = np.ascontiguousarray(np.broadcast_to(b4[None, :], (P, cfg.nc_out)))
    ber = np.ascontiguousarray(np.broadcast_to(beta3[None, :], (P, 1)))

    in_maps = []
    for c in range(cfg.n_cores):
        in_maps.append({
            "xt": per_core[c]["xt"].astype(ml_dtypes.bfloat16),
            "idx16": per_core[c]["idx16"],
            "mask": per_core[c]["mask"].astype(ml_dtypes.bfloat16),
            "w1p": w1p, "b1r": b1r, "w4r": np.ascontiguousarray(W4),
            "b4r": b4r, "beta3r": ber,
        })

    res = bass_utils.run_bass_kernel_spmd(
        nc, in_maps, core_ids=list(range(cfg.n_cores)), trace=trace)

    out = np.empty((cfg.n_nodes, cfg.nc_out), np.float32)
    for c in range(cfg.n_cores):
        oc = np.asarray(res.results[c]["out"])[:cfg.npc_raw]
        out[c * cfg.npc_raw + order_c[c]] = oc
    return out, res


def kernel(**inputs) -> np.ndarray:
    out, _ = run(FULL, inputs, trace=False)
    return out


# revision 66
# speedup vs baseline: 1.3938x; 1.0035x over previous
"""AGNNet (2-layer AGNN conv + linear head) distributed over 8 trn2 NeuronCores.

Strategy (graph/data parallel, per sharding hint):
  - nodes sharded by dst range: core c owns nodes [c*6250, (c+1)*6250)
  - host groups edges by dst, degree-sorts each core's local nodes (undone on
    output), drops self-loops (handled analytically in the softmax), and packs
    each 128-node tile's in-edge lists into per-tile slot columns
  - device: L1 matmul from host pre-blocked x shard; build a bf16 row table
    [h(16), inv_norm, 0, 0] per node; AllGather the packed table (2MB);
    redistribute it into SBUF (row r -> partition r%128, stripe r//128,
    15.7KB/partition); per node-tile one SBUF-source dma_gather (ant extended
    Q7 kernel, ~7.7ns/idx -- the Q7 descriptor-generation rate is THE
    bottleneck of this problem; indirect1d would cost ~11ns/edge) per index
    window fetches all neighbor rows; DVE softmax (no max-trick needed:
    |alpha| <= |beta|); the self-loop term exp(beta*cos(h,h)) is added
    analytically from inv_norm alone; second conv identical; bf16 head
    matmul + log_softmax.
  - dma_gather indices are int16, so the 50176-row table is addressed through
    THREE overlapping 32768-row windows (bases 0/8704/17408 as free-dim
    offsets into the SBUF table); the host balances each node's slots across
    windows per tile (~+7%% padding); padding slots point at the window base
    row and are killed by a -1e30 static per-slot mask added to alpha.
"""

import math
from contextlib import ExitStack
from dataclasses import dataclass

import numpy as np


@dataclass
class Cfg:
    n_cores: int = 8
    n_nodes: int = 50000
    f_in: int = 2000
    nh: int = 16
    nc_out: int = 20
    row: int = 20           # table row elems: 16 h, [16]=invn, [17]=bias, 18-19 pad
    wrow: int = 128         # wide-table row elems (256B stride for dma_gather)
    P: int = 128
    win: int = 32768        # dma_gather int16 index window (rows per base)

    @property
    def npc_raw(self) -> int:       # real nodes per core
        return self.n_nodes // self.n_cores

    @property
    def npc(self) -> int:           # padded nodes per core (mult of 128)
        return ((self.npc_raw + self.P - 1) // self.P) * self.P

    @property
    def ntiles(self) -> int:
        return self.npc // self.P

    @property
    def kchunks(self) -> int:       # 128-row chunks of the padded f_in
        return (self.f_in + self.P - 1) // self.P

    @property
    def f_pad(self) -> int:
        return self.kchunks * self.P

    @property
    def nfull(self) -> int:
        return self.n_cores * self.npc

    @property
    def wb(self) -> int:            # last window base
        return self.nfull - self.win

    @property
    def wbases(self) -> tuple:      # 3 overlapping 32768-row window bases
        return (0, self.wb // 2, self.wb)


FULL = Cfg()
NEG_BIG = -1.0e30
G_DO_GATHER = True
GRP = 4          # tiles per merged gather group


def pack_idx16(idx_mat: np.ndarray) -> np.ndarray:
    """[128 partitions, nch chunks] window-relative indices -> dma_gather
    int16 index tile [128, nch*8]: list position c*128+j lands on partition j
    chunk c (the ucode lane swizzle applies to both the index read and the
    dst partition, so it cancels); the flat list is wrapped over 16
    partitions and replicated across the 8 16-partition groups."""
    P, nch = idx_mat.shape
    assert P == 128
    flat = np.ascontiguousarray(idx_mat.T).reshape(-1).astype(np.int16)
    tile = np.empty((128, nch * 8), np.int16)
    for p in range(16):
        tile[p] = flat[p::16]
    tile[16:] = np.tile(tile[:16], (7, 1))
    return np.ascontiguousarray(tile)


def dma_gather_raw(nc, out_ap, in_ap, idxs_ap, num_idxs: int,
                   elem_size: int, stride_elems: int = 0,
                   sbuf_tokens_per_rank: int = 0,
                   sbuf_free_dim_per_rank: int = 0):
    """nc.gpsimd.dma_gather minus the elem_size%256 assert (non-transpose
    path in the Q7 ucode has no such restriction; only the DRAM stride must
    be a multiple of 256B) and minus the SBUF-source transpose-only assert
    (gen_descs handles src_is_sbuf in the plain path too). For SBUF source:
    idx i reads partition i % tokens_per_rank, free-byte offset
    (i // tokens_per_rank) * free_dim_per_rank. single_packet=False: a
    single packet is capped at 64 descriptors, large gathers hang with it."""
    from concourse import mybir
    g = nc.gpsimd
    src_is_sbuf = sbuf_tokens_per_rank > 0
    if src_is_sbuf:
        stride_bytes_256 = 0
        if in_ap.dtype != out_ap.dtype:
            in_ap = in_ap.bitcast(out_ap.dtype)
        _in_ap = [g.lower_ap(in_ap)]
    else:
        stride_bytes = stride_elems * mybir.dt.size(in_ap.dtype)
        assert stride_bytes % 256 == 0 and stride_bytes // 256 < 256
        stride_bytes_256 = stride_bytes // 256
        _in_ap = g.lower_ap_dma(in_ap, for_custom_bir_dma=True)
    _idxs_ap = g.lower_ap(idxs_ap)
    _out_ap = g.lower_ap(out_ap)
    return g.add_instruction(
        mybir.InstDMAGatherAnt(
            name=g.bass.get_next_instruction_name(),
            ins=[*_in_ap, _idxs_ap, g.lower_val_access(g.to_reg(num_idxs))],
            outs=[_out_ap],
            transpose=False,
            num_idxs=num_idxs,
            elem_size=elem_size,
            stride_bytes_256=stride_bytes_256,
            gen_mode=0,
            single_packet=False,
            queue_num=0,
            sbuf_tokens_per_rank=sbuf_tokens_per_rank,
            sbuf_free_dim_per_rank=sbuf_free_dim_per_rank,
            sbuf_free_dim_pad_per_rank=0,
            sbuf_byte_offset=0,
        ))


# --------------------------------------------------------------------------
# host-side preprocessing
# --------------------------------------------------------------------------

def preprocess(cfg: Cfg, x: np.ndarray, edge_index: np.ndarray):
    """Shard + layout transform.

    Returns (per_core list of dicts, kab, order_c) where kab is the shared
    [(K1_t, K2_t, K3_t)] per tile (same across cores for SPMD): slot columns
    per index window (3 overlapping 32768-row windows of the wide table)."""
    P, NPC, NPCR = cfg.P, cfg.npc, cfg.npc_raw
    n, C, NT = cfg.n_nodes, cfg.n_cores, cfg.ntiles
    W = cfg.win
    B1, B2, B3 = cfg.wbases

    src = edge_index[0].astype(np.int64)
    dst = edge_index[1].astype(np.int64)
    keep = src != dst                     # self-loops handled analytically
    src, dst = src[keep], dst[keep]

    core_of_dst = dst // NPCR

    # pass 1: per-core degree sort -> rank of each node within its core
    order_c, rank_c = [], []
    for c in range(C):
        m = core_of_dst == c
        ld = dst[m] - c * NPCR
        deg = np.bincount(ld, minlength=NPCR)
        order = np.argsort(deg, kind="stable")      # ascending degree
        rank = np.empty(NPCR, np.int64)
        rank[order] = np.arange(NPCR)
        order_c.append(order)
        rank_c.append(rank)

    # new global id after per-core permutation + padding (= wide-table row)
    new_gid = np.empty(n, np.int64)
    for c in range(C):
        new_gid[c * NPCR:(c + 1) * NPCR] = c * NPC + rank_c[c]

    # per-core per-node edge lists sorted by wide-row
    per_core_lists = []
    for c in range(C):
        m = core_of_dst == c
        ld = dst[m] - c * NPCR
        gs = new_gid[src[m]]
        er = rank_c[c][ld]                       # dst rank of each edge
        eo = np.lexsort((gs, er))
        er_s = er[eo]
        gs_s = gs[eo]
        starts = np.zeros(NPC + 1, np.int64)
        np.cumsum(np.bincount(er_s, minlength=NPC), out=starts[1:])
        per_core_lists.append((starts, gs_s))

    # per-node class counts: s1 [0,B2) strictly-W1; f12 [B2,B3); f123
    # [B3, W) (in all); f23 [W, B2+W); s3 [B2+W, nfull)
    bnds = np.array([B2, B3, W, B2 + W], np.int64)
    cnts = np.zeros((C, 5, NPC), np.int64)
    degp_all = np.zeros((C, NPC), np.int64)
    for c in range(C):
        starts, gs_s = per_core_lists[c]
        d_cnt = np.diff(starts)
        degp_all[c] = d_cnt
        er_of_edge = np.repeat(np.arange(NPC), d_cnt)
        cls = np.searchsorted(bnds, gs_s, side="right")
        for k in range(5):
            cnts[c, k] = np.bincount(
                er_of_edge, weights=(cls == k), minlength=NPC)

    def split_node(T, s1, f12, f123, f23, s3, deg):
        """-> (n1, n3) target-T balanced; n2 = deg - n1 - n3 (all W2-ok)."""
        n1 = np.maximum(s1, np.minimum(T, s1 + f12 + f123))
        f123_left = f123 - np.maximum(0, n1 - (s1 + f12))
        n3 = np.maximum(s3, np.minimum(T, s3 + f23 + f123_left))
        return n1, n3

    kab = []
    for t in range(NT):
        s, e = t * P, (t + 1) * P
        degs = degp_all[:, s:e]
        s1, f12, f123, f23, s3 = (cnts[:, k, s:e] for k in range(5))
        kmax = int(degs.max())
        best = None
        for T in range(kmax + 1):
            n1, n3 = split_node(T, s1, f12, f123, f23, s3, degs)
            K1 = int(n1.max())
            K3 = int(n3.max())
            K2 = int((degs - n1 - n3).max())
            if best is None or K1 + K2 + K3 < best[0]:
                best = (K1 + K2 + K3, T, K1, K2, K3)
        _, T, K1, K2, K3 = best
        kab.append((max(K1, 1), max(K2, 1), max(K3, 1), T))

    # build per-core idx16 + per-slot mask inputs (pad slots point at the
    # window base row -- a real row -- and are killed by a -1e30 mask).
    # Gathers are merged per GRP-tile group: one instruction per window,
    # slot layout [W1 t0..t3 | W2 t0..t3 | W3 t0..t3].
    per_core = []
    for c in range(C):
        starts, gs_s = per_core_lists[c]
        tiles = []
        for t in range(NT):
            K1, K2, K3, T = kab[t]
            idx1 = np.zeros((P, K1), np.int64)
            idx2 = np.zeros((P, K2), np.int64)
            idx3 = np.zeros((P, K3), np.int64)
            mask = np.full((P, K1 + K2 + K3), NEG_BIG, np.float32)
            sl = slice(t * P, (t + 1) * P)
            s1, f12, f123, f23, s3 = (cnts[c, k, sl] for k in range(5))
            degs = degp_all[c, sl]
            n1, n3 = split_node(T, s1, f12, f123, f23, s3, degs)
            n1 = np.minimum(n1, K1)
            n3 = np.minimum(n3, K3)
            for p in range(P):
                node = t * P + p
                lst = gs_s[starts[node]:starts[node + 1]]  # sorted by wrow
                a = int(n1[p]); b = int(n3[p])
                w1 = lst[:a]                       # s1 + leading f12/f123
                rest = lst[a:]
                w3 = rest[rest.size - b:] if b else rest[:0]
                w2 = rest[:rest.size - b]
                idx1[p, :a] = w1 - B1
                idx2[p, :w2.size] = w2 - B2
                idx3[p, :b] = w3 - B3
                mask[p, :a] = 0.0
                mask[p, K1:K1 + w2.size] = 0.0
                mask[p, K1 + K2:K1 + K2 + b] = 0.0
            for arr in (idx1, idx2, idx3):
                assert arr.min() >= 0 and arr.max() < W
            tiles.append((idx1, idx2, idx3, mask[:, :K1],
                          mask[:, K1:K1 + K2], mask[:, K1 + K2:]))
        packs = []
        masks = []
        for g0 in range(0, NT, GRP):
            grp = range(g0, min(g0 + GRP, NT))
            for w in range(3):
                packs.append(pack_idx16(np.concatenate(
                    [tiles[t][w] for t in grp], axis=1)))
                masks.append(np.concatenate(
                    [tiles[t][3 + w] for t in grp], axis=1))
        idx16 = np.concatenate(packs, axis=1)
        maskv = np.concatenate(masks, axis=1)

        # x shard: permuted, padded, transposed, f-padded, then pre-blocked
        # per tile as [t][p][chunk][m] so each tile's load is one contiguous
        # 4KB-per-partition DMA: xt2[t*128+p, c*128+m] = x^T[c*128+p, t*128+m]
        xs = x[c * NPCR:(c + 1) * NPCR][order_c[c]]      # [NPCR, f_in]
        xt = np.zeros((cfg.f_pad, NPC), np.float32)
        xt[:cfg.f_in, :NPCR] = xs.T
        KCh = cfg.kchunks
        xt2 = (xt.reshape(KCh, P, NT, P)
               .transpose(2, 1, 0, 3)                    # [t, p, c, m]
               .reshape(NT * P, KCh * P))
        per_core.append({"xt": np.ascontiguousarray(xt2), "idx16": idx16,
                         "mask": maskv})

    return per_core, kab, order_c


# --------------------------------------------------------------------------
# device kernel builder
# --------------------------------------------------------------------------

def build_kernel(cfg: Cfg, kab, phases: str = "ABCDE"):
    import concourse.bacc as bacc
    import concourse.tile as tile
    from concourse import bass, mybir
    from concourse.masks import make_identity

    P = cfg.P
    NH, NCO, ROW, WROW = cfg.nh, cfg.nc_out, cfg.row, cfg.wrow
    NPC, NT, KC = cfg.npc, cfg.ntiles, cfg.kchunks
    NFULL, WB, W = cfg.nfull, cfg.wb, cfg.win
    f32 = mybir.dt.float32
    bf16 = mybir.dt.bfloat16
    i16 = mybir.dt.int16
    AX = mybir.AxisListType.X
    OP = mybir.AluOpType
    AF = mybir.ActivationFunctionType
    idxw = sum(8 * (k1 + k2 + k3) for (k1, k2, k3, _) in kab)
    sumk = idxw // 8
    WBASES = cfg.wbases
    RPP = NFULL // P                # wide-table rows per partition (bounce)

    nc = bacc.Bacc("TRN2", target_bir_lowering=False, debug=False,
                   num_devices=cfg.n_cores)

    xt_d = nc.dram_tensor("xt", [NT * P, KC * P], bf16, kind="ExternalInput")
    idx_d = nc.dram_tensor("idx16", [P, idxw], i16, kind="ExternalInput")
    mask_d = nc.dram_tensor("mask", [P, sumk], bf16, kind="ExternalInput")
    w1_d = nc.dram_tensor("w1p", [P, KC * NH], bf16, kind="ExternalInput")
    b1_d = nc.dram_tensor("b1r", [P, NH], f32, kind="ExternalInput")
    w4_d = nc.dram_tensor("w4r", [NH, NCO], f32, kind="ExternalInput")
    b4_d = nc.dram_tensor("b4r", [P, NCO], f32, kind="ExternalInput")
    be_d = nc.dram_tensor("beta3r", [P, 1], f32, kind="ExternalInput")
    out_d = nc.dram_tensor("out", [NPC, NCO], f32, kind="ExternalOutput")

    tabA_l = nc.dram_tensor("tabA_l", [NPC, ROW], bf16)
    tabA_f = nc.dram_tensor("tabA_f", [NFULL, ROW], bf16, addr_space="Shared")
    tabB_l = nc.dram_tensor("tabB_l", [NPC, ROW], bf16)
    tabB_f = nc.dram_tensor("tabB_f", [NFULL, ROW], bf16, addr_space="Shared")
    tabA_w = nc.dram_tensor("tabA_w", [NFULL, WROW], bf16)  # G-phase only

    # persistent SBUF (survives across TileContexts)
    def sb(name, shape, dtype):
        return nc.alloc_sbuf_tensor(name, list(shape), dtype)

    ident_b = sb("identb", [P, P], bf16)
    w1_sb = sb("w1sb", [P, KC * NH], bf16)
    b1_sb = sb("b1sb", [P, NH], f32)
    w4_sb = sb("w4sb", [NH, NCO], f32)
    b4_sb = sb("b4sb", [P, NCO], f32)
    be_sb = sb("besb", [P, 1], f32)
    bee_sb = sb("beesb", [P, 1], f32)      # exp-ready: beta3 value
    h_sb = sb("hsb", [P, NT * NH], f32)
    rows_sb = sb("rowssb", [P, NT * ROW], bf16)
    sq_sb = sb("sqsb", [P, NT * NH], f32)
    ss_sb = sb("sssb", [P, NT], f32)
    inv_sb = sb("invsb", [P, NT], f32)
    idx_sb = sb("idxsb", [P, idxw], i16)    # all tiles' gather indices
    mask_sb = sb("masksb", [P, sumk], bf16)  # per-slot pad masks
    # SBUF-resident gather table: row r at partition r%128, stripe r//128
    table_sb = sb("tablesb", [P, RPP * ROW], bf16)

    def epilogue_rows(tab_local):
        """h_sb -> inv norm -> rows_sb -> DMA to tab_local."""
        h3v = h_sb.ap().rearrange("p (t j) -> p t j", t=NT)
        nc.vector.tensor_mul(sq_sb.ap(), h_sb.ap(), h_sb.ap())
        nc.vector.reduce_sum(
            ss_sb.ap(), sq_sb.ap().rearrange("p (t j) -> p t j", t=NT), axis=AX)
        nc.scalar.sqrt(ss_sb.ap(), ss_sb.ap())
        nc.vector.tensor_scalar_add(ss_sb.ap(), ss_sb.ap(), 1.0e-12)
        nc.vector.reciprocal(inv_sb.ap(), ss_sb.ap())
        rv = rows_sb.ap().rearrange("p (t j) -> p t j", t=NT)
        nc.vector.tensor_copy(rv[:, :, 0:16], h3v)
        nc.vector.tensor_copy(rv[:, :, 16], inv_sb.ap())
        nc.sync.dma_start(
            out=tab_local[:, :].rearrange("(t p) j -> p t j", p=P),
            in_=rv)

    def allgather(tab_local, tab_full):
        nc.gpsimd.collective_compute(
            "AllGather", OP.bypass,
            replica_groups=[list(range(cfg.n_cores))],
            ins=[tab_local.ap().opt()],
            outs=[tab_full.ap()[0:NFULL, :].opt()])

    def load_table(tab_full):
        """packed DRAM [NFULL, 20] -> SBUF table (row r at partition r%128,
        stripe r//128). SWDGE regular-pattern DMAs (~0.34ns/desc), chunked
        under the 16384-descriptor carveout limit."""
        tv = table_sb.ap().rearrange("p (r j) -> p r j", j=ROW)
        src = tab_full[:, :].rearrange("(r p) j -> p r j", p=P)
        CH = 56          # stripes per chunk: 2 chunks fit the 16k-desc ring,
        for o in range(0, RPP, CH):  # so generation overlaps drain
            n = min(CH, RPP - o)
            nc.gpsimd.dma_start(out=tv[:, o:o + n, :], in_=src[:, o:o + n, :])

    # ---------------- phase A: consts, idx preload, L1, table A, AG1 -------
    with tile.TileContext(nc) as tc:
        make_identity(nc, ident_b.ap())
        nc.gpsimd.memset(rows_sb.ap(), 0.0)
        nc.sync.dma_start(out=idx_sb.ap(), in_=idx_d[:, :])
        nc.sync.dma_start(out=mask_sb.ap(), in_=mask_d[:, :])
        nc.sync.dma_start(out=w1_sb.ap(), in_=w1_d[:, :])
        nc.sync.dma_start(out=b1_sb.ap(), in_=b1_d[:, :])
        nc.sync.dma_start(out=w4_sb.ap(), in_=w4_d[:, :])
        nc.sync.dma_start(out=b4_sb.ap(), in_=b4_d[:, :])
        nc.sync.dma_start(out=be_sb.ap(), in_=be_d[:, :])

        with tc.tile_pool(name="l1x", bufs=3) as xp, \
             tc.tile_pool(name="l1p", bufs=4, space="PSUM") as pp:
            for t in range(NT):
                xw = xp.tile([P, KC * P], bf16, tag="xw")
                nc.sync.dma_start(
                    out=xw[:], in_=xt_d[t * P:(t + 1) * P, :])
                ps = pp.tile([P, NH], f32, tag="l1ps")
                for c in range(KC):
                    nc.tensor.matmul(
                        out=ps[:], lhsT=xw[:, c * P:(c + 1) * P],
                        rhs=w1_sb.ap()[:, c * NH:(c + 1) * NH],
                        start=(c == 0), stop=(c == KC - 1))
                hsl = h_sb.ap()[:, t * NH:(t + 1) * NH]
                nc.vector.tensor_add(hsl, ps[:], b1_sb.ap())
                nc.vector.tensor_scalar_max(hsl, hsl, 0.0)
        epilogue_rows(tabA_l)
        allgather(tabA_l, tabA_f)
    with tile.TileContext(nc) as tc:
        load_table(tabA_f)

    # ---------------- conv layer -------------------------------------------
    def conv(tab_local, beta_ap):
        """beta_ap: None for beta=1 (conv2), else [P,1] AP with beta value.

        One dma_gather per (GRP-tile group, window); slot layout per group =
        [W1 t0..t3 | W2 t0..t3 | W3 t0..t3]; DVE ops run per ragged
        (tile, window) column range."""
        off = 0
        moff = 0
        with tile.TileContext(nc) as tc:
            with tc.tile_pool(name="cv", bufs=3) as cv, \
                 tc.tile_pool(name="cvs", bufs=2) as cvs:
                for g0 in range(0, NT, GRP):
                    grp = list(range(g0, min(g0 + GRP, NT)))
                    gsz = len(grp)
                    kws = [kab[t][:3] for t in grp]
                    swid = [sum(k[w] for k in kws) for w in range(3)]
                    K = sum(swid)
                    hs = cv.tile([P, K * ROW], bf16, tag="hs")
                    boff = [0, swid[0], swid[0] + swid[1]]
                    for w, base in enumerate(WBASES):
                        s0 = (base // P) * ROW
                        dma_gather_raw(
                            nc,
                            out_ap=hs[:, boff[w] * ROW:
                                      (boff[w] + swid[w]) * ROW]
                            .rearrange("p (c j) -> p c j", j=ROW),
                            in_ap=table_sb.ap()[:, s0:s0 + 256 * ROW],
                            idxs_ap=idx_sb.ap()[:, off:off + 8 * swid[w]],
                            num_idxs=P * swid[w], elem_size=ROW,
                            sbuf_tokens_per_rank=P,
                            sbuf_free_dim_per_rank=ROW * 2)
                        off += 8 * swid[w]

                    hd = cvs.tile([P, gsz * ROW], bf16, tag="hd")
                    nc.sync.dma_start(
                        out=hd[:].rearrange("p (g j) -> p g j", g=gsz),
                        in_=tab_local[g0 * P:(g0 + gsz) * P, :]
                        .rearrange("(g p) j -> p g j", p=P))
                    hd3 = hd[:].rearrange("p (g j) -> p g j", g=gsz)

                    # ragged (tile-in-group, column start, width) ranges in
                    # hs/alpha order
                    rng = []
                    for w in range(3):
                        c = boff[w]
                        for ti in range(gsz):
                            rng.append((ti, c, kws[ti][w]))
                            c += kws[ti][w]

                    invd = cvs.tile([P, gsz], f32, tag="invd")
                    if beta_ap is None:
                        nc.vector.tensor_copy(invd[:], hd3[:, :, 16])
                    else:
                        nc.vector.tensor_mul(
                            invd[:], hd3[:, :, 16],
                            beta_ap.to_broadcast([P, gsz]))

                    alpha = cv.tile([P, K], f32, tag="alpha")
                    tmp = cv.tile([P, K * NH], bf16, tag="tmp")
                    for (ti, c0, kw) in rng:
                        hsw = hs[:, c0 * ROW:(c0 + kw) * ROW].rearrange(
                            "p (k j) -> p k j", k=kw)
                        tm = tmp[:, c0 * NH:(c0 + kw) * NH].rearrange(
                            "p (k j) -> p k j", k=kw)
                        hdr = hd[:, ti * ROW:ti * ROW + 16]
                        nc.vector.tensor_mul(
                            tm, hsw[:, :, 0:16],
                            hdr[:, None, :].to_broadcast([P, kw, 16]))
                        asl = alpha[:, c0:c0 + kw]
                        nc.vector.reduce_sum(
                            asl, tmp[:, c0 * NH:(c0 + kw) * NH].rearrange(
                                "p (k j) -> p k j", j=NH), axis=AX)
                        nc.vector.tensor_mul(asl, asl, hsw[:, :, 16])
                        nc.vector.tensor_mul(
                            asl, asl,
                            invd[:, ti:ti + 1].to_broadcast([P, kw]))
                    # pad slots killed by the -1e30 static mask (group-wide)
                    nc.vector.tensor_add(
                        alpha[:], alpha[:], mask_sb.ap()[:, moff:moff + K])
                    moff += K
                    # |alpha| <= |beta| for real slots: exp without max-trick
                    e_bf = cv.tile([P, K], bf16, tag="e")
                    nc.scalar.activation(e_bf[:], alpha[:], AF.Exp)
                    # per-(tile,window) partial sums -> s [P, gsz]
                    se = cvs.tile([P, gsz * 3], f32, tag="se")
                    for i, (ti, c0, kw) in enumerate(rng):
                        w = i // gsz
                        nc.vector.reduce_sum(
                            se[:, ti * 3 + w:ti * 3 + w + 1],
                            e_bf[:, None, c0:c0 + kw], axis=AX)
                    s = cvs.tile([P, gsz], f32, tag="s")
                    nc.vector.reduce_sum(
                        s[:], se[:].rearrange("p (g w) -> p g w", g=gsz),
                        axis=AX)
                    # analytic self-loop: cos(h,h) = (1 - 1e-12*invn)^2
                    selfa = cvs.tile([P, gsz], f32, tag="selfa")
                    nc.vector.tensor_scalar(
                        selfa[:], hd3[:, :, 16], -1.0e-12, 1.0,
                        op0=OP.mult, op1=OP.add)
                    nc.vector.tensor_mul(selfa[:], selfa[:], selfa[:])
                    if beta_ap is not None:
                        nc.vector.tensor_mul(
                            selfa[:], selfa[:],
                            beta_ap.to_broadcast([P, gsz]))
                    es = cvs.tile([P, gsz], f32, tag="es")
                    nc.scalar.activation(es[:], selfa[:], AF.Exp)
                    nc.vector.tensor_add(s[:], s[:], es[:])
                    nc.vector.tensor_scalar_add(s[:], s[:], 1.0e-16)
                    r = cvs.tile([P, gsz], f32, tag="r")
                    nc.vector.reciprocal(r[:], s[:])
                    coef = cv.tile([P, K], bf16, tag="coef")
                    tmp2 = cv.tile([P, K * NH], bf16, tag="tmp2")
                    hp = cvs.tile([P, NH], f32, tag="hp")
                    for i, (ti, c0, kw) in enumerate(rng):
                        hsw = hs[:, c0 * ROW:(c0 + kw) * ROW].rearrange(
                            "p (k j) -> p k j", k=kw)
                        csl = coef[:, c0:c0 + kw]
                        nc.vector.tensor_mul(
                            csl, e_bf[:, c0:c0 + kw],
                            r[:, ti:ti + 1].to_broadcast([P, kw]))
                        t23 = tmp2[:, c0 * NH:(c0 + kw) * NH].rearrange(
                            "p (k j) -> p k j", k=kw)
                        nc.vector.tensor_mul(
                            t23, hsw[:, :, 0:16],
                            csl[:, :, None].to_broadcast([P, kw, 16]))
                        h2v = h_sb.ap()[:, grp[ti] * NH:(grp[ti] + 1) * NH]
                        if i < gsz:        # first window block for this tile
                            nc.vector.reduce_sum(
                                h2v, tmp2[:, c0 * NH:(c0 + kw) * NH]
                                .rearrange("p (k j) -> p j k", k=kw), axis=AX)
                        else:
                            nc.vector.reduce_sum(
                                hp[:], tmp2[:, c0 * NH:(c0 + kw) * NH]
                                .rearrange("p (k j) -> p j k", k=kw), axis=AX)
                            nc.vector.tensor_add(h2v, h2v, hp[:])
                    # += (exp(self)/s) * h_d  (group-wide)
                    rs = cvs.tile([P, gsz], f32, tag="rs")
                    nc.vector.tensor_mul(rs[:], es[:], r[:])
                    sh = cvs.tile([P, gsz * NH], f32, tag="sh")
                    nc.vector.tensor_mul(
                        sh[:].rearrange("p (g j) -> p g j", g=gsz),
                        hd3[:, :, 0:16],
                        rs[:][:, :, None].to_broadcast([P, gsz, NH]))
                    h2g = h_sb.ap()[:, g0 * NH:(g0 + gsz) * NH]
                    nc.vector.tensor_add(h2g, h2g, sh[:])
        assert off == idxw

    if "G" in phases:
        # debug: dma_gather NCH chunks of table rows via the ant gather and
        # dump raw rows (as f32). G_MODE: dram (256B-strided) | sbuf
        import os
        NCH = int(os.environ.get("G_NCH", "49"))
        G_NINST = int(os.environ.get("G_NINST", "1"))
        G_MODE = os.environ.get("G_MODE", "dram")
        nidx = NCH * P
        widx = nidx // 16
        idx16_d = nc.dram_tensor("idx16g", [P, widx], mybir.dt.int16,
                                 kind="ExternalInput")
        with tile.TileContext(nc) as tc:
            if G_MODE == "dram":
                nc.sync.dma_start(out=tabA_w[:, 0:ROW], in_=tabA_f[:, :])
            else:
                # sbuf layout: row r -> partition r%128, stripe r//128
                load_table(tabA_f)
            with tc.tile_pool(name="dbg", bufs=1) as dbg:
                idx_sbg = dbg.tile([P, widx], mybir.dt.int16, tag="idx16")
                nc.sync.dma_start(out=idx_sbg[:], in_=idx16_d[:, :])
                hs = dbg.tile([P, NCH * ROW], bf16, tag="hs")
                nc.gpsimd.memset(hs[:], 0.0)
                if G_DO_GATHER:
                    for _ in range(G_NINST):
                        if G_MODE == "dram":
                            dma_gather_raw(
                                nc,
                                out_ap=hs[:].rearrange(
                                    "p (c j) -> p c j", j=ROW),
                                in_ap=tabA_w[:, 0:ROW], idxs_ap=idx_sbg[:],
                                num_idxs=nidx, elem_size=ROW,
                                stride_elems=128)
                        else:
                            dma_gather_raw(
                                nc,
                                out_ap=hs[:].rearrange(
                                    "p (c j) -> p c j", j=ROW),
                                in_ap=table_sb.ap()[:, 0:RPP * ROW],
                                idxs_ap=idx_sbg[:],
                                num_idxs=nidx, elem_size=ROW,
                                sbuf_tokens_per_rank=P,
                                sbuf_free_dim_per_rank=ROW * 2)
                ncols = min(NCH * ROW, 980)
                hf = dbg.tile([P, ncols], f32, tag="hf")
                nc.vector.tensor_copy(hf[:], hs[:, 0:ncols])
                ov = out_d.ap().rearrange("(p q) j -> p (q j)", p=P)
                nc.sync.dma_start(out=ov[:, 0:ncols], in_=hf[:])
        nc.compile()
        return nc

    if "B" in phases:
        conv(tabA_l, None)

    # ---------------- phase C: table B + AG2 -------------------------------
    if "C" in phases:
        with tile.TileContext(nc) as tc:
            epilogue_rows(tabB_l)
            allgather(tabB_l, tabB_f)
        with tile.TileContext(nc) as tc:
            load_table(tabB_f)

    if "D" in phases:
        conv(tabB_l, be_sb.ap()[:, 0:1])

    if "E" not in phases:
        # debug: dump h_sb (and inv_sb) into out
        with tile.TileContext(nc) as tc:
            ov = out_d.ap().rearrange("(p q) j -> p (q j)", p=P)
            nc.sync.dma_start(out=ov[:, 0:NT * NH], in_=h_sb.ap())
            nc.sync.dma_start(out=ov[:, NT * NH:NT * NH + NT], in_=inv_sb.ap())
        nc.compile()
        return nc

    with tile.TileContext(nc) as tc:
        with tc.tile_pool(name="hd", bufs=1) as hp, \
             tc.tile_pool(name="hdp", bufs=4, space="PSUM") as hpp:
            hb = hp.tile([P, NT * NH], bf16, tag="hb")
            nc.vector.tensor_copy(hb[:], h_sb.ap())
            h3t = hp.tile([NH, NT * P], bf16, tag="h3t")
            for t in range(NT):
                pst = hpp.tile([NH, P], bf16, tag="pst")
                nc.tensor.transpose(
                    out=pst[:], in_=hb[:, t * NH:(t + 1) * NH],
                    identity=ident_b.ap())
                nc.vector.tensor_copy(h3t[:, t * P:(t + 1) * P], pst[:])
            w4b = hp.tile([NH, NCO], bf16, tag="w4b")
            nc.vector.tensor_copy(w4b[:], w4_sb.ap())
            lg = hp.tile([P, NT * NCO], f32, tag="lg")
            for t in range(NT):
                psl = hpp.tile([P, NCO], f32, tag="psl")
                nc.tensor.matmul(
                    out=psl[:], lhsT=h3t[:, t * P:(t + 1) * P], rhs=w4b[:],
                    start=True, stop=True)
                nc.vector.tensor_add(
                    lg[:, t * NCO:(t + 1) * NCO], psl[:], b4_sb.ap())
            lg3 = lg[:].rearrange("p (t j) -> p t j", t=NT)
            nm = hp.tile([P, NT], f32, tag="hnm")
            nc.vector.reduce_max(nm[:], lg3, axis=AX, negate=True)
            nc.vector.tensor_add(
                lg3, lg3, nm[:][:, :, None].to_broadcast([P, NT, NCO]))
            ex = hp.tile([P, NT * NCO], f32, tag="ex")
            nc.scalar.activation(ex[:], lg[:], AF.Exp)
            s = hp.tile([P, NT], f32, tag="hs_sum")
            nc.vector.reduce_sum(
                s[:], ex[:].rearrange("p (t j) -> p t j", t=NT), axis=AX)
            ls = hp.tile([P, NT], f32, tag="ls")
            nc.scalar.activation(ls[:], s[:], AF.Ln)
            nc.vector.tensor_sub(
                lg3, lg3, ls[:][:, :, None].to_broadcast([P, NT, NCO]))
            # partition-major contiguous store (128 x 3.9KB descriptors vs
            # 6272 strided 80B RMW writes); host undoes the layout
            nc.sync.dma_start(
                out=out_d[:, :].rearrange("(p q) j -> p (q j)", p=P),
                in_=lg[:])

    nc.compile()
    return nc


# --------------------------------------------------------------------------
# entry point
# --------------------------------------------------------------------------

def run(cfg: Cfg, inputs: dict, trace: bool = False):
    from concourse import bass_utils

    x = np.asarray(inputs["x"], np.float32)
    edge_index = np.asarray(inputs["edge_index"])
    W1 = np.asarray(inputs["W1"], np.float32)
    b1 = np.asarray(inputs["b1"], np.float32)
    W4 = np.asarray(inputs["W4"], np.float32)
    b4 = np.asarray(inputs["b4"], np.float32)
    beta3 = np.asarray(inputs["beta3"], np.float32)

    import ml_dtypes

    per_core, kab, order_c = preprocess(cfg, x, edge_index)
    nc = build_kernel(cfg, kab)

    P, KC, NH = cfg.P, cfg.kchunks, cfg.nh
    w1p = np.zeros((cfg.f_pad, NH), np.float32)
    w1p[:cfg.f_in] = W1
    w1p = np.ascontiguousarray(
        w1p.reshape(KC, P, NH).transpose(1, 0, 2).reshape(P, KC * NH)
    ).astype(ml_dtypes.bfloat16)
    b1r = np.ascontiguousarray(np.broadcast_to(b1[None, :], (P, NH)))
    b4r = np.ascontiguousarray(np.broadcast_to(b4[None, :], (P, cfg.nc_out)))
    ber = np.ascontiguousarray(np.broadcast_to(beta3[None, :], (P, 1)))

    in_maps = []
    for c in range(cfg.n_cores):
        in_maps.append({
            "xt": per_core[c]["xt"].astype(ml_dtypes.bfloat16),
            "idx16": per_core[c]["idx16"],
            "mask": per_core[c]["mask"].astype(ml_dtypes.bfloat16),
            "w1p": w1p, "b1r": b1r, "w4r": np.ascontiguousarray(W4),
            "b4r": b4r, "beta3r": ber,
        })

    res = bass_utils.run_bass_kernel_spmd(
        nc, in_maps, core_ids=list(range(cfg.n_cores)), trace=trace)

    out = np.empty((cfg.n_nodes, cfg.nc_out), np.float32)
    NTl, Pl = cfg.ntiles, cfg.P
    for c in range(cfg.n_cores):
        arr = np.asarray(res.results[c]["out"])
        # undo the partition-major store: rank t*128+p is at row p*NT+t
        oc = (arr.reshape(Pl, NTl, cfg.nc_out).transpose(1, 0, 2)
              .reshape(cfg.npc, cfg.nc_out))[:cfg.npc_raw]
        out[c * cfg.npc_raw + order_c[c]] = oc
    return out, res


def kernel(**inputs) -> np.ndarray:
    out, _ = run(FULL, inputs, trace=False)
    return out
